# revision 39
# baseline (speedup 1.0000x reference)
"""NetTGCN forward pass on 8 Trainium2 NeuronCores (Bass/Tile).

Key structure (v2):
  real(FFT) rank-16 fold: real(FFT(x, t)) = x @ Ccos with rank(Ccos)=16
  (cos(2pi t f/30) columns f and 30-f coincide), so x is host-folded to
  x~ = x @ U [B, N0, 16] and W1~[k] = V @ W1[k]; the layer-1 Chebyshev
  recurrence runs on 16 taps instead of 32 - half the matmul work.

  Layer 1 (4096-node graph): 4-way node-shard x 2-way batch-shard.
  M = 4*A^2 even/odd chains as before, but the per-step AllGather is
  replaced by direct SBUF->SBUF remote_dma_broadcast pushes into the
  peers' gather buffers (XOR-distance slots), signalled by per-peer
  arrival semaphores. Buffer reuse is safe without credits because the
  recurrence dataflow implies peers consumed parity p before the next
  write to p can be produced. Scheduling-sim deadlock is avoided by
  emitting arrival waits as >=0 and patching the real thresholds after
  Tile scheduling.

  Transition/layer 2/head: identical to the baseline (AllToAll to
  batch-parallel layer 2, fc1 sharded over contraction + AllReduce).
"""

import sys

if "/opt/trn_rl_repo" not in sys.path:
    sys.path.insert(0, "/opt/trn_rl_repo")

import numpy as np
import ml_dtypes

import concourse.bacc as bacc
import concourse.mybir as mybir
import concourse.bass_utils as _bu
from concourse.bass_utils import run_bass_kernel_spmd
from concourse.tile import TileContext
from concourse.tile_rust import add_dep_helper
from concourse.masks import make_identity

_bu.upload_artifacts = lambda tmpdir: f"file://{tmpdir}"  # no bucket in sandbox

F32 = mybir.dt.float32
BF16 = mybir.dt.bfloat16
AX = mybir.AxisListType
ALU = mybir.AluOpType
ACT = mybir.ActivationFunctionType

B, N0, T, K = 32, 4096, 30, 25
G1, G2, D, C = 32, 64, 512, 10
N2 = N0 // 4
NCORES = 8
NB = 4                 # layer-1 node shards
BL = B // 2            # 16 batches per layer-1 batch-half
TF = 16                # folded taps (rank of Ccos)
C1 = BL * TF           # 256 layer-1 channels per core
NBLK = N0 // NB        # 1024 nodes per layer-1 shard
NTL = NBLK // 128      # 8 state tiles
NT0 = N0 // 128        # 32 gathered-node tiles
P2BLK = N2 // NB       # 256 pooled nodes per layer-1 shard
B2 = 4                 # batches per layer-2 core
C2 = B2 * G1           # 128 layer-2 channels
FBLK = (N2 * G2) // NCORES  # 8192 fc1 contraction rows per core

G4 = [[0, 1, 2, 3], [4, 5, 6, 7]]
G8 = [list(range(NCORES))]


def _b16(a):
    return np.ascontiguousarray(a.astype(ml_dtypes.bfloat16))


def _dense_adj(edge_index, n):
    row = edge_index[0].astype(np.int64)
    col = edge_index[1].astype(np.int64)
    deg = np.zeros(n, np.float32)
    np.add.at(deg, row, 1.0)
    dis = np.where(deg > 0, 1.0 / np.sqrt(np.maximum(deg, 1.0)), 0.0).astype(np.float32)
    w = (-dis[row] * dis[col]).astype(np.float32)
    a = np.zeros((n, n), np.float32)
    np.add.at(a, (row, col), w)
    return a


def _fold_uv():
    """Ccos = U @ V with U [30,16], V [16,30]."""
    t = np.arange(T)
    U = np.cos(2.0 * np.pi * np.outer(t, np.arange(TF)) / T).astype(np.float32)
    Vm = np.zeros((TF, T), np.float32)
    for j in range(TF):
        Vm[j, j] = 1.0
        if 0 < j < TF - 1:
            Vm[j, T - j] += 1.0
    return U, Vm


def build_program(dbg=False):
    nc = bacc.Bacc("TRN2", target_bir_lowering=False, debug=False,
                   num_devices=NCORES)

    a1t_in = nc.dram_tensor("a1t", [N0, NBLK], BF16, kind="ExternalInput")
    m1t_in = nc.dram_tensor("m1t", [N0, NBLK], BF16, kind="ExternalInput")
    a2t_in = nc.dram_tensor("a2t", [N2, N2], BF16, kind="ExternalInput")
    x_nm_in = nc.dram_tensor("x_nm", [N0, C1], BF16, kind="ExternalInput")
    w1_in = nc.dram_tensor("w1a", [128, K * 2 * 128], BF16, kind="ExternalInput")
    w2_in = nc.dram_tensor("w2a", [128, K * 2 * 128], BF16, kind="ExternalInput")
    b1_in = nc.dram_tensor("b1v", [128, 1], F32, kind="ExternalInput")
    b2_in = nc.dram_tensor("b2v", [128, 2], F32, kind="ExternalInput")
    fc1w_in = nc.dram_tensor("fc1w", [FBLK, D], BF16, kind="ExternalInput")
    fc1b_in = nc.dram_tensor("fc1b", [B, D], F32, kind="ExternalInput")
    fc2w_in = nc.dram_tensor("fc2w", [D, C], BF16, kind="ExternalInput")
    fc2b_in = nc.dram_tensor("fc2b", [B, C], F32, kind="ExternalInput")

    out_t = nc.dram_tensor("out", [B, C], F32, kind="ExternalOutput")
    if dbg:
        h1_dbg = nc.dram_tensor("h1_dbg", [512, NBLK], F32, kind="ExternalOutput")
        l2i_dbg = nc.dram_tensor("l2i_dbg", [N2, C2], F32, kind="ExternalOutput")
        h2_dbg = nc.dram_tensor("h2_dbg", [256, N2], F32, kind="ExternalOutput")
        z_dbg = nc.dram_tensor("z_dbg", [B, D], F32, kind="ExternalOutput")

    ccp_in = nc.dram_tensor("ccp_in", [NCORES * P2BLK, 2 * G1], BF16)
    ccp_out = nc.dram_tensor("ccp_out", [NCORES * P2BLK, 2 * G1], BF16)
    cch_in = nc.dram_tensor("cch_in", [N2 * G2, B2], BF16)
    cch_out = nc.dram_tensor("cch_out", [N2 * G2, B2], BF16)
    ccz_in = nc.dram_tensor("ccz_in", [B, D], F32)
    ccz_out = nc.dram_tensor("ccz_out", [B, D], F32, addr_space="Shared")

    # arrival semaphores: rsem[d-1] counts pushes from the peer at
    # XOR-distance d (+2 per 8-slot broadcast arrival, FIFO per peer).
    rsem = [nc.alloc_semaphore(f"rsem{d}") for d in (1, 2, 3)]
    lsem = nc.alloc_semaphore("lsem")
    patches = []

    with TileContext(nc) as tc:
        with tc.tile_pool(name="const", bufs=1) as cpool:
            ident = cpool.tile([128, 128], F32)
            make_identity(nc, ident[:])
            identb = cpool.tile([128, 128], BF16)
            nc.vector.tensor_copy(identb[:], ident[:])
            # (-2I), (-1I), (-3I) in bf16: Chebyshev corrections run on the
            # PE as extra contraction tiles (exact small-int coefficients).
            nid2 = cpool.tile([128, 128], BF16)
            nc.vector.tensor_scalar_mul(nid2[:], identb[:], -2.0)
            nid1 = cpool.tile([128, 128], BF16)
            nc.vector.tensor_scalar_mul(nid1[:], identb[:], -1.0)
            nid3 = cpool.tile([128, 128], BF16)
            nc.vector.tensor_scalar_mul(nid3[:], identb[:], -3.0)

            # NOTE: no manual sem_clear here - the preamble's per-kernel
            # sem_clear zeroes all Bass-managed sems BEFORE the prelude
            # AllGather, so peer pushes can never race a clear.
            bar = nc.gpsimd.bir_kernel_barrier_wait(replica_groups=G8)
            bar_wait = bar.ins.sync_info.on_wait[0]
            patches.append((bar_wait, bar_wait.wait_value))
            bar_wait.wait_value = 0

            # ======================= LAYER 1 =======================
            with tc.tile_pool(name="l1", bufs=1) as l1, \
                 tc.tile_pool(name="l1g", bufs=6) as l1g, \
                 tc.tile_pool(name="l1a", bufs=2) as l1a, \
                 tc.tile_pool(name="l1cm", bufs=1) as l1cm, \
                 tc.tile_pool(name="ps_y", bufs=1, space="PSUM") as ps_y, \
                 tc.tile_pool(name="ps_tr", bufs=2, space="PSUM") as ps_tr, \
                 tc.tile_pool(name="ps_ct", bufs=2, space="PSUM") as ps_ct:

                m1t = l1.tile([128, NT0, NBLK], BF16)
                w1a = l1.tile([128, K, 2, 128], BF16)
                nc.sync.dma_start(w1a[:], w1_in.ap().rearrange("p (k h c) -> p k h c", k=K, h=2))
                h1_sb = l1.tile([128, 4, NBLK], F32)
                nc.any.memset(h1_sb[:], 0.0)

                # gather buffers: [chain][parity] -> [128, 4 slots, 8 nt, C1]
                # slot 0 = own block (local bf16 copy), slot d = XOR-peer d.
                gb = [[l1.tile([128, NB, NTL, C1], BF16, tag=f"gb{c}{q}",
                               name=f"gb{c}{q}")
                       for q in range(2)] for c in range(2)]

                # own x~ block, bf16 (slot-0 image of x_nm)
                tx0 = l1.tile([128, NTL, C1], BF16)
                nc.sync.dma_start(
                    tx0[:],
                    x_nm_in.ap().rearrange("(kt p) c -> p kt c", p=128)[:, 0:NTL])

                def l1_contract(src, kk):
                    # src: [128, NTL, C1] bf16 state; DMA-transpose (XBAR)
                    # builds cm off the PE/DVE entirely.
                    cm = l1cm.tile([128, 2, NBLK], BF16, tag="cm", name=f"cm{kk}")
                    for cc in range(2):
                        for nt in range(NTL):
                            nc.sync.dma_start_transpose(
                                cm[:, cc, 128 * nt:128 * (nt + 1)],
                                src[:, nt, 128 * cc:128 * (cc + 1)])
                    for cc in range(2):
                        for h in range(2):
                            for ch in range(2):
                                cps = ps_ct.tile([128, 512], F32, tag="ct",
                                                 name=f"ct{kk}_{cc}_{h}_{ch}")
                                nc.tensor.matmul(
                                    cps[:], w1a[:, kk, h, :],
                                    cm[:, cc, 512 * ch:512 * (ch + 1)],
                                    start=True, stop=True)
                                nc.vector.tensor_tensor(
                                    h1_sb[:, 2 * cc + h, 512 * ch:512 * (ch + 1)],
                                    h1_sb[:, 2 * cc + h, 512 * ch:512 * (ch + 1)],
                                    cps[:], ALU.add)

                l1_contract(tx0, 0)

                for k in range(1, K):
                    cq = (k % 2, (k // 2) % 2)
                    gdst = gb[cq[0]][cq[1]]

                    # Chebyshev corrections (-2tx_{k-2}, -tx_{k-4}, -3tx_1,
                    # -x~) are folded into the PE accumulation groups as
                    # identity matmuls; the recurrence is one PSUM->bf16
                    # copy into the gather buffer's own slot.
                    def do_recur(ot, yap, k=k, gdst=gdst):
                        if k <= 2:
                            nc.vector.tensor_scalar_mul(
                                gdst[:, 0, ot, :], yap, 0.5)
                        else:
                            nc.vector.tensor_copy(gdst[:, 0, ot, :], yap)

                    if k == 2:
                        # m1t is first needed here; deferring + chunking the
                        # 8MB load keeps the k=1 streams off the DMA queues.
                        m1v = m1t_in.ap().rearrange("(t p) n -> p t n", p=128)
                        for mc in range(4):
                            nc.sync.dma_start(
                                m1t[:, 8 * mc:8 * (mc + 1), :],
                                m1v[:, 8 * mc:8 * (mc + 1), :])
                    if k <= 2:
                        # streamed rhs (x~): kt-outer needs bank-aligned
                        # accumulation groups -> two half-passes of 4 out
                        # tiles padded to one bank each.
                        for oh in range(2):
                            yp4 = ps_y.tile([128, 4, 512], F32, tag="y",
                                            name=f"y{k}_{oh}")
                            for kt in range(NT0):
                                rhs = l1g.tile([128, C1], BF16, tag="gkt",
                                               name=f"g{k}_{oh}_{kt}")
                                nc.sync.dma_start(
                                    rhs[:],
                                    x_nm_in.ap().rearrange(
                                        "(t p) c -> t p c", p=128)[kt])
                                if k == 1:
                                    op = l1a.tile([128, NBLK], BF16, tag="aop",
                                                  name=f"a{oh}_{kt}")
                                    nc.sync.dma_start(
                                        op[:], a1t_in.ap().rearrange(
                                            "(t p) n -> t p n", p=128)[kt])
                                    opv = op[:]
                                else:
                                    opv = m1t[:, kt, :]
                                for j in range(4):
                                    oi = 4 * oh + j
                                    last = (kt == NT0 - 1) and k == 1
                                    nc.tensor.matmul(
                                        yp4[:, j, 0:C1],
                                        opv[:, 128 * oi:128 * (oi + 1)],
                                        rhs[:],
                                        start=(kt == 0), stop=last)
                            for j in range(4):
                                oi = 4 * oh + j
                                if k == 2:  # tx_2 = 0.5(M x~ - 2 x~)
                                    nc.tensor.matmul(
                                        yp4[:, j, 0:C1], nid2[:],
                                        tx0[:, oi, :],
                                        start=False, stop=True)
                                do_recur(oi, yp4[:, j, 0:C1])
                    else:
                        # SBUF gather source: oi-outer so each PSUM
                        # accumulation group completes before the next
                        # starts (groups share banks at 1KB offsets).
                        gsrc = gb[(k - 2) % 2][((k - 2) // 2) % 2]
                        yp = ps_y.tile([128, NTL, C1], F32, tag="y",
                                       name=f"y{k}")
                        kwaits = []
                        for oi in range(NTL):
                            for kt in range(NT0):
                                mm = nc.tensor.matmul(
                                    yp[:, oi, :],
                                    m1t[:, kt, 128 * oi:128 * (oi + 1)],
                                    gsrc[:, kt // NTL, kt % NTL, :],
                                    start=(kt == 0), stop=False)
                                if oi == 0 and kt == NTL - 1:
                                    # arrival waits anchored after the
                                    # own-slot tiles of the first group so
                                    # the scheduler cannot hoist them ahead
                                    # of the sends peers depend on.
                                    for d in (1, 2, 3):
                                        w = nc.tensor.wait_ge(rsem[d - 1], 0)
                                        patches.append(
                                            (w.ins.sync_info.on_wait[0],
                                             2 * (k - 2)))
                                        add_dep_helper(
                                            w.ins, mm.ins,
                                            reason="wait after own tiles")
                                        kwaits.append(w)
                                if kt % NTL == 0 and kt > 0:
                                    add_dep_helper(
                                        mm.ins, kwaits[kt // NTL - 1].ins,
                                        reason="gather arrival")
                            # fold the Chebyshev corrections into the group
                            if k == 3:  # tx_3 = M tx_1 - 3 tx_1
                                nc.tensor.matmul(
                                    yp[:, oi, :], nid3[:],
                                    gsrc[:, 0, oi, :],
                                    start=False, stop=True)
                            else:       # tx_k = M tx_{k-2} -2tx_{k-2} -tx_{k-4}
                                nc.tensor.matmul(
                                    yp[:, oi, :], nid2[:],
                                    gsrc[:, 0, oi, :],
                                    start=False, stop=False)
                                p4 = (tx0[:, oi, :] if k == 4
                                      else gdst[:, 0, oi, :])
                                nc.tensor.matmul(
                                    yp[:, oi, :], nid1[:], p4,
                                    start=False, stop=True)
                        # copies batched after all groups: the PE runs the 8
                        # groups back-to-back without PSUM bank locks against
                        # the DVE reads.
                        for oi in range(NTL):
                            do_recur(oi, yp[:, oi, :])

                    # push own block to the 3 XOR-peers' matching slots
                    if k <= K - 3:
                        for d in (1, 2, 3):
                            rd = [None] * 8
                            rd[d] = (0, d)
                            prep = nc.gpsimd.remote_dma_broadcast(
                                gdst[:, d, :, :], gdst[:, 0, :, :],
                                remote_sem=rsem[d - 1], local_sem=lsem,
                                rdests=rd)
                            add_dep_helper(prep.ins, bar.ins,
                                           reason="send after barrier")
                        nc.gpsimd.trigger_dma(count=None)

                    l1_contract(gdst[:, 0, :, :], k)

                # bias + relu + maxpool4 along nodes
                b1v = l1.tile([128, 1], F32)
                nc.sync.dma_start(b1v[:], b1_in.ap())
                h1p = l1.tile([128, 4, P2BLK], F32)
                for cht in range(4):
                    nc.scalar.activation(h1_sb[:, cht, :], h1_sb[:, cht, :], ACT.Relu,
                                         bias=b1v[:])
                    h4 = h1_sb[:, cht, :].rearrange("p (n f) -> p n f", f=4)
                    nc.vector.tensor_tensor(h1p[:, cht, :], h4[:, :, 0], h4[:, :, 1],
                                            ALU.max)
                    nc.vector.tensor_tensor(h1p[:, cht, :], h1p[:, cht, :], h4[:, :, 2],
                                            ALU.max)
                    nc.vector.tensor_tensor(h1p[:, cht, :], h1p[:, cht, :], h4[:, :, 3],
                                            ALU.max)
                if dbg:
                    nc.sync.dma_start(
                        h1_dbg.ap().rearrange("(t p) n -> p t n", p=128), h1_sb[:])

                # transpose pooled block -> [n2_local, (b_loc, g)] bf16
                h1pt = l1.tile([128, P2BLK // 128, BL * G1], BF16)
                for cht in range(4):
                    for nt in range(P2BLK // 128):
                        trp = ps_tr.tile([128, 128], F32, tag="tr")
                        nc.tensor.transpose(
                            trp[:], h1p[:, cht, 128 * nt:128 * (nt + 1)], ident[:])
                        nc.any.tensor_copy(
                            out=h1pt[:, nt, 128 * cht:128 * (cht + 1)], in_=trp[:])

                ccp_iv = ccp_in.ap().rearrange("(s t p) c -> s p t c", p=128,
                                               t=P2BLK // 128)
                for s in range(NCORES):
                    nc.sync.dma_start(ccp_iv[s],
                                      h1pt[:, :, 64 * s:64 * (s + 1)])
                nc.gpsimd.collective_compute(
                    "AllToAll", ALU.bypass, replica_groups=G8,
                    ins=[ccp_in.ap()], outs=[ccp_out.ap()])

            # ======================= LAYER 2 =======================
            # ccp_out rows: src_rank * P2BLK + n2l, src_rank = bh*4 + nb;
            # cols: (b_pair 2, g 32). My batches (c2 order): b = bh*2 + pair.
            with tc.tile_pool(name="l2", bufs=1) as l2, \
                 tc.tile_pool(name="l2st", bufs=3) as l2st, \
                 tc.tile_pool(name="l2bf", bufs=2) as l2bf, \
                 tc.tile_pool(name="l2cm", bufs=2) as l2cm, \
                 tc.tile_pool(name="ps2_y", bufs=2, space="PSUM") as ps2_y, \
                 tc.tile_pool(name="ps2_tr", bufs=2, space="PSUM") as ps2_tr, \
                 tc.tile_pool(name="ps2_ct", bufs=2, space="PSUM") as ps2_ct:

                a2t = l2.tile([128, N2 // 128, N2], BF16)
                nc.sync.dma_start(a2t[:], a2t_in.ap().rearrange("(t p) n -> p t n", p=128))
                w2a = l2.tile([128, K, 2, 128], BF16)
                nc.sync.dma_start(
                    w2a[:], w2_in.ap().rearrange("p (k h g) -> p k h g", k=K, h=2))

                # init state: [128 n2, 8 nt, (b 4, g 32)] from ccp_out
                st0_bf = l2bf.tile([128, N2 // 128, C2], BF16, tag="st2bf")
                ccp_v = ccp_out.ap().rearrange(
                    "(bh nb t p) c -> bh nb p t c", bh=2, nb=NB, t=P2BLK // 128)
                for bh in range(2):
                    for nb in range(NB):
                        # dest cols [bh*64, +64) = (b = bh*2 + pair, g)
                        nc.sync.dma_start(
                            st0_bf[:, 2 * nb:2 * (nb + 1),
                                   64 * bh:64 * (bh + 1)],
                            ccp_v[bh, nb])
                st0 = l2st.tile([128, N2 // 128, C2], F32, tag="st2")
                nc.vector.tensor_copy(st0[:], st0_bf[:])
                if dbg:
                    nc.sync.dma_start(
                        l2i_dbg.ap().rearrange("(t p) c -> p t c", p=128), st0[:])

                h2a = l2.tile([128, 2, N2], F32)
                nc.any.memset(h2a[:], 0.0)

                def l2_contract(src_f32, kk):
                    cm = l2cm.tile([128, N2], BF16, tag="cm2")
                    for nt in range(N2 // 128):
                        trp = ps2_tr.tile([128, 128], F32, tag="tr2")
                        nc.tensor.transpose(trp[:], src_f32[:, nt, :], ident[:])
                        nc.any.tensor_copy(
                            out=cm[:, 128 * nt:128 * (nt + 1)], in_=trp[:])
                    for hh in range(2):
                        cps = ps2_ct.tile([128, N2], F32, tag="ct2")
                        for ch in range(N2 // 512):
                            nc.tensor.matmul(
                                cps[:, 512 * ch:512 * (ch + 1)],
                                w2a[:, kk, hh, :],
                                cm[:, 512 * ch:512 * (ch + 1)],
                                start=True, stop=True)
                        nc.vector.tensor_tensor(h2a[:, hh, :], h2a[:, hh, :],
                                                cps[:], ALU.add)

                l2_contract(st0, 0)
                tx2_pp = None
                tx2_prev = st0
                gath2 = st0_bf
                for k in range(1, K):
                    yps = []
                    for g in range(2):
                        yp = ps2_y.tile([128, 4, 128], F32, tag="y2")
                        yps.append(yp)
                        for oi in range(4):
                            ot = 4 * g + oi
                            for kt in range(N2 // 128):
                                nc.tensor.matmul(
                                    yp[:, oi, :],
                                    a2t[:, kt, 128 * ot:128 * (ot + 1)],
                                    gath2[:, kt, :],
                                    start=(kt == 0), stop=(kt == N2 // 128 - 1))
                    tx2_new = l2st.tile([128, N2 // 128, C2], F32, tag="st2")
                    g2bf = l2bf.tile([128, N2 // 128, C2], BF16, tag="st2bf")
                    for ot in range(8):
                        yap = yps[ot // 4][:, ot % 4, :]
                        if k == 1:
                            nc.vector.tensor_scalar_mul(tx2_new[:, ot, :], yap, 0.5)
                        else:
                            nc.vector.tensor_tensor(tx2_new[:, ot, :], yap,
                                                    tx2_pp[:, ot, :], ALU.subtract)
                        nc.vector.tensor_copy(g2bf[:, ot, :], tx2_new[:, ot, :])
                    l2_contract(tx2_new, k)
                    gath2 = g2bf
                    tx2_pp = tx2_prev
                    tx2_prev = tx2_new

                # bias + relu, then transpose h2 -> [n2, (b, g2)] bf16
                b2v = l2.tile([128, 2], F32)
                nc.sync.dma_start(b2v[:], b2_in.ap())
                h2r = l2.tile([128, 2, N2], F32)
                for hh in range(2):
                    nc.scalar.activation(h2r[:, hh, :], h2a[:, hh, :], ACT.Relu,
                                         bias=b2v[:, hh:hh + 1])
                if dbg:
                    nc.sync.dma_start(
                        h2_dbg.ap().rearrange("(t p) n -> p t n", p=128), h2r[:])
                # build f-major features: ft_sb[n2_l, nt, (g2 64, b 4)]
                ft_sb = l2.tile([128, N2 // 128, G2 * B2], BF16)
                for hh in range(2):
                    for nt in range(N2 // 128):
                        trp = ps2_tr.tile([128, 128], F32, tag="tr2")
                        nc.tensor.transpose(trp[:], h2r[:, hh, 128 * nt:128 * (nt + 1)],
                                            ident[:])
                        # cols of trp: (b 4, g2r 32) -> dest (g2 = hh*32+g2r, b)
                        nc.any.tensor_copy(
                            out=ft_sb[:, nt, :].rearrange("p (g b) -> p g b", g=G2)[
                                :, 32 * hh:32 * (hh + 1), :],
                            in_=trp[:].rearrange("p (b g) -> p g b", b=4))
                # AllToAll: slot j = my rows f in [FBLK*j, FBLK*(j+1))
                # cch_in rows (j, n2_l 128, g2 64), cols b
                nc.sync.dma_start(
                    cch_in.ap().rearrange("(j nl g) b -> nl j (g b)",
                                          j=NCORES, nl=128),
                    ft_sb[:])
                nc.gpsimd.collective_compute(
                    "AllToAll", ALU.bypass, replica_groups=G8,
                    ins=[cch_in.ap()], outs=[cch_out.ap()])

            # ======================= HEAD =======================
            with tc.tile_pool(name="fc", bufs=1) as fc, \
                 tc.tile_pool(name="fcw", bufs=4) as fcw, \
                 tc.tile_pool(name="ps3", bufs=2, space="PSUM") as ps3, \
                 tc.tile_pool(name="ps3z", bufs=1, space="PSUM") as ps3z:

                # flatT: [128 p, 8 r, 64 kt, 4 b]; cch rows (r, p, kt) so each
                # per-r DMA is 512B-contiguous per partition. flt[p, r, kt, b]
                # holds flat[b(r,b), f = p*64 + kt]; fc1w is host-permuted to
                # match (row kt*128+p = local f p*64+kt).
                NKT = FBLK // 128
                flt = fc.tile([128, NCORES, NKT, B2], BF16, tag="flt")
                flt2 = fc.tile([128, NKT, NCORES, B2], BF16, tag="flt2")
                zps = ps3z.tile([32, D], F32)
                cch_v = cch_out.ap().rearrange(
                    "(r p kt) b -> r p kt b", r=NCORES, p=128)
                for r in range(NCORES):
                    nc.sync.dma_start(flt[:, r, :, :], cch_v[r])
                nc.vector.tensor_copy(
                    flt2[:], flt[:].rearrange("p r kt b -> p kt r b"))
                fc1v = fc1w_in.ap().rearrange("(c kt p) d -> c p kt d",
                                              p=128, kt=8)
                for cb in range(NKT // 8):
                    fw = fcw.tile([128, 8, D], BF16, tag="fw")
                    nc.sync.dma_start(fw[:], fc1v[cb])
                    for j in range(8):
                        kt = 8 * cb + j
                        nc.tensor.matmul(
                            zps[:],
                            flt2[:, kt, :, :].rearrange("p r b -> p (r b)"),
                            fw[:, j, :],
                            start=(kt == 0), stop=(kt == NKT - 1))
                zblk = fc.tile([32, D], F32)
                nc.vector.tensor_copy(zblk[:], zps[:])
                nc.sync.dma_start(ccz_in.ap(), zblk[:])
                nc.gpsimd.collective_compute(
                    "AllReduce", ALU.add, replica_groups=G8,
                    ins=[ccz_in.ap()], outs=[ccz_out.ap()])
                zfull = fc.tile([32, D], F32)
                nc.sync.dma_start(zfull[:], ccz_out.ap())
                zb = fc.tile([32, D], F32)
                nc.sync.dma_start(zb[:], fc1b_in.ap())
                nc.vector.tensor_tensor(zfull[:], zfull[:], zb[:], ALU.add)
                zr = fc.tile([32, D], F32)
                nc.scalar.activation(zr[:], zfull[:], ACT.Relu)
                if dbg:
                    nc.sync.dma_start(z_dbg.ap(), zr[:])

                # fc2: transpose z, then [32, 10] = sum_kt zT[kt].T @ fc2w[kt]
                f2w = fc.tile([128, 4, C], BF16)
                nc.sync.dma_start(f2w[:],
                                  fc2w_in.ap().rearrange("(t p) c -> p t c", p=128))
                lps = ps3.tile([32, C], F32, tag="lg")
                for t4 in range(4):
                    ztp = ps3.tile([128, 32], F32, tag="zt")
                    nc.tensor.transpose(ztp[:], zr[:, 128 * t4:128 * (t4 + 1)],
                                        ident[:32, :32])
                    zts = fc.tile([128, 32], BF16, tag="zts")
                    nc.any.tensor_copy(out=zts[:], in_=ztp[:])
                    nc.tensor.matmul(lps[:], zts[:], f2w[:, t4, :],
                                     start=(t4 == 0), stop=(t4 == 3))
                logits = fc.tile([32, C], F32)
                f2b = fc.tile([32, C], F32)
                nc.sync.dma_start(f2b[:], fc2b_in.ap())
                nc.vector.tensor_tensor(logits[:], lps[:], f2b[:], ALU.add)

                mx = fc.tile([32, 1], F32)
                nc.vector.tensor_reduce(mx[:], logits[:], axis=AX.X, op=ALU.max)
                sh = fc.tile([32, C], F32)
                nc.vector.tensor_tensor(sh[:], logits[:], mx[:].to_broadcast((32, C)),
                                        ALU.subtract)
                ex = fc.tile([32, C], F32)
                nc.scalar.activation(ex[:], sh[:], ACT.Exp)
                sm = fc.tile([32, 1], F32)
                nc.vector.tensor_reduce(sm[:], ex[:], axis=AX.X, op=ALU.add)
                lg = fc.tile([32, 1], F32)
                nc.scalar.activation(lg[:], sm[:], ACT.Ln)
                res = fc.tile([32, C], F32)
                nc.vector.tensor_tensor(res[:], sh[:], lg[:].to_broadcast((32, C)),
                                        ALU.subtract)
                nc.sync.dma_start(out_t.ap(), res[:])

    # restore the real wait thresholds the scheduling sim couldn't model
    for wait_obj, val in patches:
        wait_obj.wait_value = val
    nc.compile()
    return nc


def make_inputs(x, edge_index0, edge_index2, W1, b1, W2, b2,
                fc1_w, fc1_b, fc2_w, fc2_b):
    """Build the 8 per-core input maps."""
    A0 = _dense_adj(np.asarray(edge_index0), N0)
    A2 = _dense_adj(np.asarray(edge_index2), N2)
    A1T2 = (2.0 * A0).T.astype(np.float32)     # [N0, N0] cols -> row blocks
    M1T = (4.0 * (A0 @ A0)).T.astype(np.float32)
    A2T2 = _b16((2.0 * A2).T)                  # [N2, N2]

    U, Vm = _fold_uv()
    W1f = np.asarray(W1, np.float32)
    # W~[k] = Vm @ W1[k]  [16, G1]
    W1t = np.einsum("jf,kfg->kjg", Vm, W1f)
    # block-diag pack: rows 64h+16i..+16, cols 32i..+32 = W~[k]
    w1a = np.zeros((128, K, 2, 128), np.float32)
    for h in range(2):
        for i in range(4):
            w1a[64 * h + 16 * i:64 * h + 16 * (i + 1), :, h,
                32 * i:32 * (i + 1)] = W1t.transpose(1, 0, 2)
    w1a = _b16(w1a.reshape(128, K * 2 * 128))

    W2f = np.asarray(W2, np.float32)       # [K, G1, G2]
    # block-diag pack: rows 32i..+32, cols 32i..+32 = W2[k][:, hh half]
    w2a = np.zeros((128, K, 2, 128), np.float32)
    for bb in range(4):
        for hh in range(2):
            w2a[32 * bb:32 * bb + 32, :, hh, 32 * bb:32 * bb + 32] = \
                W2f[:, :, 32 * hh:32 * hh + 32].transpose(1, 0, 2)
    w2a = _b16(w2a.reshape(128, K * 2 * 128))

    b1v = np.tile(np.asarray(b1, np.float32), 4).reshape(128, 1)
    b2f = np.asarray(b2, np.float32)
    b2v = np.stack([np.tile(b2f[:32], 4), np.tile(b2f[32:], 4)], 1).astype(np.float32)

    fc1b = np.tile(np.asarray(fc1_b, np.float32)[None, :], (B, 1))
    fc2b = np.tile(np.asarray(fc2_b, np.float32)[None, :], (B, 1))
    fc2w = _b16(np.asarray(fc2_w, np.float32))

    # fc1w row permutation: stored row kt*128+p holds local f = p*64+kt
    NKT = FBLK // 128
    kt_a = np.arange(NKT)
    fperm = (np.arange(128)[None, :] * NKT + kt_a[:, None]).reshape(-1)

    xt = np.einsum("bnt,tj->bnj", np.asarray(x, np.float32), U)  # [B, N0, 16]
    fc1wf = np.asarray(fc1_w, np.float32)   # [N2*G2, D]

    # stored-row -> node permutation per core: row kt*128+p holds node
    # (nb^ (kt//8))*1024 + (kt%8)*128 + p
    kt_i = np.arange(NT0)
    p_i = np.arange(128)
    ins = []
    for core in range(NCORES):
        bh, nb = core // 4, core % 4
        slot_rank = (nb ^ (kt_i // NTL))
        node_idx = (slot_rank[:, None] * NBLK
                    + (kt_i % NTL)[:, None] * 128 + p_i[None, :]).reshape(-1)
        xs = xt[16 * bh:16 * (bh + 1)]          # [16, N0, 16]
        x_all = np.ascontiguousarray(
            xs.transpose(1, 0, 2).reshape(N0, C1))  # c = b_loc*16 + t
        ins.append({
            "a1t": _b16(A1T2[node_idx][:, NBLK * nb:NBLK * (nb + 1)]),
            "m1t": _b16(M1T[node_idx][:, NBLK * nb:NBLK * (nb + 1)]),
            "a2t": A2T2,
            "x_nm": _b16(x_all[node_idx]),
            "w1a": w1a, "w2a": w2a, "b1v": b1v, "b2v": b2v,
            "fc1w": _b16(fc1wf[FBLK * core:FBLK * (core + 1), :][fperm]),
            "fc1b": fc1b, "fc2b": fc2b, "fc2w": fc2w,
        })
    return ins


def batch_perm():
    """flat row order (r, b_c2) -> global batch id."""
    perm = []
    for r in range(NCORES):
        for b_c2 in range(4):
            bh, pair = b_c2 // 2, b_c2 % 2
            perm.append(16 * bh + 2 * r + pair)
    return np.array(perm)


_CACHED = {}


def kernel(**inputs):
    if "nc" not in _CACHED:
        _CACHED["nc"] = build_program(dbg=False)
    nc = _CACHED["nc"]
    ins = make_inputs(**inputs)
    res = run_bass_kernel_spmd(nc, ins, core_ids=list(range(NCORES)))
    out = np.zeros((B, C), np.float32)
    out[batch_perm()] = res.results[0]["out"]
    return out


# revision 45
# speedup vs baseline: 1.2725x; 1.2725x over previous
"""NetTGCN forward pass on 8 Trainium2 NeuronCores (Bass/Tile).

Key structure (v2):
  real(FFT) rank-16 fold: real(FFT(x, t)) = x @ Ccos with rank(Ccos)=16
  (cos(2pi t f/30) columns f and 30-f coincide), so x is host-folded to
  x~ = x @ U [B, N0, 16] and W1~[k] = V @ W1[k]; the layer-1 Chebyshev
  recurrence runs on 16 taps instead of 32 - half the matmul work.

  Layer 1 (4096-node graph): 4-way node-shard x 2-way batch-shard.
  M = 4*A^2 even/odd chains as before, but the per-step AllGather is
  replaced by direct SBUF->SBUF remote_dma_broadcast pushes into the
  peers' gather buffers (XOR-distance slots), signalled by per-peer
  arrival semaphores. Buffer reuse is safe without credits because the
  recurrence dataflow implies peers consumed parity p before the next
  write to p can be produced. Scheduling-sim deadlock is avoided by
  emitting arrival waits as >=0 and patching the real thresholds after
  Tile scheduling.

  Transition/layer 2/head: identical to the baseline (AllToAll to
  batch-parallel layer 2, fc1 sharded over contraction + AllReduce).
"""

import sys

if "/opt/trn_rl_repo" not in sys.path:
    sys.path.insert(0, "/opt/trn_rl_repo")

import numpy as np
import ml_dtypes

import concourse.bacc as bacc
import concourse.mybir as mybir
import concourse.bass_utils as _bu
from concourse.bass_utils import run_bass_kernel_spmd
from concourse.tile import TileContext
from concourse.tile_rust import add_dep_helper
from concourse.masks import make_identity

_bu.upload_artifacts = lambda tmpdir: f"file://{tmpdir}"  # no bucket in sandbox

F32 = mybir.dt.float32
BF16 = mybir.dt.bfloat16
AX = mybir.AxisListType
ALU = mybir.AluOpType
ACT = mybir.ActivationFunctionType

B, N0, T, K = 32, 4096, 30, 25
G1, G2, D, C = 32, 64, 512, 10
N2 = N0 // 4
NCORES = 8
NB = 4                 # layer-1 node shards
BL = B // 2            # 16 batches per layer-1 batch-half
TF = 16                # folded taps (rank of Ccos)
C1 = BL * TF           # 256 layer-1 channels per core
NBLK = N0 // NB        # 1024 nodes per layer-1 shard
NTL = NBLK // 128      # 8 state tiles
NT0 = N0 // 128        # 32 gathered-node tiles
P2BLK = N2 // NB       # 256 pooled nodes per layer-1 shard
B2 = 4                 # batches per layer-2 core
C2 = B2 * G1           # 128 layer-2 channels
FBLK = (N2 * G2) // NCORES  # 8192 fc1 contraction rows per core

G4 = [[0, 1, 2, 3], [4, 5, 6, 7]]
G8 = [list(range(NCORES))]


def _b16(a):
    return np.ascontiguousarray(a.astype(ml_dtypes.bfloat16))


def _dense_adj(edge_index, n):
    row = edge_index[0].astype(np.int64)
    col = edge_index[1].astype(np.int64)
    deg = np.zeros(n, np.float32)
    np.add.at(deg, row, 1.0)
    dis = np.where(deg > 0, 1.0 / np.sqrt(np.maximum(deg, 1.0)), 0.0).astype(np.float32)
    w = (-dis[row] * dis[col]).astype(np.float32)
    a = np.zeros((n, n), np.float32)
    np.add.at(a, (row, col), w)
    return a


def _fold_uv():
    """Ccos = U @ V with U [30,16], V [16,30]."""
    t = np.arange(T)
    U = np.cos(2.0 * np.pi * np.outer(t, np.arange(TF)) / T).astype(np.float32)
    Vm = np.zeros((TF, T), np.float32)
    for j in range(TF):
        Vm[j, j] = 1.0
        if 0 < j < TF - 1:
            Vm[j, T - j] += 1.0
    return U, Vm


def build_program(dbg=False):
    nc = bacc.Bacc("TRN2", target_bir_lowering=False, debug=False,
                   num_devices=NCORES)

    a1t_in = nc.dram_tensor("a1t", [N0, NBLK], BF16, kind="ExternalInput")
    m1t_in = nc.dram_tensor("m1t", [N0, NBLK], BF16, kind="ExternalInput")
    a2t_in = nc.dram_tensor("a2t", [N2, N2], BF16, kind="ExternalInput")
    x_nm_in = nc.dram_tensor("x_nm", [N0, C1], BF16, kind="ExternalInput")
    w1_in = nc.dram_tensor("w1a", [128, K * 2 * 128], BF16, kind="ExternalInput")
    w2_in = nc.dram_tensor("w2a", [128, K * 2 * 128], BF16, kind="ExternalInput")
    b1_in = nc.dram_tensor("b1v", [128, 1], F32, kind="ExternalInput")
    b2_in = nc.dram_tensor("b2v", [128, 2], F32, kind="ExternalInput")
    fc1w_in = nc.dram_tensor("fc1w", [FBLK, D], BF16, kind="ExternalInput")
    fc1b_in = nc.dram_tensor("fc1b", [B, D], F32, kind="ExternalInput")
    fc2w_in = nc.dram_tensor("fc2w", [D, C], BF16, kind="ExternalInput")
    fc2b_in = nc.dram_tensor("fc2b", [B, C], F32, kind="ExternalInput")

    out_t = nc.dram_tensor("out", [B, C], F32, kind="ExternalOutput")
    if dbg:
        h1_dbg = nc.dram_tensor("h1_dbg", [512, NBLK], F32, kind="ExternalOutput")
        l2i_dbg = nc.dram_tensor("l2i_dbg", [N2, C2], F32, kind="ExternalOutput")
        h2_dbg = nc.dram_tensor("h2_dbg", [256, N2], F32, kind="ExternalOutput")
        z_dbg = nc.dram_tensor("z_dbg", [B, D], F32, kind="ExternalOutput")

    ccp_in = nc.dram_tensor("ccp_in", [NCORES * P2BLK, 2 * G1], BF16)
    ccp_out = nc.dram_tensor("ccp_out", [NCORES * P2BLK, 2 * G1], BF16)
    cch_in = nc.dram_tensor("cch_in", [N2 * G2, B2], BF16)
    cch_out = nc.dram_tensor("cch_out", [N2 * G2, B2], BF16)
    ccz_in = nc.dram_tensor("ccz_in", [B, D], F32)
    ccz_out = nc.dram_tensor("ccz_out", [B, D], F32, addr_space="Shared")

    # arrival semaphores: rsem[d-1] counts pushes from the peer at
    # XOR-distance d (+2 per 8-slot broadcast arrival, FIFO per peer).
    rsem = [nc.alloc_semaphore(f"rsem{d}") for d in (1, 2, 3)]
    lsem = nc.alloc_semaphore("lsem")
    patches = []

    with TileContext(nc) as tc:
        with tc.tile_pool(name="const", bufs=1) as cpool:
            ident = cpool.tile([128, 128], F32)
            make_identity(nc, ident[:])
            identb = cpool.tile([128, 128], BF16)
            nc.vector.tensor_copy(identb[:], ident[:])
            # (-2I), (-1I), (-3I) in bf16: Chebyshev corrections run on the
            # PE as extra contraction tiles (exact small-int coefficients).
            nid2 = cpool.tile([128, 128], BF16)
            nc.vector.tensor_scalar_mul(nid2[:], identb[:], -2.0)
            nid1 = cpool.tile([128, 128], BF16)
            nc.vector.tensor_scalar_mul(nid1[:], identb[:], -1.0)
            nid3 = cpool.tile([128, 128], BF16)
            nc.vector.tensor_scalar_mul(nid3[:], identb[:], -3.0)

            # NOTE: no manual sem_clear here - the preamble's per-kernel
            # sem_clear zeroes all Bass-managed sems BEFORE the prelude
            # AllGather, so peer pushes can never race a clear.
            bar = nc.gpsimd.bir_kernel_barrier_wait(replica_groups=G8)
            bar_wait = bar.ins.sync_info.on_wait[0]
            patches.append((bar_wait, bar_wait.wait_value))
            bar_wait.wait_value = 0

            # ======================= LAYER 1 =======================
            with tc.tile_pool(name="l1", bufs=1) as l1, \
                 tc.tile_pool(name="l1g", bufs=6) as l1g, \
                 tc.tile_pool(name="l1a", bufs=2) as l1a, \
                 tc.tile_pool(name="l1cm", bufs=2) as l1cm, \
                 tc.tile_pool(name="ps_y", bufs=1, space="PSUM") as ps_y, \
                 tc.tile_pool(name="ps_tr", bufs=2, space="PSUM") as ps_tr, \
                 tc.tile_pool(name="ps_ct", bufs=2, space="PSUM") as ps_ct:

                m1t = l1.tile([128, NT0, NBLK], BF16)
                w1a = l1.tile([128, K, 2, 128], BF16)
                nc.sync.dma_start(w1a[:], w1_in.ap().rearrange("p (k h c) -> p k h c", k=K, h=2))
                h1_sb = l1.tile([128, 4, NBLK], F32)
                nc.any.memset(h1_sb[:], 0.0)

                # gather buffers: [chain][parity] -> [128, 4 slots, 8 nt, C1]
                # slot 0 = own block (local bf16 copy), slot d = XOR-peer d.
                gb = [[l1.tile([128, NB, NTL, C1], BF16, tag=f"gb{c}{q}",
                               name=f"gb{c}{q}")
                       for q in range(2)] for c in range(2)]

                # own x~ block, bf16 (slot-0 image of x_nm)
                tx0 = l1.tile([128, NTL, C1], BF16)
                nc.sync.dma_start(
                    tx0[:],
                    x_nm_in.ap().rearrange("(kt p) c -> p kt c", p=128)[:, 0:NTL])

                def l1_contract(src, kk):
                    # src: [128, NTL, C1] bf16 state -> cm via PE transpose
                    # (bf16 in/out, 1 cyc/row; no cross-engine stall).
                    cm = l1cm.tile([128, 2, NBLK], BF16, tag="cm", name=f"cm{kk}")
                    for cc in range(2):
                        for nt in range(NTL):
                            trt = ps_tr.tile([128, 128], BF16, tag="tr",
                                             name=f"tr{kk}_{cc}_{nt}")
                            nc.tensor.transpose(
                                trt[:], src[:, nt, 128 * cc:128 * (cc + 1)],
                                identb[:])
                            nc.any.tensor_copy(
                                out=cm[:, cc, 128 * nt:128 * (nt + 1)],
                                in_=trt[:])
                    for cc in range(2):
                        for h in range(2):
                            for ch in range(2):
                                cps = ps_ct.tile([128, 512], F32, tag="ct",
                                                 name=f"ct{kk}_{cc}_{h}_{ch}")
                                nc.tensor.matmul(
                                    cps[:], w1a[:, kk, h, :],
                                    cm[:, cc, 512 * ch:512 * (ch + 1)],
                                    start=True, stop=True)
                                nc.vector.tensor_tensor(
                                    h1_sb[:, 2 * cc + h, 512 * ch:512 * (ch + 1)],
                                    h1_sb[:, 2 * cc + h, 512 * ch:512 * (ch + 1)],
                                    cps[:], ALU.add)

                l1_contract(tx0, 0)

                for k in range(1, K):
                    cq = (k % 2, (k // 2) % 2)
                    gdst = gb[cq[0]][cq[1]]

                    # Chebyshev corrections (-2tx_{k-2}, -tx_{k-4}, -3tx_1,
                    # -x~) are folded into the PE accumulation groups as
                    # identity matmuls; the recurrence is one PSUM->bf16
                    # copy into the gather buffer's own slot.
                    def do_recur(ot, yap, k=k, gdst=gdst):
                        if k <= 2:
                            nc.vector.tensor_scalar_mul(
                                gdst[:, 0, ot, :], yap, 0.5)
                        else:
                            nc.vector.tensor_copy(gdst[:, 0, ot, :], yap)

                    if k == 2:
                        # m1t is first needed here; deferring + chunking the
                        # 8MB load keeps the k=1 streams off the DMA queues.
                        m1v = m1t_in.ap().rearrange("(t p) n -> p t n", p=128)
                        for mc in range(4):
                            nc.sync.dma_start(
                                m1t[:, 8 * mc:8 * (mc + 1), :],
                                m1v[:, 8 * mc:8 * (mc + 1), :])
                    if k <= 2:
                        # streamed rhs (x~): kt-outer needs bank-aligned
                        # accumulation groups -> two half-passes of 4 out
                        # tiles padded to one bank each.
                        for oh in range(2):
                            yp4 = ps_y.tile([128, 4, 512], F32, tag="y",
                                            name=f"y{k}_{oh}")
                            for kt in range(NT0):
                                rhs = l1g.tile([128, C1], BF16, tag="gkt",
                                               name=f"g{k}_{oh}_{kt}")
                                nc.sync.dma_start(
                                    rhs[:],
                                    x_nm_in.ap().rearrange(
                                        "(t p) c -> t p c", p=128)[kt])
                                if k == 1:
                                    op = l1a.tile([128, NBLK], BF16, tag="aop",
                                                  name=f"a{oh}_{kt}")
                                    nc.sync.dma_start(
                                        op[:], a1t_in.ap().rearrange(
                                            "(t p) n -> t p n", p=128)[kt])
                                    opv = op[:]
                                else:
                                    opv = m1t[:, kt, :]
                                for j in range(4):
                                    oi = 4 * oh + j
                                    last = (kt == NT0 - 1) and k == 1
                                    nc.tensor.matmul(
                                        yp4[:, j, 0:C1],
                                        opv[:, 128 * oi:128 * (oi + 1)],
                                        rhs[:],
                                        start=(kt == 0), stop=last)
                            for j in range(4):
                                oi = 4 * oh + j
                                if k == 2:  # tx_2 = 0.5(M x~ - 2 x~)
                                    nc.tensor.matmul(
                                        yp4[:, j, 0:C1], nid2[:],
                                        tx0[:, oi, :],
                                        start=False, stop=True)
                                do_recur(oi, yp4[:, j, 0:C1])
                    else:
                        # SBUF gather source: oi-outer so each PSUM
                        # accumulation group completes before the next
                        # starts (groups share banks at 1KB offsets).
                        gsrc = gb[(k - 2) % 2][((k - 2) // 2) % 2]
                        yp = ps_y.tile([128, NTL, C1], F32, tag="y",
                                       name=f"y{k}")
                        kwaits = []
                        for oi in range(NTL):
                            for kt in range(NT0):
                                mm = nc.tensor.matmul(
                                    yp[:, oi, :],
                                    m1t[:, kt, 128 * oi:128 * (oi + 1)],
                                    gsrc[:, kt // NTL, kt % NTL, :],
                                    start=(kt == 0), stop=False)
                                if oi == 0 and kt == NTL - 1:
                                    # arrival waits anchored after the
                                    # own-slot tiles of the first group so
                                    # the scheduler cannot hoist them ahead
                                    # of the sends peers depend on.
                                    for d in (1, 2, 3):
                                        w = nc.tensor.wait_ge(rsem[d - 1], 0)
                                        patches.append(
                                            (w.ins.sync_info.on_wait[0],
                                             2 * (k - 2)))
                                        add_dep_helper(
                                            w.ins, mm.ins,
                                            reason="wait after own tiles")
                                        kwaits.append(w)
                                if kt % NTL == 0 and kt > 0:
                                    add_dep_helper(
                                        mm.ins, kwaits[kt // NTL - 1].ins,
                                        reason="gather arrival")
                            # fold the Chebyshev corrections into the group
                            if k == 3:  # tx_3 = M tx_1 - 3 tx_1
                                nc.tensor.matmul(
                                    yp[:, oi, :], nid3[:],
                                    gsrc[:, 0, oi, :],
                                    start=False, stop=True)
                            else:       # tx_k = M tx_{k-2} -2tx_{k-2} -tx_{k-4}
                                nc.tensor.matmul(
                                    yp[:, oi, :], nid2[:],
                                    gsrc[:, 0, oi, :],
                                    start=False, stop=False)
                                p4 = (tx0[:, oi, :] if k == 4
                                      else gdst[:, 0, oi, :])
                                nc.tensor.matmul(
                                    yp[:, oi, :], nid1[:], p4,
                                    start=False, stop=True)
                        # copies batched after all groups: the PE runs the 8
                        # groups back-to-back without PSUM bank locks against
                        # the DVE reads.
                        for oi in range(NTL):
                            do_recur(oi, yp[:, oi, :])

                    # push own block to the 3 XOR-peers' matching slots
                    if k <= K - 3:
                        for d in (1, 2, 3):
                            rd = [None] * 8
                            rd[d] = (0, d)
                            prep = nc.gpsimd.remote_dma_broadcast(
                                gdst[:, d, :, :], gdst[:, 0, :, :],
                                remote_sem=rsem[d - 1], local_sem=lsem,
                                rdests=rd)
                            add_dep_helper(prep.ins, bar.ins,
                                           reason="send after barrier")
                        nc.gpsimd.trigger_dma(count=None)

                    l1_contract(gdst[:, 0, :, :], k)

                # bias + relu + maxpool4 along nodes
                b1v = l1.tile([128, 1], F32)
                nc.sync.dma_start(b1v[:], b1_in.ap())
                h1p = l1.tile([128, 4, P2BLK], F32)
                for cht in range(4):
                    nc.scalar.activation(h1_sb[:, cht, :], h1_sb[:, cht, :], ACT.Relu,
                                         bias=b1v[:])
                    h4 = h1_sb[:, cht, :].rearrange("p (n f) -> p n f", f=4)
                    nc.vector.tensor_tensor(h1p[:, cht, :], h4[:, :, 0], h4[:, :, 1],
                                            ALU.max)
                    nc.vector.tensor_tensor(h1p[:, cht, :], h1p[:, cht, :], h4[:, :, 2],
                                            ALU.max)
                    nc.vector.tensor_tensor(h1p[:, cht, :], h1p[:, cht, :], h4[:, :, 3],
                                            ALU.max)
                if dbg:
                    nc.sync.dma_start(
                        h1_dbg.ap().rearrange("(t p) n -> p t n", p=128), h1_sb[:])

                # transpose pooled block -> [n2_local, (b_loc, g)] bf16
                h1pt = l1.tile([128, P2BLK // 128, BL * G1], BF16)
                for cht in range(4):
                    for nt in range(P2BLK // 128):
                        trp = ps_tr.tile([128, 128], F32, tag="tr")
                        nc.tensor.transpose(
                            trp[:], h1p[:, cht, 128 * nt:128 * (nt + 1)], ident[:])
                        nc.any.tensor_copy(
                            out=h1pt[:, nt, 128 * cht:128 * (cht + 1)], in_=trp[:])

                ccp_iv = ccp_in.ap().rearrange("(s t p) c -> s p t c", p=128,
                                               t=P2BLK // 128)
                for s in range(NCORES):
                    nc.sync.dma_start(ccp_iv[s],
                                      h1pt[:, :, 64 * s:64 * (s + 1)])
                nc.gpsimd.collective_compute(
                    "AllToAll", ALU.bypass, replica_groups=G8,
                    ins=[ccp_in.ap()], outs=[ccp_out.ap()])

            # ======================= LAYER 2 =======================
            # ccp_out rows: src_rank * P2BLK + n2l, src_rank = bh*4 + nb;
            # cols: (b_pair 2, g 32). My batches (c2 order): b = bh*2 + pair.
            with tc.tile_pool(name="l2", bufs=1) as l2, \
                 tc.tile_pool(name="l2st", bufs=3) as l2st, \
                 tc.tile_pool(name="l2bf", bufs=2) as l2bf, \
                 tc.tile_pool(name="l2cm", bufs=2) as l2cm, \
                 tc.tile_pool(name="ps2_y", bufs=2, space="PSUM") as ps2_y, \
                 tc.tile_pool(name="ps2_tr", bufs=2, space="PSUM") as ps2_tr, \
                 tc.tile_pool(name="ps2_ct", bufs=2, space="PSUM") as ps2_ct:

                a2t = l2.tile([128, N2 // 128, N2], BF16)
                nc.sync.dma_start(a2t[:], a2t_in.ap().rearrange("(t p) n -> p t n", p=128))
                w2a = l2.tile([128, K, 2, 128], BF16)
                nc.sync.dma_start(
                    w2a[:], w2_in.ap().rearrange("p (k h g) -> p k h g", k=K, h=2))

                # init state: [128 n2, 8 nt, (b 4, g 32)] from ccp_out
                st0_bf = l2bf.tile([128, N2 // 128, C2], BF16, tag="st2bf")
                ccp_v = ccp_out.ap().rearrange(
                    "(bh nb t p) c -> bh nb p t c", bh=2, nb=NB, t=P2BLK // 128)
                for bh in range(2):
                    for nb in range(NB):
                        # dest cols [bh*64, +64) = (b = bh*2 + pair, g)
                        nc.sync.dma_start(
                            st0_bf[:, 2 * nb:2 * (nb + 1),
                                   64 * bh:64 * (bh + 1)],
                            ccp_v[bh, nb])
                st0 = l2st.tile([128, N2 // 128, C2], F32, tag="st2")
                nc.vector.tensor_copy(st0[:], st0_bf[:])
                if dbg:
                    nc.sync.dma_start(
                        l2i_dbg.ap().rearrange("(t p) c -> p t c", p=128), st0[:])

                h2a = l2.tile([128, 2, N2], F32)
                nc.any.memset(h2a[:], 0.0)

                def l2_contract(src_f32, kk):
                    cm = l2cm.tile([128, N2], BF16, tag="cm2")
                    for nt in range(N2 // 128):
                        trp = ps2_tr.tile([128, 128], F32, tag="tr2")
                        nc.tensor.transpose(trp[:], src_f32[:, nt, :], ident[:])
                        nc.any.tensor_copy(
                            out=cm[:, 128 * nt:128 * (nt + 1)], in_=trp[:])
                    for hh in range(2):
                        cps = ps2_ct.tile([128, N2], F32, tag="ct2")
                        for ch in range(N2 // 512):
                            nc.tensor.matmul(
                                cps[:, 512 * ch:512 * (ch + 1)],
                                w2a[:, kk, hh, :],
                                cm[:, 512 * ch:512 * (ch + 1)],
                                start=True, stop=True)
                        nc.vector.tensor_tensor(h2a[:, hh, :], h2a[:, hh, :],
                                                cps[:], ALU.add)

                l2_contract(st0, 0)
                tx2_pp = None
                tx2_prev = st0
                gath2 = st0_bf
                for k in range(1, K):
                    yps = []
                    for g in range(2):
                        yp = ps2_y.tile([128, 4, 128], F32, tag="y2")
                        yps.append(yp)
                        for oi in range(4):
                            ot = 4 * g + oi
                            for kt in range(N2 // 128):
                                nc.tensor.matmul(
                                    yp[:, oi, :],
                                    a2t[:, kt, 128 * ot:128 * (ot + 1)],
                                    gath2[:, kt, :],
                                    start=(kt == 0), stop=(kt == N2 // 128 - 1))
                    tx2_new = l2st.tile([128, N2 // 128, C2], F32, tag="st2")
                    g2bf = l2bf.tile([128, N2 // 128, C2], BF16, tag="st2bf")
                    for ot in range(8):
                        yap = yps[ot // 4][:, ot % 4, :]
                        if k == 1:
                            nc.vector.tensor_scalar_mul(tx2_new[:, ot, :], yap, 0.5)
                        else:
                            nc.vector.tensor_tensor(tx2_new[:, ot, :], yap,
                                                    tx2_pp[:, ot, :], ALU.subtract)
                        nc.vector.tensor_copy(g2bf[:, ot, :], tx2_new[:, ot, :])
                    l2_contract(tx2_new, k)
                    gath2 = g2bf
                    tx2_pp = tx2_prev
                    tx2_prev = tx2_new

                # bias + relu, then transpose h2 -> [n2, (b, g2)] bf16
                b2v = l2.tile([128, 2], F32)
                nc.sync.dma_start(b2v[:], b2_in.ap())
                h2r = l2.tile([128, 2, N2], F32)
                for hh in range(2):
                    nc.scalar.activation(h2r[:, hh, :], h2a[:, hh, :], ACT.Relu,
                                         bias=b2v[:, hh:hh + 1])
                if dbg:
                    nc.sync.dma_start(
                        h2_dbg.ap().rearrange("(t p) n -> p t n", p=128), h2r[:])
                # build f-major features: ft_sb[n2_l, nt, (g2 64, b 4)]
                ft_sb = l2.tile([128, N2 // 128, G2 * B2], BF16)
                for hh in range(2):
                    for nt in range(N2 // 128):
                        trp = ps2_tr.tile([128, 128], F32, tag="tr2")
                        nc.tensor.transpose(trp[:], h2r[:, hh, 128 * nt:128 * (nt + 1)],
                                            ident[:])
                        # cols of trp: (b 4, g2r 32) -> dest (g2 = hh*32+g2r, b)
                        nc.any.tensor_copy(
                            out=ft_sb[:, nt, :].rearrange("p (g b) -> p g b", g=G2)[
                                :, 32 * hh:32 * (hh + 1), :],
                            in_=trp[:].rearrange("p (b g) -> p g b", b=4))
                # AllToAll: slot j = my rows f in [FBLK*j, FBLK*(j+1))
                # cch_in rows (j, n2_l 128, g2 64), cols b
                nc.sync.dma_start(
                    cch_in.ap().rearrange("(j nl g) b -> nl j (g b)",
                                          j=NCORES, nl=128),
                    ft_sb[:])
                nc.gpsimd.collective_compute(
                    "AllToAll", ALU.bypass, replica_groups=G8,
                    ins=[cch_in.ap()], outs=[cch_out.ap()])

            # ======================= HEAD =======================
            with tc.tile_pool(name="fc", bufs=1) as fc, \
                 tc.tile_pool(name="fcw", bufs=4) as fcw, \
                 tc.tile_pool(name="ps3", bufs=2, space="PSUM") as ps3, \
                 tc.tile_pool(name="ps3z", bufs=1, space="PSUM") as ps3z:

                # flatT: [128 p, 8 r, 64 kt, 4 b]; cch rows (r, p, kt) so each
                # per-r DMA is 512B-contiguous per partition. flt[p, r, kt, b]
                # holds flat[b(r,b), f = p*64 + kt]; fc1w is host-permuted to
                # match (row kt*128+p = local f p*64+kt).
                NKT = FBLK // 128
                flt = fc.tile([128, NCORES, NKT, B2], BF16, tag="flt")
                flt2 = fc.tile([128, NKT, NCORES, B2], BF16, tag="flt2")
                zps = ps3z.tile([32, D], F32)
                cch_v = cch_out.ap().rearrange(
                    "(r p kt) b -> r p kt b", r=NCORES, p=128)
                for r in range(NCORES):
                    nc.sync.dma_start(flt[:, r, :, :], cch_v[r])
                nc.vector.tensor_copy(
                    flt2[:], flt[:].rearrange("p r kt b -> p kt r b"))
                fc1v = fc1w_in.ap().rearrange("(c kt p) d -> c p kt d",
                                              p=128, kt=8)
                for cb in range(NKT // 8):
                    fw = fcw.tile([128, 8, D], BF16, tag="fw")
                    nc.sync.dma_start(fw[:], fc1v[cb])
                    for j in range(8):
                        kt = 8 * cb + j
                        nc.tensor.matmul(
                            zps[:],
                            flt2[:, kt, :, :].rearrange("p r b -> p (r b)"),
                            fw[:, j, :],
                            start=(kt == 0), stop=(kt == NKT - 1))
                zblk = fc.tile([32, D], F32)
                nc.vector.tensor_copy(zblk[:], zps[:])
                nc.sync.dma_start(ccz_in.ap(), zblk[:])
                nc.gpsimd.collective_compute(
                    "AllReduce", ALU.add, replica_groups=G8,
                    ins=[ccz_in.ap()], outs=[ccz_out.ap()])
                zfull = fc.tile([32, D], F32)
                nc.sync.dma_start(zfull[:], ccz_out.ap())
                zb = fc.tile([32, D], F32)
                nc.sync.dma_start(zb[:], fc1b_in.ap())
                nc.vector.tensor_tensor(zfull[:], zfull[:], zb[:], ALU.add)
                zr = fc.tile([32, D], F32)
                nc.scalar.activation(zr[:], zfull[:], ACT.Relu)
                if dbg:
                    nc.sync.dma_start(z_dbg.ap(), zr[:])

                # fc2: transpose z, then [32, 10] = sum_kt zT[kt].T @ fc2w[kt]
                f2w = fc.tile([128, 4, C], BF16)
                nc.sync.dma_start(f2w[:],
                                  fc2w_in.ap().rearrange("(t p) c -> p t c", p=128))
                lps = ps3.tile([32, C], F32, tag="lg")
                for t4 in range(4):
                    ztp = ps3.tile([128, 32], F32, tag="zt")
                    nc.tensor.transpose(ztp[:], zr[:, 128 * t4:128 * (t4 + 1)],
                                        ident[:32, :32])
                    zts = fc.tile([128, 32], BF16, tag="zts")
                    nc.any.tensor_copy(out=zts[:], in_=ztp[:])
                    nc.tensor.matmul(lps[:], zts[:], f2w[:, t4, :],
                                     start=(t4 == 0), stop=(t4 == 3))
                logits = fc.tile([32, C], F32)
                f2b = fc.tile([32, C], F32)
                nc.sync.dma_start(f2b[:], fc2b_in.ap())
                nc.vector.tensor_tensor(logits[:], lps[:], f2b[:], ALU.add)

                mx = fc.tile([32, 1], F32)
                nc.vector.tensor_reduce(mx[:], logits[:], axis=AX.X, op=ALU.max)
                sh = fc.tile([32, C], F32)
                nc.vector.tensor_tensor(sh[:], logits[:], mx[:].to_broadcast((32, C)),
                                        ALU.subtract)
                ex = fc.tile([32, C], F32)
                nc.scalar.activation(ex[:], sh[:], ACT.Exp)
                sm = fc.tile([32, 1], F32)
                nc.vector.tensor_reduce(sm[:], ex[:], axis=AX.X, op=ALU.add)
                lg = fc.tile([32, 1], F32)
                nc.scalar.activation(lg[:], sm[:], ACT.Ln)
                res = fc.tile([32, C], F32)
                nc.vector.tensor_tensor(res[:], sh[:], lg[:].to_broadcast((32, C)),
                                        ALU.subtract)
                nc.sync.dma_start(out_t.ap(), res[:])

    # restore the real wait thresholds the scheduling sim couldn't model
    for wait_obj, val in patches:
        wait_obj.wait_value = val
    nc.compile()
    return nc


def make_inputs(x, edge_index0, edge_index2, W1, b1, W2, b2,
                fc1_w, fc1_b, fc2_w, fc2_b):
    """Build the 8 per-core input maps."""
    A0 = _dense_adj(np.asarray(edge_index0), N0)
    A2 = _dense_adj(np.asarray(edge_index2), N2)
    A1T2 = (2.0 * A0).T.astype(np.float32)     # [N0, N0] cols -> row blocks
    M1T = (4.0 * (A0 @ A0)).T.astype(np.float32)
    A2T2 = _b16((2.0 * A2).T)                  # [N2, N2]

    U, Vm = _fold_uv()
    W1f = np.asarray(W1, np.float32)
    # W~[k] = Vm @ W1[k]  [16, G1]
    W1t = np.einsum("jf,kfg->kjg", Vm, W1f)
    # block-diag pack: rows 64h+16i..+16, cols 32i..+32 = W~[k]
    w1a = np.zeros((128, K, 2, 128), np.float32)
    for h in range(2):
        for i in range(4):
            w1a[64 * h + 16 * i:64 * h + 16 * (i + 1), :, h,
                32 * i:32 * (i + 1)] = W1t.transpose(1, 0, 2)
    w1a = _b16(w1a.reshape(128, K * 2 * 128))

    W2f = np.asarray(W2, np.float32)       # [K, G1, G2]
    # block-diag pack: rows 32i..+32, cols 32i..+32 = W2[k][:, hh half]
    w2a = np.zeros((128, K, 2, 128), np.float32)
    for bb in range(4):
        for hh in range(2):
            w2a[32 * bb:32 * bb + 32, :, hh, 32 * bb:32 * bb + 32] = \
                W2f[:, :, 32 * hh:32 * hh + 32].transpose(1, 0, 2)
    w2a = _b16(w2a.reshape(128, K * 2 * 128))

    b1v = np.tile(np.asarray(b1, np.float32), 4).reshape(128, 1)
    b2f = np.asarray(b2, np.float32)
    b2v = np.stack([np.tile(b2f[:32], 4), np.tile(b2f[32:], 4)], 1).astype(np.float32)

    fc1b = np.tile(np.asarray(fc1_b, np.float32)[None, :], (B, 1))
    fc2b = np.tile(np.asarray(fc2_b, np.float32)[None, :], (B, 1))
    fc2w = _b16(np.asarray(fc2_w, np.float32))

    # fc1w row permutation: stored row kt*128+p holds local f = p*64+kt
    NKT = FBLK // 128
    kt_a = np.arange(NKT)
    fperm = (np.arange(128)[None, :] * NKT + kt_a[:, None]).reshape(-1)

    xt = np.einsum("bnt,tj->bnj", np.asarray(x, np.float32), U)  # [B, N0, 16]
    fc1wf = np.asarray(fc1_w, np.float32)   # [N2*G2, D]

    # stored-row -> node permutation per core: row kt*128+p holds node
    # (nb^ (kt//8))*1024 + (kt%8)*128 + p
    kt_i = np.arange(NT0)
    p_i = np.arange(128)
    ins = []
    for core in range(NCORES):
        bh, nb = core // 4, core % 4
        slot_rank = (nb ^ (kt_i // NTL))
        node_idx = (slot_rank[:, None] * NBLK
                    + (kt_i % NTL)[:, None] * 128 + p_i[None, :]).reshape(-1)
        xs = xt[16 * bh:16 * (bh + 1)]          # [16, N0, 16]
        x_all = np.ascontiguousarray(
            xs.transpose(1, 0, 2).reshape(N0, C1))  # c = b_loc*16 + t
        ins.append({
            "a1t": _b16(A1T2[node_idx][:, NBLK * nb:NBLK * (nb + 1)]),
            "m1t": _b16(M1T[node_idx][:, NBLK * nb:NBLK * (nb + 1)]),
            "a2t": A2T2,
            "x_nm": _b16(x_all[node_idx]),
            "w1a": w1a, "w2a": w2a, "b1v": b1v, "b2v": b2v,
            "fc1w": _b16(fc1wf[FBLK * core:FBLK * (core + 1), :][fperm]),
            "fc1b": fc1b, "fc2b": fc2b, "fc2w": fc2w,
        })
    return ins


def batch_perm():
    """flat row order (r, b_c2) -> global batch id."""
    perm = []
    for r in range(NCORES):
        for b_c2 in range(4):
            bh, pair = b_c2 // 2, b_c2 % 2
            perm.append(16 * bh + 2 * r + pair)
    return np.array(perm)


_CACHED = {}


def kernel(**inputs):
    if "nc" not in _CACHED:
        _CACHED["nc"] = build_program(dbg=False)
    nc = _CACHED["nc"]
    ins = make_inputs(**inputs)
    res = run_bass_kernel_spmd(nc, ins, core_ids=list(range(NCORES)))
    out = np.zeros((B, C), np.float32)
    out[batch_perm()] = res.results[0]["out"]
    return out


# revision 47
# speedup vs baseline: 1.3022x; 1.0234x over previous
"""NetTGCN forward pass on 8 Trainium2 NeuronCores (Bass/Tile).

Key structure (v2):
  real(FFT) rank-16 fold: real(FFT(x, t)) = x @ Ccos with rank(Ccos)=16
  (cos(2pi t f/30) columns f and 30-f coincide), so x is host-folded to
  x~ = x @ U [B, N0, 16] and W1~[k] = V @ W1[k]; the layer-1 Chebyshev
  recurrence runs on 16 taps instead of 32 - half the matmul work.

  Layer 1 (4096-node graph): 4-way node-shard x 2-way batch-shard.
  M = 4*A^2 even/odd chains as before, but the per-step AllGather is
  replaced by direct SBUF->SBUF remote_dma_broadcast pushes into the
  peers' gather buffers (XOR-distance slots), signalled by per-peer
  arrival semaphores. Buffer reuse is safe without credits because the
  recurrence dataflow implies peers consumed parity p before the next
  write to p can be produced. Scheduling-sim deadlock is avoided by
  emitting arrival waits as >=0 and patching the real thresholds after
  Tile scheduling.

  Transition/layer 2/head: identical to the baseline (AllToAll to
  batch-parallel layer 2, fc1 sharded over contraction + AllReduce).
"""

import sys

if "/opt/trn_rl_repo" not in sys.path:
    sys.path.insert(0, "/opt/trn_rl_repo")

import numpy as np
import ml_dtypes

import concourse.bacc as bacc
import concourse.mybir as mybir
import concourse.bass_utils as _bu
from concourse.bass_utils import run_bass_kernel_spmd
from concourse.tile import TileContext
from concourse.tile_rust import add_dep_helper
from concourse.masks import make_identity

_bu.upload_artifacts = lambda tmpdir: f"file://{tmpdir}"  # no bucket in sandbox

F32 = mybir.dt.float32
BF16 = mybir.dt.bfloat16
AX = mybir.AxisListType
ALU = mybir.AluOpType
ACT = mybir.ActivationFunctionType

B, N0, T, K = 32, 4096, 30, 25
G1, G2, D, C = 32, 64, 512, 10
N2 = N0 // 4
NCORES = 8
NB = 4                 # layer-1 node shards
BL = B // 2            # 16 batches per layer-1 batch-half
TF = 16                # folded taps (rank of Ccos)
C1 = BL * TF           # 256 layer-1 channels per core
NBLK = N0 // NB        # 1024 nodes per layer-1 shard
NTL = NBLK // 128      # 8 state tiles
NT0 = N0 // 128        # 32 gathered-node tiles
P2BLK = N2 // NB       # 256 pooled nodes per layer-1 shard
B2 = 4                 # batches per layer-2 core
C2 = B2 * G1           # 128 layer-2 channels
FBLK = (N2 * G2) // NCORES  # 8192 fc1 contraction rows per core

G4 = [[0, 1, 2, 3], [4, 5, 6, 7]]
G8 = [list(range(NCORES))]


def _b16(a):
    return np.ascontiguousarray(a.astype(ml_dtypes.bfloat16))


def _dense_adj(edge_index, n):
    row = edge_index[0].astype(np.int64)
    col = edge_index[1].astype(np.int64)
    deg = np.zeros(n, np.float32)
    np.add.at(deg, row, 1.0)
    dis = np.where(deg > 0, 1.0 / np.sqrt(np.maximum(deg, 1.0)), 0.0).astype(np.float32)
    w = (-dis[row] * dis[col]).astype(np.float32)
    a = np.zeros((n, n), np.float32)
    np.add.at(a, (row, col), w)
    return a


def _fold_uv():
    """Ccos = U @ V with U [30,16], V [16,30]."""
    t = np.arange(T)
    U = np.cos(2.0 * np.pi * np.outer(t, np.arange(TF)) / T).astype(np.float32)
    Vm = np.zeros((TF, T), np.float32)
    for j in range(TF):
        Vm[j, j] = 1.0
        if 0 < j < TF - 1:
            Vm[j, T - j] += 1.0
    return U, Vm


def build_program(dbg=False):
    nc = bacc.Bacc("TRN2", target_bir_lowering=False, debug=False,
                   num_devices=NCORES)

    a1t_in = nc.dram_tensor("a1t", [N0, NBLK], BF16, kind="ExternalInput")
    m1t_in = nc.dram_tensor("m1t", [N0, NBLK], BF16, kind="ExternalInput")
    a2t_in = nc.dram_tensor("a2t", [N2, N2], BF16, kind="ExternalInput")
    x_nm_in = nc.dram_tensor("x_nm", [N0, C1], BF16, kind="ExternalInput")
    w1_in = nc.dram_tensor("w1a", [128, K * 2 * 128], BF16, kind="ExternalInput")
    w2_in = nc.dram_tensor("w2a", [128, K * 2 * 128], BF16, kind="ExternalInput")
    b1_in = nc.dram_tensor("b1v", [128, 1], F32, kind="ExternalInput")
    b2_in = nc.dram_tensor("b2v", [128, 2], F32, kind="ExternalInput")
    fc1w_in = nc.dram_tensor("fc1w", [FBLK, D], BF16, kind="ExternalInput")
    fc1b_in = nc.dram_tensor("fc1b", [B, D], F32, kind="ExternalInput")
    fc2w_in = nc.dram_tensor("fc2w", [D, C], BF16, kind="ExternalInput")
    fc2b_in = nc.dram_tensor("fc2b", [B, C], F32, kind="ExternalInput")

    out_t = nc.dram_tensor("out", [B, C], F32, kind="ExternalOutput")
    if dbg:
        h1_dbg = nc.dram_tensor("h1_dbg", [512, NBLK], F32, kind="ExternalOutput")
        l2i_dbg = nc.dram_tensor("l2i_dbg", [N2, C2], F32, kind="ExternalOutput")
        h2_dbg = nc.dram_tensor("h2_dbg", [256, N2], F32, kind="ExternalOutput")
        z_dbg = nc.dram_tensor("z_dbg", [B, D], F32, kind="ExternalOutput")

    ccp_in = nc.dram_tensor("ccp_in", [NCORES * P2BLK, 2 * G1], BF16)
    ccp_out = nc.dram_tensor("ccp_out", [NCORES * P2BLK, 2 * G1], BF16)
    cch_in = nc.dram_tensor("cch_in", [N2 * G2, B2], BF16)
    cch_out = nc.dram_tensor("cch_out", [N2 * G2, B2], BF16)
    ccz_in = nc.dram_tensor("ccz_in", [B, D], F32)
    ccz_out = nc.dram_tensor("ccz_out", [B, D], F32, addr_space="Shared")

    # arrival semaphores: rsem[d-1] counts pushes from the peer at
    # XOR-distance d (+2 per 8-slot broadcast arrival, FIFO per peer).
    rsem = [nc.alloc_semaphore(f"rsem{d}") for d in (1, 2, 3)]
    lsem = nc.alloc_semaphore("lsem")
    patches = []

    with TileContext(nc) as tc:
        with tc.tile_pool(name="const", bufs=1) as cpool:
            ident = cpool.tile([128, 128], F32)
            make_identity(nc, ident[:])
            identb = cpool.tile([128, 128], BF16)
            nc.vector.tensor_copy(identb[:], ident[:])
            # (-2I), (-1I), (-3I) in bf16: Chebyshev corrections run on the
            # PE as extra contraction tiles (exact small-int coefficients).
            nid2 = cpool.tile([128, 128], BF16)
            nc.vector.tensor_scalar_mul(nid2[:], identb[:], -2.0)
            nid1 = cpool.tile([128, 128], BF16)
            nc.vector.tensor_scalar_mul(nid1[:], identb[:], -1.0)
            nid3 = cpool.tile([128, 128], BF16)
            nc.vector.tensor_scalar_mul(nid3[:], identb[:], -3.0)

            # NOTE: no manual sem_clear here - the preamble's per-kernel
            # sem_clear zeroes all Bass-managed sems BEFORE the prelude
            # AllGather, so peer pushes can never race a clear.
            bar = nc.gpsimd.bir_kernel_barrier_wait(replica_groups=G8)
            bar_wait = bar.ins.sync_info.on_wait[0]
            patches.append((bar_wait, bar_wait.wait_value))
            bar_wait.wait_value = 0

            # ======================= LAYER 1 =======================
            with tc.tile_pool(name="l1", bufs=1) as l1, \
                 tc.tile_pool(name="l1g", bufs=10) as l1g, \
                 tc.tile_pool(name="l1a", bufs=3) as l1a, \
                 tc.tile_pool(name="l1cm", bufs=2) as l1cm, \
                 tc.tile_pool(name="ps_y", bufs=1, space="PSUM") as ps_y, \
                 tc.tile_pool(name="ps_tr", bufs=2, space="PSUM") as ps_tr, \
                 tc.tile_pool(name="ps_ct", bufs=2, space="PSUM") as ps_ct:

                m1t = l1.tile([128, NT0, NBLK], BF16)
                w1a = l1.tile([128, K, 2, 128], BF16)
                nc.sync.dma_start(w1a[:], w1_in.ap().rearrange("p (k h c) -> p k h c", k=K, h=2))
                h1_sb = l1.tile([128, 4, NBLK], F32)
                nc.any.memset(h1_sb[:], 0.0)

                # gather buffers: [chain][parity] -> [128, 4 slots, 8 nt, C1]
                # slot 0 = own block (local bf16 copy), slot d = XOR-peer d.
                gb = [[l1.tile([128, NB, NTL, C1], BF16, tag=f"gb{c}{q}",
                               name=f"gb{c}{q}")
                       for q in range(2)] for c in range(2)]

                # own x~ block, bf16 (slot-0 image of x_nm)
                tx0 = l1.tile([128, NTL, C1], BF16)
                nc.sync.dma_start(
                    tx0[:],
                    x_nm_in.ap().rearrange("(kt p) c -> p kt c", p=128)[:, 0:NTL])

                def l1_contract(src, kk):
                    # src: [128, NTL, C1] bf16 state -> cm via PE transpose
                    # (bf16 in/out, 1 cyc/row; no cross-engine stall).
                    cm = l1cm.tile([128, 2, NBLK], BF16, tag="cm", name=f"cm{kk}")
                    for cc in range(2):
                        for nt in range(NTL):
                            trt = ps_tr.tile([128, 128], BF16, tag="tr",
                                             name=f"tr{kk}_{cc}_{nt}")
                            nc.tensor.transpose(
                                trt[:], src[:, nt, 128 * cc:128 * (cc + 1)],
                                identb[:])
                            nc.any.tensor_copy(
                                out=cm[:, cc, 128 * nt:128 * (nt + 1)],
                                in_=trt[:])
                    for cc in range(2):
                        for h in range(2):
                            for ch in range(2):
                                cps = ps_ct.tile([128, 512], F32, tag="ct",
                                                 name=f"ct{kk}_{cc}_{h}_{ch}")
                                nc.tensor.matmul(
                                    cps[:], w1a[:, kk, h, :],
                                    cm[:, cc, 512 * ch:512 * (ch + 1)],
                                    start=True, stop=True)
                                nc.vector.tensor_tensor(
                                    h1_sb[:, 2 * cc + h, 512 * ch:512 * (ch + 1)],
                                    h1_sb[:, 2 * cc + h, 512 * ch:512 * (ch + 1)],
                                    cps[:], ALU.add)

                l1_contract(tx0, 0)

                for k in range(1, K):
                    cq = (k % 2, (k // 2) % 2)
                    gdst = gb[cq[0]][cq[1]]

                    # Chebyshev corrections (-2tx_{k-2}, -tx_{k-4}, -3tx_1,
                    # -x~) are folded into the PE accumulation groups as
                    # identity matmuls; the recurrence is one PSUM->bf16
                    # copy into the gather buffer's own slot.
                    def do_recur(ot, yap, k=k, gdst=gdst):
                        if k <= 2:
                            nc.vector.tensor_scalar_mul(
                                gdst[:, 0, ot, :], yap, 0.5)
                        elif ot % 2:
                            # odd tiles via nc.any (usually ACT): frees the
                            # shared PSUM banks in parallel with DVE.
                            nc.any.tensor_copy(out=gdst[:, 0, ot, :], in_=yap)
                        else:
                            nc.vector.tensor_copy(gdst[:, 0, ot, :], yap)

                    if k == 2:
                        # m1t is first needed here; deferring + chunking the
                        # 8MB load keeps the k=1 streams off the DMA queues.
                        m1v = m1t_in.ap().rearrange("(t p) n -> p t n", p=128)
                        for mc in range(4):
                            nc.sync.dma_start(
                                m1t[:, 8 * mc:8 * (mc + 1), :],
                                m1v[:, 8 * mc:8 * (mc + 1), :])
                    if k <= 2:
                        # streamed rhs (x~): kt-outer needs bank-aligned
                        # accumulation groups -> two half-passes of 4 out
                        # tiles padded to one bank each.
                        for oh in range(2):
                            yp4 = ps_y.tile([128, 4, 512], F32, tag="y",
                                            name=f"y{k}_{oh}")
                            for kt in range(NT0):
                                rhs = l1g.tile([128, C1], BF16, tag="gkt",
                                               name=f"g{k}_{oh}_{kt}")
                                nc.sync.dma_start(
                                    rhs[:],
                                    x_nm_in.ap().rearrange(
                                        "(t p) c -> t p c", p=128)[kt])
                                if k == 1:
                                    op = l1a.tile([128, NBLK], BF16, tag="aop",
                                                  name=f"a{oh}_{kt}")
                                    nc.sync.dma_start(
                                        op[:], a1t_in.ap().rearrange(
                                            "(t p) n -> t p n", p=128)[kt])
                                    opv = op[:]
                                else:
                                    opv = m1t[:, kt, :]
                                for j in range(4):
                                    oi = 4 * oh + j
                                    last = (kt == NT0 - 1) and k == 1
                                    nc.tensor.matmul(
                                        yp4[:, j, 0:C1],
                                        opv[:, 128 * oi:128 * (oi + 1)],
                                        rhs[:],
                                        start=(kt == 0), stop=last)
                            for j in range(4):
                                oi = 4 * oh + j
                                if k == 2:  # tx_2 = 0.5(M x~ - 2 x~)
                                    nc.tensor.matmul(
                                        yp4[:, j, 0:C1], nid2[:],
                                        tx0[:, oi, :],
                                        start=False, stop=True)
                                do_recur(oi, yp4[:, j, 0:C1])
                    else:
                        # SBUF gather source: oi-outer so each PSUM
                        # accumulation group completes before the next
                        # starts (groups share banks at 1KB offsets).
                        gsrc = gb[(k - 2) % 2][((k - 2) // 2) % 2]
                        yp = ps_y.tile([128, NTL, C1], F32, tag="y",
                                       name=f"y{k}")
                        kwaits = []
                        for oi in range(NTL):
                            for kt in range(NT0):
                                mm = nc.tensor.matmul(
                                    yp[:, oi, :],
                                    m1t[:, kt, 128 * oi:128 * (oi + 1)],
                                    gsrc[:, kt // NTL, kt % NTL, :],
                                    start=(kt == 0), stop=False)
                                if oi == 0 and kt == NTL - 1:
                                    # arrival waits anchored after the
                                    # own-slot tiles of the first group so
                                    # the scheduler cannot hoist them ahead
                                    # of the sends peers depend on.
                                    for d in (1, 2, 3):
                                        w = nc.tensor.wait_ge(rsem[d - 1], 0)
                                        patches.append(
                                            (w.ins.sync_info.on_wait[0],
                                             2 * (k - 2)))
                                        add_dep_helper(
                                            w.ins, mm.ins,
                                            reason="wait after own tiles")
                                        kwaits.append(w)
                                if kt % NTL == 0 and kt > 0:
                                    add_dep_helper(
                                        mm.ins, kwaits[kt // NTL - 1].ins,
                                        reason="gather arrival")
                            # fold the Chebyshev corrections into the group
                            if k == 3:  # tx_3 = M tx_1 - 3 tx_1
                                nc.tensor.matmul(
                                    yp[:, oi, :], nid3[:],
                                    gsrc[:, 0, oi, :],
                                    start=False, stop=True)
                            else:       # tx_k = M tx_{k-2} -2tx_{k-2} -tx_{k-4}
                                nc.tensor.matmul(
                                    yp[:, oi, :], nid2[:],
                                    gsrc[:, 0, oi, :],
                                    start=False, stop=False)
                                p4 = (tx0[:, oi, :] if k == 4
                                      else gdst[:, 0, oi, :])
                                nc.tensor.matmul(
                                    yp[:, oi, :], nid1[:], p4,
                                    start=False, stop=True)
                        # copies batched after all groups: the PE runs the 8
                        # groups back-to-back without PSUM bank locks against
                        # the DVE reads.
                        for oi in range(NTL):
                            do_recur(oi, yp[:, oi, :])

                    # push own block to the 3 XOR-peers' matching slots
                    if k <= K - 3:
                        for d in (1, 2, 3):
                            rd = [None] * 8
                            rd[d] = (0, d)
                            prep = nc.gpsimd.remote_dma_broadcast(
                                gdst[:, d, :, :], gdst[:, 0, :, :],
                                remote_sem=rsem[d - 1], local_sem=lsem,
                                rdests=rd)
                            add_dep_helper(prep.ins, bar.ins,
                                           reason="send after barrier")
                        nc.gpsimd.trigger_dma(count=None)

                    l1_contract(gdst[:, 0, :, :], k)

                # bias + relu + maxpool4 along nodes
                b1v = l1.tile([128, 1], F32)
                nc.sync.dma_start(b1v[:], b1_in.ap())
                h1p = l1.tile([128, 4, P2BLK], F32)
                for cht in range(4):
                    nc.scalar.activation(h1_sb[:, cht, :], h1_sb[:, cht, :], ACT.Relu,
                                         bias=b1v[:])
                    h4 = h1_sb[:, cht, :].rearrange("p (n f) -> p n f", f=4)
                    nc.vector.tensor_tensor(h1p[:, cht, :], h4[:, :, 0], h4[:, :, 1],
                                            ALU.max)
                    nc.vector.tensor_tensor(h1p[:, cht, :], h1p[:, cht, :], h4[:, :, 2],
                                            ALU.max)
                    nc.vector.tensor_tensor(h1p[:, cht, :], h1p[:, cht, :], h4[:, :, 3],
                                            ALU.max)
                if dbg:
                    nc.sync.dma_start(
                        h1_dbg.ap().rearrange("(t p) n -> p t n", p=128), h1_sb[:])

                # transpose pooled block -> [n2_local, (b_loc, g)] bf16
                h1pt = l1.tile([128, P2BLK // 128, BL * G1], BF16)
                for cht in range(4):
                    for nt in range(P2BLK // 128):
                        trp = ps_tr.tile([128, 128], F32, tag="tr")
                        nc.tensor.transpose(
                            trp[:], h1p[:, cht, 128 * nt:128 * (nt + 1)], ident[:])
                        nc.any.tensor_copy(
                            out=h1pt[:, nt, 128 * cht:128 * (cht + 1)], in_=trp[:])

                ccp_iv = ccp_in.ap().rearrange("(s t p) c -> s p t c", p=128,
                                               t=P2BLK // 128)
                for s in range(NCORES):
                    nc.sync.dma_start(ccp_iv[s],
                                      h1pt[:, :, 64 * s:64 * (s + 1)])
                nc.gpsimd.collective_compute(
                    "AllToAll", ALU.bypass, replica_groups=G8,
                    ins=[ccp_in.ap()], outs=[ccp_out.ap()])

            # ======================= LAYER 2 =======================
            # ccp_out rows: src_rank * P2BLK + n2l, src_rank = bh*4 + nb;
            # cols: (b_pair 2, g 32). My batches (c2 order): b = bh*2 + pair.
            with tc.tile_pool(name="l2", bufs=1) as l2, \
                 tc.tile_pool(name="l2st", bufs=3) as l2st, \
                 tc.tile_pool(name="l2bf", bufs=2) as l2bf, \
                 tc.tile_pool(name="l2cm", bufs=2) as l2cm, \
                 tc.tile_pool(name="ps2_y", bufs=2, space="PSUM") as ps2_y, \
                 tc.tile_pool(name="ps2_tr", bufs=2, space="PSUM") as ps2_tr, \
                 tc.tile_pool(name="ps2_ct", bufs=2, space="PSUM") as ps2_ct:

                a2t = l2.tile([128, N2 // 128, N2], BF16)
                nc.sync.dma_start(a2t[:], a2t_in.ap().rearrange("(t p) n -> p t n", p=128))
                w2a = l2.tile([128, K, 2, 128], BF16)
                nc.sync.dma_start(
                    w2a[:], w2_in.ap().rearrange("p (k h g) -> p k h g", k=K, h=2))

                # init state: [128 n2, 8 nt, (b 4, g 32)] from ccp_out
                st0_bf = l2bf.tile([128, N2 // 128, C2], BF16, tag="st2bf")
                ccp_v = ccp_out.ap().rearrange(
                    "(bh nb t p) c -> bh nb p t c", bh=2, nb=NB, t=P2BLK // 128)
                for bh in range(2):
                    for nb in range(NB):
                        # dest cols [bh*64, +64) = (b = bh*2 + pair, g)
                        nc.sync.dma_start(
                            st0_bf[:, 2 * nb:2 * (nb + 1),
                                   64 * bh:64 * (bh + 1)],
                            ccp_v[bh, nb])
                st0 = l2st.tile([128, N2 // 128, C2], F32, tag="st2")
                nc.vector.tensor_copy(st0[:], st0_bf[:])
                if dbg:
                    nc.sync.dma_start(
                        l2i_dbg.ap().rearrange("(t p) c -> p t c", p=128), st0[:])

                h2a = l2.tile([128, 2, N2], F32)
                nc.any.memset(h2a[:], 0.0)

                def l2_contract(src_f32, kk):
                    cm = l2cm.tile([128, N2], BF16, tag="cm2")
                    for nt in range(N2 // 128):
                        trp = ps2_tr.tile([128, 128], F32, tag="tr2")
                        nc.tensor.transpose(trp[:], src_f32[:, nt, :], ident[:])
                        nc.any.tensor_copy(
                            out=cm[:, 128 * nt:128 * (nt + 1)], in_=trp[:])
                    for hh in range(2):
                        cps = ps2_ct.tile([128, N2], F32, tag="ct2")
                        for ch in range(N2 // 512):
                            nc.tensor.matmul(
                                cps[:, 512 * ch:512 * (ch + 1)],
                                w2a[:, kk, hh, :],
                                cm[:, 512 * ch:512 * (ch + 1)],
                                start=True, stop=True)
                        nc.vector.tensor_tensor(h2a[:, hh, :], h2a[:, hh, :],
                                                cps[:], ALU.add)

                l2_contract(st0, 0)
                tx2_pp = None
                tx2_prev = st0
                gath2 = st0_bf
                for k in range(1, K):
                    yps = []
                    for g in range(2):
                        yp = ps2_y.tile([128, 4, 128], F32, tag="y2")
                        yps.append(yp)
                        for oi in range(4):
                            ot = 4 * g + oi
                            for kt in range(N2 // 128):
                                nc.tensor.matmul(
                                    yp[:, oi, :],
                                    a2t[:, kt, 128 * ot:128 * (ot + 1)],
                                    gath2[:, kt, :],
                                    start=(kt == 0), stop=(kt == N2 // 128 - 1))
                    tx2_new = l2st.tile([128, N2 // 128, C2], F32, tag="st2")
                    g2bf = l2bf.tile([128, N2 // 128, C2], BF16, tag="st2bf")
                    for ot in range(8):
                        yap = yps[ot // 4][:, ot % 4, :]
                        if k == 1:
                            nc.vector.tensor_scalar_mul(tx2_new[:, ot, :], yap, 0.5)
                        else:
                            nc.vector.tensor_tensor(tx2_new[:, ot, :], yap,
                                                    tx2_pp[:, ot, :], ALU.subtract)
                        nc.vector.tensor_copy(g2bf[:, ot, :], tx2_new[:, ot, :])
                    l2_contract(tx2_new, k)
                    gath2 = g2bf
                    tx2_pp = tx2_prev
                    tx2_prev = tx2_new

                # bias + relu, then transpose h2 -> [n2, (b, g2)] bf16
                b2v = l2.tile([128, 2], F32)
                nc.sync.dma_start(b2v[:], b2_in.ap())
                h2r = l2.tile([128, 2, N2], F32)
                for hh in range(2):
                    nc.scalar.activation(h2r[:, hh, :], h2a[:, hh, :], ACT.Relu,
                                         bias=b2v[:, hh:hh + 1])
                if dbg:
                    nc.sync.dma_start(
                        h2_dbg.ap().rearrange("(t p) n -> p t n", p=128), h2r[:])
                # build f-major features: ft_sb[n2_l, nt, (g2 64, b 4)]
                ft_sb = l2.tile([128, N2 // 128, G2 * B2], BF16)
                for hh in range(2):
                    for nt in range(N2 // 128):
                        trp = ps2_tr.tile([128, 128], F32, tag="tr2")
                        nc.tensor.transpose(trp[:], h2r[:, hh, 128 * nt:128 * (nt + 1)],
                                            ident[:])
                        # cols of trp: (b 4, g2r 32) -> dest (g2 = hh*32+g2r, b)
                        nc.any.tensor_copy(
                            out=ft_sb[:, nt, :].rearrange("p (g b) -> p g b", g=G2)[
                                :, 32 * hh:32 * (hh + 1), :],
                            in_=trp[:].rearrange("p (b g) -> p g b", b=4))
                # AllToAll: slot j = my rows f in [FBLK*j, FBLK*(j+1))
                # cch_in rows (j, n2_l 128, g2 64), cols b
                nc.sync.dma_start(
                    cch_in.ap().rearrange("(j nl g) b -> nl j (g b)",
                                          j=NCORES, nl=128),
                    ft_sb[:])
                nc.gpsimd.collective_compute(
                    "AllToAll", ALU.bypass, replica_groups=G8,
                    ins=[cch_in.ap()], outs=[cch_out.ap()])

            # ======================= HEAD =======================
            with tc.tile_pool(name="fc", bufs=1) as fc, \
                 tc.tile_pool(name="fcw", bufs=4) as fcw, \
                 tc.tile_pool(name="ps3", bufs=2, space="PSUM") as ps3, \
                 tc.tile_pool(name="ps3z", bufs=1, space="PSUM") as ps3z:

                # flatT: [128 p, 8 r, 64 kt, 4 b]; cch rows (r, p, kt) so each
                # per-r DMA is 512B-contiguous per partition. flt[p, r, kt, b]
                # holds flat[b(r,b), f = p*64 + kt]; fc1w is host-permuted to
                # match (row kt*128+p = local f p*64+kt).
                NKT = FBLK // 128
                flt = fc.tile([128, NCORES, NKT, B2], BF16, tag="flt")
                flt2 = fc.tile([128, NKT, NCORES, B2], BF16, tag="flt2")
                zps = ps3z.tile([32, D], F32)
                cch_v = cch_out.ap().rearrange(
                    "(r p kt) b -> r p kt b", r=NCORES, p=128)
                for r in range(NCORES):
                    nc.sync.dma_start(flt[:, r, :, :], cch_v[r])
                nc.vector.tensor_copy(
                    flt2[:], flt[:].rearrange("p r kt b -> p kt r b"))
                fc1v = fc1w_in.ap().rearrange("(c kt p) d -> c p kt d",
                                              p=128, kt=8)
                for cb in range(NKT // 8):
                    fw = fcw.tile([128, 8, D], BF16, tag="fw")
                    nc.sync.dma_start(fw[:], fc1v[cb])
                    for j in range(8):
                        kt = 8 * cb + j
                        nc.tensor.matmul(
                            zps[:],
                            flt2[:, kt, :, :].rearrange("p r b -> p (r b)"),
                            fw[:, j, :],
                            start=(kt == 0), stop=(kt == NKT - 1))
                zblk = fc.tile([32, D], F32)
                nc.vector.tensor_copy(zblk[:], zps[:])
                nc.sync.dma_start(ccz_in.ap(), zblk[:])
                nc.gpsimd.collective_compute(
                    "AllReduce", ALU.add, replica_groups=G8,
                    ins=[ccz_in.ap()], outs=[ccz_out.ap()])
                zfull = fc.tile([32, D], F32)
                nc.sync.dma_start(zfull[:], ccz_out.ap())
                zb = fc.tile([32, D], F32)
                nc.sync.dma_start(zb[:], fc1b_in.ap())
                nc.vector.tensor_tensor(zfull[:], zfull[:], zb[:], ALU.add)
                zr = fc.tile([32, D], F32)
                nc.scalar.activation(zr[:], zfull[:], ACT.Relu)
                if dbg:
                    nc.sync.dma_start(z_dbg.ap(), zr[:])

                # fc2: transpose z, then [32, 10] = sum_kt zT[kt].T @ fc2w[kt]
                f2w = fc.tile([128, 4, C], BF16)
                nc.sync.dma_start(f2w[:],
                                  fc2w_in.ap().rearrange("(t p) c -> p t c", p=128))
                lps = ps3.tile([32, C], F32, tag="lg")
                for t4 in range(4):
                    ztp = ps3.tile([128, 32], F32, tag="zt")
                    nc.tensor.transpose(ztp[:], zr[:, 128 * t4:128 * (t4 + 1)],
                                        ident[:32, :32])
                    zts = fc.tile([128, 32], BF16, tag="zts")
                    nc.any.tensor_copy(out=zts[:], in_=ztp[:])
                    nc.tensor.matmul(lps[:], zts[:], f2w[:, t4, :],
                                     start=(t4 == 0), stop=(t4 == 3))
                logits = fc.tile([32, C], F32)
                f2b = fc.tile([32, C], F32)
                nc.sync.dma_start(f2b[:], fc2b_in.ap())
                nc.vector.tensor_tensor(logits[:], lps[:], f2b[:], ALU.add)

                mx = fc.tile([32, 1], F32)
                nc.vector.tensor_reduce(mx[:], logits[:], axis=AX.X, op=ALU.max)
                sh = fc.tile([32, C], F32)
                nc.vector.tensor_tensor(sh[:], logits[:], mx[:].to_broadcast((32, C)),
                                        ALU.subtract)
                ex = fc.tile([32, C], F32)
                nc.scalar.activation(ex[:], sh[:], ACT.Exp)
                sm = fc.tile([32, 1], F32)
                nc.vector.tensor_reduce(sm[:], ex[:], axis=AX.X, op=ALU.add)
                lg = fc.tile([32, 1], F32)
                nc.scalar.activation(lg[:], sm[:], ACT.Ln)
                res = fc.tile([32, C], F32)
                nc.vector.tensor_tensor(res[:], sh[:], lg[:].to_broadcast((32, C)),
                                        ALU.subtract)
                nc.sync.dma_start(out_t.ap(), res[:])

    # restore the real wait thresholds the scheduling sim couldn't model
    for wait_obj, val in patches:
        wait_obj.wait_value = val
    nc.compile()
    return nc


def make_inputs(x, edge_index0, edge_index2, W1, b1, W2, b2,
                fc1_w, fc1_b, fc2_w, fc2_b):
    """Build the 8 per-core input maps."""
    A0 = _dense_adj(np.asarray(edge_index0), N0)
    A2 = _dense_adj(np.asarray(edge_index2), N2)
    A1T2 = (2.0 * A0).T.astype(np.float32)     # [N0, N0] cols -> row blocks
    M1T = (4.0 * (A0 @ A0)).T.astype(np.float32)
    A2T2 = _b16((2.0 * A2).T)                  # [N2, N2]

    U, Vm = _fold_uv()
    W1f = np.asarray(W1, np.float32)
    # W~[k] = Vm @ W1[k]  [16, G1]
    W1t = np.einsum("jf,kfg->kjg", Vm, W1f)
    # block-diag pack: rows 64h+16i..+16, cols 32i..+32 = W~[k]
    w1a = np.zeros((128, K, 2, 128), np.float32)
    for h in range(2):
        for i in range(4):
            w1a[64 * h + 16 * i:64 * h + 16 * (i + 1), :, h,
                32 * i:32 * (i + 1)] = W1t.transpose(1, 0, 2)
    w1a = _b16(w1a.reshape(128, K * 2 * 128))

    W2f = np.asarray(W2, np.float32)       # [K, G1, G2]
    # block-diag pack: rows 32i..+32, cols 32i..+32 = W2[k][:, hh half]
    w2a = np.zeros((128, K, 2, 128), np.float32)
    for bb in range(4):
        for hh in range(2):
            w2a[32 * bb:32 * bb + 32, :, hh, 32 * bb:32 * bb + 32] = \
                W2f[:, :, 32 * hh:32 * hh + 32].transpose(1, 0, 2)
    w2a = _b16(w2a.reshape(128, K * 2 * 128))

    b1v = np.tile(np.asarray(b1, np.float32), 4).reshape(128, 1)
    b2f = np.asarray(b2, np.float32)
    b2v = np.stack([np.tile(b2f[:32], 4), np.tile(b2f[32:], 4)], 1).astype(np.float32)

    fc1b = np.tile(np.asarray(fc1_b, np.float32)[None, :], (B, 1))
    fc2b = np.tile(np.asarray(fc2_b, np.float32)[None, :], (B, 1))
    fc2w = _b16(np.asarray(fc2_w, np.float32))

    # fc1w row permutation: stored row kt*128+p holds local f = p*64+kt
    NKT = FBLK // 128
    kt_a = np.arange(NKT)
    fperm = (np.arange(128)[None, :] * NKT + kt_a[:, None]).reshape(-1)

    xt = np.einsum("bnt,tj->bnj", np.asarray(x, np.float32), U)  # [B, N0, 16]
    fc1wf = np.asarray(fc1_w, np.float32)   # [N2*G2, D]

    # stored-row -> node permutation per core: row kt*128+p holds node
    # (nb^ (kt//8))*1024 + (kt%8)*128 + p
    kt_i = np.arange(NT0)
    p_i = np.arange(128)
    ins = []
    for core in range(NCORES):
        bh, nb = core // 4, core % 4
        slot_rank = (nb ^ (kt_i // NTL))
        node_idx = (slot_rank[:, None] * NBLK
                    + (kt_i % NTL)[:, None] * 128 + p_i[None, :]).reshape(-1)
        xs = xt[16 * bh:16 * (bh + 1)]          # [16, N0, 16]
        x_all = np.ascontiguousarray(
            xs.transpose(1, 0, 2).reshape(N0, C1))  # c = b_loc*16 + t
        ins.append({
            "a1t": _b16(A1T2[node_idx][:, NBLK * nb:NBLK * (nb + 1)]),
            "m1t": _b16(M1T[node_idx][:, NBLK * nb:NBLK * (nb + 1)]),
            "a2t": A2T2,
            "x_nm": _b16(x_all[node_idx]),
            "w1a": w1a, "w2a": w2a, "b1v": b1v, "b2v": b2v,
            "fc1w": _b16(fc1wf[FBLK * core:FBLK * (core + 1), :][fperm]),
            "fc1b": fc1b, "fc2b": fc2b, "fc2w": fc2w,
        })
    return ins


def batch_perm():
    """flat row order (r, b_c2) -> global batch id."""
    perm = []
    for r in range(NCORES):
        for b_c2 in range(4):
            bh, pair = b_c2 // 2, b_c2 % 2
            perm.append(16 * bh + 2 * r + pair)
    return np.array(perm)


_CACHED = {}


def kernel(**inputs):
    if "nc" not in _CACHED:
        _CACHED["nc"] = build_program(dbg=False)
    nc = _CACHED["nc"]
    ins = make_inputs(**inputs)
    res = run_bass_kernel_spmd(nc, ins, core_ids=list(range(NCORES)))
    out = np.zeros((B, C), np.float32)
    out[batch_perm()] = res.results[0]["out"]
    return out


# revision 50
# speedup vs baseline: 1.3167x; 1.0111x over previous
"""NetTGCN forward pass on 8 Trainium2 NeuronCores (Bass/Tile).

Key structure (v2):
  real(FFT) rank-16 fold: real(FFT(x, t)) = x @ Ccos with rank(Ccos)=16
  (cos(2pi t f/30) columns f and 30-f coincide), so x is host-folded to
  x~ = x @ U [B, N0, 16] and W1~[k] = V @ W1[k]; the layer-1 Chebyshev
  recurrence runs on 16 taps instead of 32 - half the matmul work.

  Layer 1 (4096-node graph): 4-way node-shard x 2-way batch-shard.
  M = 4*A^2 even/odd chains as before, but the per-step AllGather is
  replaced by direct SBUF->SBUF remote_dma_broadcast pushes into the
  peers' gather buffers (XOR-distance slots), signalled by per-peer
  arrival semaphores. Buffer reuse is safe without credits because the
  recurrence dataflow implies peers consumed parity p before the next
  write to p can be produced. Scheduling-sim deadlock is avoided by
  emitting arrival waits as >=0 and patching the real thresholds after
  Tile scheduling.

  Transition/layer 2/head: identical to the baseline (AllToAll to
  batch-parallel layer 2, fc1 sharded over contraction + AllReduce).
"""

import sys

if "/opt/trn_rl_repo" not in sys.path:
    sys.path.insert(0, "/opt/trn_rl_repo")

import numpy as np
import ml_dtypes

import concourse.bacc as bacc
import concourse.mybir as mybir
import concourse.bass_utils as _bu
from concourse.bass_utils import run_bass_kernel_spmd
from concourse.tile import TileContext
from concourse.tile_rust import add_dep_helper
from concourse.masks import make_identity

_bu.upload_artifacts = lambda tmpdir: f"file://{tmpdir}"  # no bucket in sandbox

F32 = mybir.dt.float32
BF16 = mybir.dt.bfloat16
AX = mybir.AxisListType
ALU = mybir.AluOpType
ACT = mybir.ActivationFunctionType

B, N0, T, K = 32, 4096, 30, 25
G1, G2, D, C = 32, 64, 512, 10
N2 = N0 // 4
NCORES = 8
NB = 4                 # layer-1 node shards
BL = B // 2            # 16 batches per layer-1 batch-half
TF = 16                # folded taps (rank of Ccos)
C1 = BL * TF           # 256 layer-1 channels per core
NBLK = N0 // NB        # 1024 nodes per layer-1 shard
NTL = NBLK // 128      # 8 state tiles
NT0 = N0 // 128        # 32 gathered-node tiles
P2BLK = N2 // NB       # 256 pooled nodes per layer-1 shard
B2 = 4                 # batches per layer-2 core
C2 = B2 * G1           # 128 layer-2 channels
FBLK = (N2 * G2) // NCORES  # 8192 fc1 contraction rows per core

G4 = [[0, 1, 2, 3], [4, 5, 6, 7]]
G8 = [list(range(NCORES))]


def _b16(a):
    return np.ascontiguousarray(a.astype(ml_dtypes.bfloat16))


def _dense_adj(edge_index, n):
    row = edge_index[0].astype(np.int64)
    col = edge_index[1].astype(np.int64)
    deg = np.zeros(n, np.float32)
    np.add.at(deg, row, 1.0)
    dis = np.where(deg > 0, 1.0 / np.sqrt(np.maximum(deg, 1.0)), 0.0).astype(np.float32)
    w = (-dis[row] * dis[col]).astype(np.float32)
    a = np.zeros((n, n), np.float32)
    np.add.at(a, (row, col), w)
    return a


def _fold_uv():
    """Ccos = U @ V with U [30,16], V [16,30]."""
    t = np.arange(T)
    U = np.cos(2.0 * np.pi * np.outer(t, np.arange(TF)) / T).astype(np.float32)
    Vm = np.zeros((TF, T), np.float32)
    for j in range(TF):
        Vm[j, j] = 1.0
        if 0 < j < TF - 1:
            Vm[j, T - j] += 1.0
    return U, Vm


def build_program(dbg=False):
    nc = bacc.Bacc("TRN2", target_bir_lowering=False, debug=False,
                   num_devices=NCORES)

    a1t_in = nc.dram_tensor("a1t", [N0, NBLK], BF16, kind="ExternalInput")
    m1t_in = nc.dram_tensor("m1t", [N0, NBLK], BF16, kind="ExternalInput")
    a2t_in = nc.dram_tensor("a2t", [N2, N2], BF16, kind="ExternalInput")
    x_nm_in = nc.dram_tensor("x_nm", [N0, C1], BF16, kind="ExternalInput")
    w1_in = nc.dram_tensor("w1a", [128, K * 2 * 128], BF16, kind="ExternalInput")
    w2_in = nc.dram_tensor("w2a", [128, K * 2 * 128], BF16, kind="ExternalInput")
    b1_in = nc.dram_tensor("b1v", [128, 1], F32, kind="ExternalInput")
    b2_in = nc.dram_tensor("b2v", [128, 2], F32, kind="ExternalInput")
    fc1w_in = nc.dram_tensor("fc1w", [FBLK, D], BF16, kind="ExternalInput")
    fc1b_in = nc.dram_tensor("fc1b", [B, D], F32, kind="ExternalInput")
    fc2w_in = nc.dram_tensor("fc2w", [D, C], BF16, kind="ExternalInput")
    fc2b_in = nc.dram_tensor("fc2b", [B, C], F32, kind="ExternalInput")

    out_t = nc.dram_tensor("out", [B, C], F32, kind="ExternalOutput")
    if dbg:
        h1_dbg = nc.dram_tensor("h1_dbg", [512, NBLK], F32, kind="ExternalOutput")
        l2i_dbg = nc.dram_tensor("l2i_dbg", [N2, C2], F32, kind="ExternalOutput")
        h2_dbg = nc.dram_tensor("h2_dbg", [256, N2], F32, kind="ExternalOutput")
        z_dbg = nc.dram_tensor("z_dbg", [B, D], F32, kind="ExternalOutput")

    ccp_in = nc.dram_tensor("ccp_in", [NCORES * P2BLK, 2 * G1], BF16)
    ccp_out = nc.dram_tensor("ccp_out", [NCORES * P2BLK, 2 * G1], BF16)
    cch_in = nc.dram_tensor("cch_in", [N2 * G2, B2], BF16)
    cch_out = nc.dram_tensor("cch_out", [N2 * G2, B2], BF16)
    ccz_in = nc.dram_tensor("ccz_in", [B, D], F32)
    ccz_out = nc.dram_tensor("ccz_out", [B, D], F32, addr_space="Shared")

    # arrival semaphores: rsem[d-1] counts pushes from the peer at
    # XOR-distance d (+2 per 8-slot broadcast arrival, FIFO per peer).
    rsem = [nc.alloc_semaphore(f"rsem{d}") for d in (1, 2, 3)]
    lsem = nc.alloc_semaphore("lsem")
    patches = []

    with TileContext(nc) as tc:
        with tc.tile_pool(name="const", bufs=1) as cpool:
            ident = cpool.tile([128, 128], F32)
            make_identity(nc, ident[:])
            identb = cpool.tile([128, 128], BF16)
            nc.vector.tensor_copy(identb[:], ident[:])
            # (-2I), (-1I), (-3I) in bf16: Chebyshev corrections run on the
            # PE as extra contraction tiles (exact small-int coefficients).
            nid2 = cpool.tile([128, 128], BF16)
            nc.vector.tensor_scalar_mul(nid2[:], identb[:], -2.0)
            nid1 = cpool.tile([128, 128], BF16)
            nc.vector.tensor_scalar_mul(nid1[:], identb[:], -1.0)
            nid3 = cpool.tile([128, 128], BF16)
            nc.vector.tensor_scalar_mul(nid3[:], identb[:], -3.0)

            # NOTE: no manual sem_clear here - the preamble's per-kernel
            # sem_clear zeroes all Bass-managed sems BEFORE the prelude
            # AllGather, so peer pushes can never race a clear.
            bar = nc.gpsimd.bir_kernel_barrier_wait(replica_groups=G8)
            bar_wait = bar.ins.sync_info.on_wait[0]
            patches.append((bar_wait, bar_wait.wait_value))
            bar_wait.wait_value = 0

            # ======================= LAYER 1 =======================
            with tc.tile_pool(name="l1", bufs=1) as l1, \
                 tc.tile_pool(name="l1g", bufs=10) as l1g, \
                 tc.tile_pool(name="l1a", bufs=3) as l1a, \
                 tc.tile_pool(name="l1cm", bufs=2) as l1cm, \
                 tc.tile_pool(name="ps_y", bufs=1, space="PSUM") as ps_y, \
                 tc.tile_pool(name="ps_tr", bufs=2, space="PSUM") as ps_tr, \
                 tc.tile_pool(name="ps_ct", bufs=2, space="PSUM") as ps_ct:

                m1t = l1.tile([128, NT0, NBLK], BF16)
                w1a = l1.tile([128, K, 2, 128], BF16)
                nc.sync.dma_start(w1a[:], w1_in.ap().rearrange("p (k h c) -> p k h c", k=K, h=2))
                h1_sb = l1.tile([128, 4, NBLK], F32)
                nc.any.memset(h1_sb[:], 0.0)

                # gather buffers: [chain][parity] -> [128, 4 slots, 8 nt, C1]
                # slot 0 = own block (local bf16 copy), slot d = XOR-peer d.
                gb = [[l1.tile([128, NB, NTL, C1], BF16, tag=f"gb{c}{q}",
                               name=f"gb{c}{q}")
                       for q in range(2)] for c in range(2)]

                # own x~ block, bf16 (slot-0 image of x_nm)
                tx0 = l1.tile([128, NTL, C1], BF16)
                nc.sync.dma_start(
                    tx0[:],
                    x_nm_in.ap().rearrange("(kt p) c -> p kt c", p=128)[:, 0:NTL])

                def l1_contract(src, kk):
                    # src: [128, NTL, C1] bf16 state -> cm via PE transpose
                    # (bf16 in/out, 1 cyc/row; no cross-engine stall).
                    cm = l1cm.tile([128, 2, NBLK], BF16, tag="cm", name=f"cm{kk}")
                    for cc in range(2):
                        for nt in range(NTL):
                            trt = ps_tr.tile([128, 128], BF16, tag="tr",
                                             name=f"tr{kk}_{cc}_{nt}")
                            nc.tensor.transpose(
                                trt[:], src[:, nt, 128 * cc:128 * (cc + 1)],
                                identb[:])
                            nc.any.tensor_copy(
                                out=cm[:, cc, 128 * nt:128 * (nt + 1)],
                                in_=trt[:])
                    for cc in range(2):
                        for h in range(2):
                            for ch in range(2):
                                cps = ps_ct.tile([128, 512], F32, tag="ct",
                                                 name=f"ct{kk}_{cc}_{h}_{ch}")
                                nc.tensor.matmul(
                                    cps[:], w1a[:, kk, h, :],
                                    cm[:, cc, 512 * ch:512 * (ch + 1)],
                                    start=True, stop=True)
                                nc.vector.tensor_tensor(
                                    h1_sb[:, 2 * cc + h, 512 * ch:512 * (ch + 1)],
                                    h1_sb[:, 2 * cc + h, 512 * ch:512 * (ch + 1)],
                                    cps[:], ALU.add)

                l1_contract(tx0, 0)

                for k in range(1, K):
                    cq = (k % 2, (k // 2) % 2)
                    gdst = gb[cq[0]][cq[1]]

                    # Chebyshev corrections (-2tx_{k-2}, -tx_{k-4}, -3tx_1,
                    # -x~) are folded into the PE accumulation groups as
                    # identity matmuls; the recurrence is one PSUM->bf16
                    # copy into the gather buffer's own slot.
                    def do_recur(ot, yap, k=k, gdst=gdst):
                        if k <= 2:
                            nc.vector.tensor_scalar_mul(
                                gdst[:, 0, ot, :], yap, 0.5)
                        elif ot % 2:
                            # odd tiles via nc.any (usually ACT): frees the
                            # shared PSUM banks in parallel with DVE.
                            nc.any.tensor_copy(out=gdst[:, 0, ot, :], in_=yap)
                        else:
                            nc.vector.tensor_copy(gdst[:, 0, ot, :], yap)

                    if k == 2:
                        # m1t is first needed here; deferring + chunking the
                        # 8MB load keeps the k=1 streams off the DMA queues.
                        m1v = m1t_in.ap().rearrange("(t p) n -> p t n", p=128)
                        for mc in range(4):
                            nc.sync.dma_start(
                                m1t[:, 8 * mc:8 * (mc + 1), :],
                                m1v[:, 8 * mc:8 * (mc + 1), :])
                    if k <= 2:
                        # streamed rhs (x~): kt-outer needs bank-aligned
                        # accumulation groups -> two half-passes of 4 out
                        # tiles padded to one bank each.
                        for oh in range(2):
                            yp4 = ps_y.tile([128, 4, 512], F32, tag="y",
                                            name=f"y{k}_{oh}")
                            for kt in range(NT0):
                                rhs = l1g.tile([128, C1], BF16, tag="gkt",
                                               name=f"g{k}_{oh}_{kt}")
                                nc.sync.dma_start(
                                    rhs[:],
                                    x_nm_in.ap().rearrange(
                                        "(t p) c -> t p c", p=128)[kt])
                                if k == 1:
                                    op = l1a.tile([128, NBLK], BF16, tag="aop",
                                                  name=f"a{oh}_{kt}")
                                    nc.sync.dma_start(
                                        op[:], a1t_in.ap().rearrange(
                                            "(t p) n -> t p n", p=128)[kt])
                                    opv = op[:]
                                else:
                                    opv = m1t[:, kt, :]
                                for j in range(4):
                                    oi = 4 * oh + j
                                    last = (kt == NT0 - 1) and k == 1
                                    nc.tensor.matmul(
                                        yp4[:, j, 0:C1],
                                        opv[:, 128 * oi:128 * (oi + 1)],
                                        rhs[:],
                                        start=(kt == 0), stop=last)
                            for j in range(4):
                                oi = 4 * oh + j
                                if k == 2:  # tx_2 = 0.5(M x~ - 2 x~)
                                    nc.tensor.matmul(
                                        yp4[:, j, 0:C1], nid2[:],
                                        tx0[:, oi, :],
                                        start=False, stop=True)
                                do_recur(oi, yp4[:, j, 0:C1])
                    else:
                        # SBUF gather source: oi-outer so each PSUM
                        # accumulation group completes before the next
                        # starts (groups share banks at 1KB offsets).
                        gsrc = gb[(k - 2) % 2][((k - 2) // 2) % 2]
                        yp = ps_y.tile([128, NTL, C1], F32, tag="y",
                                       name=f"y{k}")
                        kwaits = []
                        for oi in range(NTL):
                            for kt in range(NT0):
                                mm = nc.tensor.matmul(
                                    yp[:, oi, :],
                                    m1t[:, kt, 128 * oi:128 * (oi + 1)],
                                    gsrc[:, kt // NTL, kt % NTL, :],
                                    start=(kt == 0), stop=False)
                                if oi == 0 and kt == NTL - 1:
                                    # arrival waits anchored after the
                                    # own-slot tiles of the first group so
                                    # the scheduler cannot hoist them ahead
                                    # of the sends peers depend on.
                                    for d in (1, 2, 3):
                                        w = nc.tensor.wait_ge(rsem[d - 1], 0)
                                        patches.append(
                                            (w.ins.sync_info.on_wait[0],
                                             2 * (k - 2)))
                                        add_dep_helper(
                                            w.ins, mm.ins,
                                            reason="wait after own tiles")
                                        kwaits.append(w)
                                if kt % NTL == 0 and kt > 0:
                                    add_dep_helper(
                                        mm.ins, kwaits[kt // NTL - 1].ins,
                                        reason="gather arrival")
                            # fold the Chebyshev corrections into the group
                            if k == 3:  # tx_3 = M tx_1 - 3 tx_1
                                nc.tensor.matmul(
                                    yp[:, oi, :], nid3[:],
                                    gsrc[:, 0, oi, :],
                                    start=False, stop=True)
                            else:       # tx_k = M tx_{k-2} -2tx_{k-2} -tx_{k-4}
                                nc.tensor.matmul(
                                    yp[:, oi, :], nid2[:],
                                    gsrc[:, 0, oi, :],
                                    start=False, stop=False)
                                p4 = (tx0[:, oi, :] if k == 4
                                      else gdst[:, 0, oi, :])
                                nc.tensor.matmul(
                                    yp[:, oi, :], nid1[:], p4,
                                    start=False, stop=True)
                        # copies batched after all groups: the PE runs the 8
                        # groups back-to-back without PSUM bank locks against
                        # the DVE reads.
                        for oi in range(NTL):
                            do_recur(oi, yp[:, oi, :])

                    # push own block to the 3 XOR-peers' matching slots
                    if k <= K - 3:
                        for d in (1, 2, 3):
                            rd = [None] * 8
                            rd[d] = (0, d)
                            prep = nc.gpsimd.remote_dma_broadcast(
                                gdst[:, d, :, :], gdst[:, 0, :, :],
                                remote_sem=rsem[d - 1], local_sem=lsem,
                                rdests=rd)
                            add_dep_helper(prep.ins, bar.ins,
                                           reason="send after barrier")
                        nc.gpsimd.trigger_dma(count=None)

                    l1_contract(gdst[:, 0, :, :], k)

                # bias + relu + maxpool4 along nodes
                b1v = l1.tile([128, 1], F32)
                nc.sync.dma_start(b1v[:], b1_in.ap())
                h1p = l1.tile([128, 4, P2BLK], F32)
                for cht in range(4):
                    nc.scalar.activation(h1_sb[:, cht, :], h1_sb[:, cht, :], ACT.Relu,
                                         bias=b1v[:])
                    h4 = h1_sb[:, cht, :].rearrange("p (n f) -> p n f", f=4)
                    nc.vector.tensor_tensor(h1p[:, cht, :], h4[:, :, 0], h4[:, :, 1],
                                            ALU.max)
                    nc.vector.tensor_tensor(h1p[:, cht, :], h1p[:, cht, :], h4[:, :, 2],
                                            ALU.max)
                    nc.vector.tensor_tensor(h1p[:, cht, :], h1p[:, cht, :], h4[:, :, 3],
                                            ALU.max)
                if dbg:
                    nc.sync.dma_start(
                        h1_dbg.ap().rearrange("(t p) n -> p t n", p=128), h1_sb[:])

                # transpose pooled block -> [n2_local, (b_loc, g)] bf16
                h1pt = l1.tile([128, P2BLK // 128, BL * G1], BF16)
                for cht in range(4):
                    for nt in range(P2BLK // 128):
                        trp = ps_tr.tile([128, 128], F32, tag="tr")
                        nc.tensor.transpose(
                            trp[:], h1p[:, cht, 128 * nt:128 * (nt + 1)], ident[:])
                        nc.any.tensor_copy(
                            out=h1pt[:, nt, 128 * cht:128 * (cht + 1)], in_=trp[:])

                ccp_iv = ccp_in.ap().rearrange("(s t p) c -> s p t c", p=128,
                                               t=P2BLK // 128)
                for s in range(NCORES):
                    nc.sync.dma_start(ccp_iv[s],
                                      h1pt[:, :, 64 * s:64 * (s + 1)])
                nc.gpsimd.collective_compute(
                    "AllToAll", ALU.bypass, replica_groups=G8,
                    ins=[ccp_in.ap()], outs=[ccp_out.ap()])

            # ======================= LAYER 2 =======================
            # ccp_out rows: src_rank * P2BLK + n2l, src_rank = bh*4 + nb;
            # cols: (b_pair 2, g 32). My batches (c2 order): b = bh*2 + pair.
            with tc.tile_pool(name="l2", bufs=1) as l2, \
                 tc.tile_pool(name="l2bf", bufs=3) as l2bf, \
                 tc.tile_pool(name="l2cm", bufs=2) as l2cm, \
                 tc.tile_pool(name="ps2_y", bufs=2, space="PSUM") as ps2_y, \
                 tc.tile_pool(name="ps2_tr", bufs=2, space="PSUM") as ps2_tr, \
                 tc.tile_pool(name="ps2_ct", bufs=2, space="PSUM") as ps2_ct:

                a2t = l2.tile([128, N2 // 128, N2], BF16)
                nc.sync.dma_start(a2t[:], a2t_in.ap().rearrange("(t p) n -> p t n", p=128))
                w2a = l2.tile([128, K, 2, 128], BF16)
                nc.sync.dma_start(
                    w2a[:], w2_in.ap().rearrange("p (k h g) -> p k h g", k=K, h=2))

                # init state: [128 n2, 8 nt, (b 4, g 32)] from ccp_out
                st0_bf = l2bf.tile([128, N2 // 128, C2], BF16, tag="st2bf")
                ccp_v = ccp_out.ap().rearrange(
                    "(bh nb t p) c -> bh nb p t c", bh=2, nb=NB, t=P2BLK // 128)
                for bh in range(2):
                    for nb in range(NB):
                        # dest cols [bh*64, +64) = (b = bh*2 + pair, g)
                        nc.sync.dma_start(
                            st0_bf[:, 2 * nb:2 * (nb + 1),
                                   64 * bh:64 * (bh + 1)],
                            ccp_v[bh, nb])
                if dbg:
                    st0d = l2.tile([128, N2 // 128, C2], F32)
                    nc.vector.tensor_copy(st0d[:], st0_bf[:])
                    nc.sync.dma_start(
                        l2i_dbg.ap().rearrange("(t p) c -> p t c", p=128),
                        st0d[:])

                h2a = l2.tile([128, 2, N2], F32)
                nc.any.memset(h2a[:], 0.0)

                def l2_contract(src_bf, kk):
                    # src_bf: [128, 8, C2] bf16 state
                    cm = l2cm.tile([128, N2], BF16, tag="cm2")
                    for nt in range(N2 // 128):
                        trp = ps2_tr.tile([128, 128], BF16, tag="tr2")
                        nc.tensor.transpose(trp[:], src_bf[:, nt, :], identb[:])
                        nc.any.tensor_copy(
                            out=cm[:, 128 * nt:128 * (nt + 1)], in_=trp[:])
                    for hh in range(2):
                        cps = ps2_ct.tile([128, N2], F32, tag="ct2")
                        for ch in range(N2 // 512):
                            nc.tensor.matmul(
                                cps[:, 512 * ch:512 * (ch + 1)],
                                w2a[:, kk, hh, :],
                                cm[:, 512 * ch:512 * (ch + 1)],
                                start=True, stop=True)
                        nc.vector.tensor_tensor(h2a[:, hh, :], h2a[:, hh, :],
                                                cps[:], ALU.add)

                l2_contract(st0_bf, 0)
                bf2 = {0: st0_bf}
                for k in range(1, K):
                    g2bf = l2bf.tile([128, N2 // 128, C2], BF16, tag="st2bf")
                    bf2[k] = g2bf
                    gath2 = bf2[k - 1]
                    yps = []
                    for g in range(2):
                        yp = ps2_y.tile([128, 4, 128], F32, tag="y2")
                        yps.append(yp)
                        for oi in range(4):
                            ot = 4 * g + oi
                            for kt in range(N2 // 128):
                                nc.tensor.matmul(
                                    yp[:, oi, :],
                                    a2t[:, kt, 128 * ot:128 * (ot + 1)],
                                    gath2[:, kt, :],
                                    start=(kt == 0),
                                    stop=(k == 1 and kt == N2 // 128 - 1))
                            if k >= 2:
                                # fold -tx_{k-2} into the group
                                nc.tensor.matmul(
                                    yp[:, oi, :], nid1[:],
                                    bf2[k - 2][:, ot, :],
                                    start=False, stop=True)
                    for ot in range(8):
                        yap = yps[ot // 4][:, ot % 4, :]
                        if k == 1:
                            nc.vector.tensor_scalar_mul(g2bf[:, ot, :], yap, 0.5)
                        elif ot % 2:
                            nc.any.tensor_copy(out=g2bf[:, ot, :], in_=yap)
                        else:
                            nc.vector.tensor_copy(g2bf[:, ot, :], yap)
                    l2_contract(g2bf, k)
                    bf2.pop(k - 2, None)

                # bias + relu, then transpose h2 -> [n2, (b, g2)] bf16
                b2v = l2.tile([128, 2], F32)
                nc.sync.dma_start(b2v[:], b2_in.ap())
                h2r = l2.tile([128, 2, N2], F32)
                for hh in range(2):
                    nc.scalar.activation(h2r[:, hh, :], h2a[:, hh, :], ACT.Relu,
                                         bias=b2v[:, hh:hh + 1])
                if dbg:
                    nc.sync.dma_start(
                        h2_dbg.ap().rearrange("(t p) n -> p t n", p=128), h2r[:])
                # build f-major features: ft_sb[n2_l, nt, (g2 64, b 4)]
                ft_sb = l2.tile([128, N2 // 128, G2 * B2], BF16)
                for hh in range(2):
                    for nt in range(N2 // 128):
                        trp = ps2_tr.tile([128, 128], F32, tag="tr2")
                        nc.tensor.transpose(trp[:], h2r[:, hh, 128 * nt:128 * (nt + 1)],
                                            ident[:])
                        # cols of trp: (b 4, g2r 32) -> dest (g2 = hh*32+g2r, b)
                        nc.any.tensor_copy(
                            out=ft_sb[:, nt, :].rearrange("p (g b) -> p g b", g=G2)[
                                :, 32 * hh:32 * (hh + 1), :],
                            in_=trp[:].rearrange("p (b g) -> p g b", b=4))
                # AllToAll: slot j = my rows f in [FBLK*j, FBLK*(j+1))
                # cch_in rows (j, n2_l 128, g2 64), cols b
                nc.sync.dma_start(
                    cch_in.ap().rearrange("(j nl g) b -> nl j (g b)",
                                          j=NCORES, nl=128),
                    ft_sb[:])
                nc.gpsimd.collective_compute(
                    "AllToAll", ALU.bypass, replica_groups=G8,
                    ins=[cch_in.ap()], outs=[cch_out.ap()])

            # ======================= HEAD =======================
            with tc.tile_pool(name="fc", bufs=1) as fc, \
                 tc.tile_pool(name="fcw", bufs=4) as fcw, \
                 tc.tile_pool(name="ps3", bufs=2, space="PSUM") as ps3, \
                 tc.tile_pool(name="ps3z", bufs=1, space="PSUM") as ps3z:

                # flatT: [128 p, 8 r, 64 kt, 4 b]; cch rows (r, p, kt) so each
                # per-r DMA is 512B-contiguous per partition. flt[p, r, kt, b]
                # holds flat[b(r,b), f = p*64 + kt]; fc1w is host-permuted to
                # match (row kt*128+p = local f p*64+kt).
                NKT = FBLK // 128
                flt = fc.tile([128, NCORES, NKT, B2], BF16, tag="flt")
                flt2 = fc.tile([128, NKT, NCORES, B2], BF16, tag="flt2")
                zps = ps3z.tile([32, D], F32)
                cch_v = cch_out.ap().rearrange(
                    "(r p kt) b -> r p kt b", r=NCORES, p=128)
                for r in range(NCORES):
                    nc.sync.dma_start(flt[:, r, :, :], cch_v[r])
                nc.vector.tensor_copy(
                    flt2[:], flt[:].rearrange("p r kt b -> p kt r b"))
                fc1v = fc1w_in.ap().rearrange("(c kt p) d -> c p kt d",
                                              p=128, kt=8)
                for cb in range(NKT // 8):
                    fw = fcw.tile([128, 8, D], BF16, tag="fw")
                    nc.sync.dma_start(fw[:], fc1v[cb])
                    for j in range(8):
                        kt = 8 * cb + j
                        nc.tensor.matmul(
                            zps[:],
                            flt2[:, kt, :, :].rearrange("p r b -> p (r b)"),
                            fw[:, j, :],
                            start=(kt == 0), stop=(kt == NKT - 1))
                zblk = fc.tile([32, D], F32)
                nc.vector.tensor_copy(zblk[:], zps[:])
                nc.sync.dma_start(ccz_in.ap(), zblk[:])
                nc.gpsimd.collective_compute(
                    "AllReduce", ALU.add, replica_groups=G8,
                    ins=[ccz_in.ap()], outs=[ccz_out.ap()])
                zfull = fc.tile([32, D], F32)
                nc.sync.dma_start(zfull[:], ccz_out.ap())
                zb = fc.tile([32, D], F32)
                nc.sync.dma_start(zb[:], fc1b_in.ap())
                nc.vector.tensor_tensor(zfull[:], zfull[:], zb[:], ALU.add)
                zr = fc.tile([32, D], F32)
                nc.scalar.activation(zr[:], zfull[:], ACT.Relu)
                if dbg:
                    nc.sync.dma_start(z_dbg.ap(), zr[:])

                # fc2: transpose z, then [32, 10] = sum_kt zT[kt].T @ fc2w[kt]
                f2w = fc.tile([128, 4, C], BF16)
                nc.sync.dma_start(f2w[:],
                                  fc2w_in.ap().rearrange("(t p) c -> p t c", p=128))
                lps = ps3.tile([32, C], F32, tag="lg")
                for t4 in range(4):
                    ztp = ps3.tile([128, 32], F32, tag="zt")
                    nc.tensor.transpose(ztp[:], zr[:, 128 * t4:128 * (t4 + 1)],
                                        ident[:32, :32])
                    zts = fc.tile([128, 32], BF16, tag="zts")
                    nc.any.tensor_copy(out=zts[:], in_=ztp[:])
                    nc.tensor.matmul(lps[:], zts[:], f2w[:, t4, :],
                                     start=(t4 == 0), stop=(t4 == 3))
                logits = fc.tile([32, C], F32)
                f2b = fc.tile([32, C], F32)
                nc.sync.dma_start(f2b[:], fc2b_in.ap())
                nc.vector.tensor_tensor(logits[:], lps[:], f2b[:], ALU.add)

                mx = fc.tile([32, 1], F32)
                nc.vector.tensor_reduce(mx[:], logits[:], axis=AX.X, op=ALU.max)
                sh = fc.tile([32, C], F32)
                nc.vector.tensor_tensor(sh[:], logits[:], mx[:].to_broadcast((32, C)),
                                        ALU.subtract)
                ex = fc.tile([32, C], F32)
                nc.scalar.activation(ex[:], sh[:], ACT.Exp)
                sm = fc.tile([32, 1], F32)
                nc.vector.tensor_reduce(sm[:], ex[:], axis=AX.X, op=ALU.add)
                lg = fc.tile([32, 1], F32)
                nc.scalar.activation(lg[:], sm[:], ACT.Ln)
                res = fc.tile([32, C], F32)
                nc.vector.tensor_tensor(res[:], sh[:], lg[:].to_broadcast((32, C)),
                                        ALU.subtract)
                nc.sync.dma_start(out_t.ap(), res[:])

    # restore the real wait thresholds the scheduling sim couldn't model
    for wait_obj, val in patches:
        wait_obj.wait_value = val
    nc.compile()
    return nc


def make_inputs(x, edge_index0, edge_index2, W1, b1, W2, b2,
                fc1_w, fc1_b, fc2_w, fc2_b):
    """Build the 8 per-core input maps."""
    A0 = _dense_adj(np.asarray(edge_index0), N0)
    A2 = _dense_adj(np.asarray(edge_index2), N2)
    A1T2 = (2.0 * A0).T.astype(np.float32)     # [N0, N0] cols -> row blocks
    M1T = (4.0 * (A0 @ A0)).T.astype(np.float32)
    A2T2 = _b16((2.0 * A2).T)                  # [N2, N2]

    U, Vm = _fold_uv()
    W1f = np.asarray(W1, np.float32)
    # W~[k] = Vm @ W1[k]  [16, G1]
    W1t = np.einsum("jf,kfg->kjg", Vm, W1f)
    # block-diag pack: rows 64h+16i..+16, cols 32i..+32 = W~[k]
    w1a = np.zeros((128, K, 2, 128), np.float32)
    for h in range(2):
        for i in range(4):
            w1a[64 * h + 16 * i:64 * h + 16 * (i + 1), :, h,
                32 * i:32 * (i + 1)] = W1t.transpose(1, 0, 2)
    w1a = _b16(w1a.reshape(128, K * 2 * 128))

    W2f = np.asarray(W2, np.float32)       # [K, G1, G2]
    # block-diag pack: rows 32i..+32, cols 32i..+32 = W2[k][:, hh half]
    w2a = np.zeros((128, K, 2, 128), np.float32)
    for bb in range(4):
        for hh in range(2):
            w2a[32 * bb:32 * bb + 32, :, hh, 32 * bb:32 * bb + 32] = \
                W2f[:, :, 32 * hh:32 * hh + 32].transpose(1, 0, 2)
    w2a = _b16(w2a.reshape(128, K * 2 * 128))

    b1v = np.tile(np.asarray(b1, np.float32), 4).reshape(128, 1)
    b2f = np.asarray(b2, np.float32)
    b2v = np.stack([np.tile(b2f[:32], 4), np.tile(b2f[32:], 4)], 1).astype(np.float32)

    fc1b = np.tile(np.asarray(fc1_b, np.float32)[None, :], (B, 1))
    fc2b = np.tile(np.asarray(fc2_b, np.float32)[None, :], (B, 1))
    fc2w = _b16(np.asarray(fc2_w, np.float32))

    # fc1w row permutation: stored row kt*128+p holds local f = p*64+kt
    NKT = FBLK // 128
    kt_a = np.arange(NKT)
    fperm = (np.arange(128)[None, :] * NKT + kt_a[:, None]).reshape(-1)

    xt = np.einsum("bnt,tj->bnj", np.asarray(x, np.float32), U)  # [B, N0, 16]
    fc1wf = np.asarray(fc1_w, np.float32)   # [N2*G2, D]

    # stored-row -> node permutation per core: row kt*128+p holds node
    # (nb^ (kt//8))*1024 + (kt%8)*128 + p
    kt_i = np.arange(NT0)
    p_i = np.arange(128)
    ins = []
    for core in range(NCORES):
        bh, nb = core // 4, core % 4
        slot_rank = (nb ^ (kt_i // NTL))
        node_idx = (slot_rank[:, None] * NBLK
                    + (kt_i % NTL)[:, None] * 128 + p_i[None, :]).reshape(-1)
        xs = xt[16 * bh:16 * (bh + 1)]          # [16, N0, 16]
        x_all = np.ascontiguousarray(
            xs.transpose(1, 0, 2).reshape(N0, C1))  # c = b_loc*16 + t
        ins.append({
            "a1t": _b16(A1T2[node_idx][:, NBLK * nb:NBLK * (nb + 1)]),
            "m1t": _b16(M1T[node_idx][:, NBLK * nb:NBLK * (nb + 1)]),
            "a2t": A2T2,
            "x_nm": _b16(x_all[node_idx]),
            "w1a": w1a, "w2a": w2a, "b1v": b1v, "b2v": b2v,
            "fc1w": _b16(fc1wf[FBLK * core:FBLK * (core + 1), :][fperm]),
            "fc1b": fc1b, "fc2b": fc2b, "fc2w": fc2w,
        })
    return ins


def batch_perm():
    """flat row order (r, b_c2) -> global batch id."""
    perm = []
    for r in range(NCORES):
        for b_c2 in range(4):
            bh, pair = b_c2 // 2, b_c2 % 2
            perm.append(16 * bh + 2 * r + pair)
    return np.array(perm)


_CACHED = {}


def kernel(**inputs):
    if "nc" not in _CACHED:
        _CACHED["nc"] = build_program(dbg=False)
    nc = _CACHED["nc"]
    ins = make_inputs(**inputs)
    res = run_bass_kernel_spmd(nc, ins, core_ids=list(range(NCORES)))
    out = np.zeros((B, C), np.float32)
    out[batch_perm()] = res.results[0]["out"]
    return out


# revision 53
# speedup vs baseline: 1.3492x; 1.0247x over previous
"""NetTGCN forward pass on 8 Trainium2 NeuronCores (Bass/Tile).

Key structure (v2):
  real(FFT) rank-16 fold: real(FFT(x, t)) = x @ Ccos with rank(Ccos)=16
  (cos(2pi t f/30) columns f and 30-f coincide), so x is host-folded to
  x~ = x @ U [B, N0, 16] and W1~[k] = V @ W1[k]; the layer-1 Chebyshev
  recurrence runs on 16 taps instead of 32 - half the matmul work.

  Layer 1 (4096-node graph): 4-way node-shard x 2-way batch-shard.
  M = 4*A^2 even/odd chains as before, but the per-step AllGather is
  replaced by direct SBUF->SBUF remote_dma_broadcast pushes into the
  peers' gather buffers (XOR-distance slots), signalled by per-peer
  arrival semaphores. Buffer reuse is safe without credits because the
  recurrence dataflow implies peers consumed parity p before the next
  write to p can be produced. Scheduling-sim deadlock is avoided by
  emitting arrival waits as >=0 and patching the real thresholds after
  Tile scheduling.

  Transition/layer 2/head: identical to the baseline (AllToAll to
  batch-parallel layer 2, fc1 sharded over contraction + AllReduce).
"""

import sys

if "/opt/trn_rl_repo" not in sys.path:
    sys.path.insert(0, "/opt/trn_rl_repo")

import numpy as np
import ml_dtypes

import concourse.bacc as bacc
import concourse.mybir as mybir
import concourse.bass_utils as _bu
from concourse.bass_utils import run_bass_kernel_spmd
from concourse.tile import TileContext
from concourse.tile_rust import add_dep_helper
from concourse.masks import make_identity

_bu.upload_artifacts = lambda tmpdir: f"file://{tmpdir}"  # no bucket in sandbox

F32 = mybir.dt.float32
BF16 = mybir.dt.bfloat16
AX = mybir.AxisListType
ALU = mybir.AluOpType
ACT = mybir.ActivationFunctionType

B, N0, T, K = 32, 4096, 30, 25
G1, G2, D, C = 32, 64, 512, 10
N2 = N0 // 4
NCORES = 8
NB = 4                 # layer-1 node shards
BL = B // 2            # 16 batches per layer-1 batch-half
TF = 16                # folded taps (rank of Ccos)
C1 = BL * TF           # 256 layer-1 channels per core
NBLK = N0 // NB        # 1024 nodes per layer-1 shard
NTL = NBLK // 128      # 8 state tiles
NT0 = N0 // 128        # 32 gathered-node tiles
P2BLK = N2 // NB       # 256 pooled nodes per layer-1 shard
B2 = 4                 # batches per layer-2 core
C2 = B2 * G1           # 128 layer-2 channels
FBLK = (N2 * G2) // NCORES  # 8192 fc1 contraction rows per core

G4 = [[0, 1, 2, 3], [4, 5, 6, 7]]
G8 = [list(range(NCORES))]


def _b16(a):
    return np.ascontiguousarray(a.astype(ml_dtypes.bfloat16))


def _dense_adj(edge_index, n):
    row = edge_index[0].astype(np.int64)
    col = edge_index[1].astype(np.int64)
    deg = np.zeros(n, np.float32)
    np.add.at(deg, row, 1.0)
    dis = np.where(deg > 0, 1.0 / np.sqrt(np.maximum(deg, 1.0)), 0.0).astype(np.float32)
    w = (-dis[row] * dis[col]).astype(np.float32)
    a = np.zeros((n, n), np.float32)
    np.add.at(a, (row, col), w)
    return a


def _fold_uv():
    """Ccos = U @ V with U [30,16], V [16,30]."""
    t = np.arange(T)
    U = np.cos(2.0 * np.pi * np.outer(t, np.arange(TF)) / T).astype(np.float32)
    Vm = np.zeros((TF, T), np.float32)
    for j in range(TF):
        Vm[j, j] = 1.0
        if 0 < j < TF - 1:
            Vm[j, T - j] += 1.0
    return U, Vm


def build_program(dbg=False):
    nc = bacc.Bacc("TRN2", target_bir_lowering=False, debug=False,
                   num_devices=NCORES)

    a1t_in = nc.dram_tensor("a1t", [N0, NBLK], BF16, kind="ExternalInput")
    m1t_in = nc.dram_tensor("m1t", [N0, NBLK], BF16, kind="ExternalInput")
    a2t_in = nc.dram_tensor("a2t", [N2, N2], BF16, kind="ExternalInput")
    x_nm_in = nc.dram_tensor("x_nm", [N0, C1], BF16, kind="ExternalInput")
    w1_in = nc.dram_tensor("w1a", [128, K * 2 * 128], BF16, kind="ExternalInput")
    w2_in = nc.dram_tensor("w2a", [128, K * 2 * 128], BF16, kind="ExternalInput")
    b1_in = nc.dram_tensor("b1v", [128, 1], F32, kind="ExternalInput")
    b2_in = nc.dram_tensor("b2v", [128, 2], F32, kind="ExternalInput")
    fc1w_in = nc.dram_tensor("fc1w", [FBLK, D], BF16, kind="ExternalInput")
    fc1b_in = nc.dram_tensor("fc1b", [B, D], F32, kind="ExternalInput")
    fc2w_in = nc.dram_tensor("fc2w", [D, C], BF16, kind="ExternalInput")
    fc2b_in = nc.dram_tensor("fc2b", [B, C], F32, kind="ExternalInput")

    out_t = nc.dram_tensor("out", [B, C], F32, kind="ExternalOutput")
    if dbg:
        h1_dbg = nc.dram_tensor("h1_dbg", [512, NBLK], F32, kind="ExternalOutput")
        l2i_dbg = nc.dram_tensor("l2i_dbg", [N2, C2], F32, kind="ExternalOutput")
        h2_dbg = nc.dram_tensor("h2_dbg", [256, N2], F32, kind="ExternalOutput")
        z_dbg = nc.dram_tensor("z_dbg", [B, D], F32, kind="ExternalOutput")

    ccp_in = nc.dram_tensor("ccp_in", [NCORES * P2BLK, 2 * G1], BF16)
    ccp_out = nc.dram_tensor("ccp_out", [NCORES * P2BLK, 2 * G1], BF16)
    cch_in = nc.dram_tensor("cch_in", [N2 * G2, B2], BF16)
    cch_out = nc.dram_tensor("cch_out", [N2 * G2, B2], BF16)
    ccz_in = nc.dram_tensor("ccz_in", [B, D], F32)
    ccz_out = nc.dram_tensor("ccz_out", [B, D], F32, addr_space="Shared")

    # arrival semaphores: rsem[d-1] counts pushes from the peer at
    # XOR-distance d (+2 per 8-slot broadcast arrival, FIFO per peer).
    rsem = [nc.alloc_semaphore(f"rsem{d}") for d in (1, 2, 3)]
    lsem = nc.alloc_semaphore("lsem")
    patches = []

    with TileContext(nc) as tc:
        with tc.tile_pool(name="const", bufs=1) as cpool:
            ident = cpool.tile([128, 128], F32)
            make_identity(nc, ident[:])
            identb = cpool.tile([128, 128], BF16)
            nc.vector.tensor_copy(identb[:], ident[:])
            # (-2I), (-1I), (-3I) in bf16: Chebyshev corrections run on the
            # PE as extra contraction tiles (exact small-int coefficients).
            nid2 = cpool.tile([128, 128], BF16)
            nc.vector.tensor_scalar_mul(nid2[:], identb[:], -2.0)
            nid1 = cpool.tile([128, 128], BF16)
            nc.vector.tensor_scalar_mul(nid1[:], identb[:], -1.0)
            nid3 = cpool.tile([128, 128], BF16)
            nc.vector.tensor_scalar_mul(nid3[:], identb[:], -3.0)

            # NOTE: no manual sem_clear here - the preamble's per-kernel
            # sem_clear zeroes all Bass-managed sems BEFORE the prelude
            # AllGather, so peer pushes can never race a clear.
            bar = nc.gpsimd.bir_kernel_barrier_wait(replica_groups=G8)
            bar_wait = bar.ins.sync_info.on_wait[0]
            patches.append((bar_wait, bar_wait.wait_value))
            bar_wait.wait_value = 0

            # ======================= LAYER 1 =======================
            with tc.tile_pool(name="l1", bufs=1) as l1, \
                 tc.tile_pool(name="l1g", bufs=32) as l1g, \
                 tc.tile_pool(name="l1a", bufs=3) as l1a, \
                 tc.tile_pool(name="l1cm", bufs=2) as l1cm, \
                 tc.tile_pool(name="ps_y", bufs=1, space="PSUM") as ps_y, \
                 tc.tile_pool(name="ps_tr", bufs=2, space="PSUM") as ps_tr, \
                 tc.tile_pool(name="ps_ct", bufs=2, space="PSUM") as ps_ct:

                m1t = l1.tile([128, NT0, NBLK], BF16)
                w1a = l1.tile([128, K, 2, 128], BF16)
                nc.sync.dma_start(w1a[:], w1_in.ap().rearrange("p (k h c) -> p k h c", k=K, h=2))
                h1_sb = l1.tile([128, 4, NBLK], F32)
                nc.any.memset(h1_sb[:], 0.0)

                # gather buffers: [chain][parity] -> [128, 4 slots, 8 nt, C1]
                # slot 0 = own block (local bf16 copy), slot d = XOR-peer d.
                gb = [[l1.tile([128, NB, NTL, C1], BF16, tag=f"gb{c}{q}",
                               name=f"gb{c}{q}")
                       for q in range(2)] for c in range(2)]

                # own x~ block, bf16 (slot-0 image of x_nm)
                tx0 = l1.tile([128, NTL, C1], BF16)
                nc.sync.dma_start(
                    tx0[:],
                    x_nm_in.ap().rearrange("(kt p) c -> p kt c", p=128)[:, 0:NTL])

                def l1_contract(src, kk):
                    # src: [128, NTL, C1] bf16 state -> cm via PE transpose
                    # (bf16 in/out, 1 cyc/row; no cross-engine stall).
                    cm = l1cm.tile([128, 2, NBLK], BF16, tag="cm", name=f"cm{kk}")
                    for cc in range(2):
                        for nt in range(NTL):
                            trt = ps_tr.tile([128, 128], BF16, tag="tr",
                                             name=f"tr{kk}_{cc}_{nt}")
                            nc.tensor.transpose(
                                trt[:], src[:, nt, 128 * cc:128 * (cc + 1)],
                                identb[:])
                            nc.any.tensor_copy(
                                out=cm[:, cc, 128 * nt:128 * (nt + 1)],
                                in_=trt[:])
                    for cc in range(2):
                        for h in range(2):
                            for ch in range(2):
                                cps = ps_ct.tile([128, 512], F32, tag="ct",
                                                 name=f"ct{kk}_{cc}_{h}_{ch}")
                                nc.tensor.matmul(
                                    cps[:], w1a[:, kk, h, :],
                                    cm[:, cc, 512 * ch:512 * (ch + 1)],
                                    start=True, stop=True)
                                nc.vector.tensor_tensor(
                                    h1_sb[:, 2 * cc + h, 512 * ch:512 * (ch + 1)],
                                    h1_sb[:, 2 * cc + h, 512 * ch:512 * (ch + 1)],
                                    cps[:], ALU.add)

                l1_contract(tx0, 0)

                xtiles = {}  # x~ rhs tiles cached across the k<=2 passes
                for k in range(1, K):
                    cq = (k % 2, (k // 2) % 2)
                    gdst = gb[cq[0]][cq[1]]

                    # Chebyshev corrections (-2tx_{k-2}, -tx_{k-4}, -3tx_1,
                    # -x~) are folded into the PE accumulation groups as
                    # identity matmuls; the recurrence is one PSUM->bf16
                    # copy into the gather buffer's own slot.
                    def do_recur(ot, yap, k=k, gdst=gdst):
                        if k <= 2:
                            nc.vector.tensor_scalar_mul(
                                gdst[:, 0, ot, :], yap, 0.5)
                        elif ot % 2:
                            # odd tiles via nc.any (usually ACT): frees the
                            # shared PSUM banks in parallel with DVE.
                            nc.any.tensor_copy(out=gdst[:, 0, ot, :], in_=yap)
                        else:
                            nc.vector.tensor_copy(gdst[:, 0, ot, :], yap)

                    if k == 2:
                        # m1t is first needed here; deferring + chunking the
                        # 8MB load keeps the k=1 streams off the DMA queues.
                        m1v = m1t_in.ap().rearrange("(t p) n -> p t n", p=128)
                        for mc in range(4):
                            nc.sync.dma_start(
                                m1t[:, 8 * mc:8 * (mc + 1), :],
                                m1v[:, 8 * mc:8 * (mc + 1), :])
                    if k <= 2:
                        # streamed rhs (x~): kt-outer needs bank-aligned
                        # accumulation groups -> two half-passes of 4 out
                        # tiles padded to one bank each.
                        for oh in range(2):
                            yp4 = ps_y.tile([128, 4, 512], F32, tag="y",
                                            name=f"y{k}_{oh}")
                            for kt in range(NT0):
                                if kt in xtiles:
                                    rhs = xtiles[kt]
                                else:
                                    rhs = l1g.tile([128, C1], BF16, tag="gkt",
                                                   name=f"g{kt}")
                                    nc.sync.dma_start(
                                        rhs[:],
                                        x_nm_in.ap().rearrange(
                                            "(t p) c -> t p c", p=128)[kt])
                                    xtiles[kt] = rhs
                                if k == 1:
                                    # stream only this half-pass's columns
                                    op = l1a.tile([128, 512], BF16, tag="aop",
                                                  name=f"a{oh}_{kt}")
                                    nc.sync.dma_start(
                                        op[:], a1t_in.ap().rearrange(
                                            "(t p) n -> t p n", p=128)[kt][
                                            :, 512 * oh:512 * (oh + 1)])
                                for j in range(4):
                                    oi = 4 * oh + j
                                    opv = (op[:, 128 * j:128 * (j + 1)]
                                           if k == 1 else
                                           m1t[:, kt, 128 * oi:128 * (oi + 1)])
                                    last = (kt == NT0 - 1) and k == 1
                                    nc.tensor.matmul(
                                        yp4[:, j, 0:C1],
                                        opv,
                                        rhs[:],
                                        start=(kt == 0), stop=last)
                            for j in range(4):
                                oi = 4 * oh + j
                                if k == 2:  # tx_2 = 0.5(M x~ - 2 x~)
                                    nc.tensor.matmul(
                                        yp4[:, j, 0:C1], nid2[:],
                                        tx0[:, oi, :],
                                        start=False, stop=True)
                                do_recur(oi, yp4[:, j, 0:C1])
                    else:
                        # SBUF gather source: oi-outer so each PSUM
                        # accumulation group completes before the next
                        # starts (groups share banks at 1KB offsets).
                        gsrc = gb[(k - 2) % 2][((k - 2) // 2) % 2]
                        yp = ps_y.tile([128, NTL, C1], F32, tag="y",
                                       name=f"y{k}")
                        kwaits = []
                        for oi in range(NTL):
                            for kt in range(NT0):
                                mm = nc.tensor.matmul(
                                    yp[:, oi, :],
                                    m1t[:, kt, 128 * oi:128 * (oi + 1)],
                                    gsrc[:, kt // NTL, kt % NTL, :],
                                    start=(kt == 0), stop=False)
                                if oi == 0 and kt == NTL - 1:
                                    # arrival waits anchored after the
                                    # own-slot tiles of the first group so
                                    # the scheduler cannot hoist them ahead
                                    # of the sends peers depend on.
                                    for d in (1, 2, 3):
                                        w = nc.tensor.wait_ge(rsem[d - 1], 0)
                                        patches.append(
                                            (w.ins.sync_info.on_wait[0],
                                             2 * (k - 2)))
                                        add_dep_helper(
                                            w.ins, mm.ins,
                                            reason="wait after own tiles")
                                        kwaits.append(w)
                                if kt % NTL == 0 and kt > 0:
                                    add_dep_helper(
                                        mm.ins, kwaits[kt // NTL - 1].ins,
                                        reason="gather arrival")
                            # fold the Chebyshev corrections into the group
                            if k == 3:  # tx_3 = M tx_1 - 3 tx_1
                                nc.tensor.matmul(
                                    yp[:, oi, :], nid3[:],
                                    gsrc[:, 0, oi, :],
                                    start=False, stop=True)
                            else:       # tx_k = M tx_{k-2} -2tx_{k-2} -tx_{k-4}
                                nc.tensor.matmul(
                                    yp[:, oi, :], nid2[:],
                                    gsrc[:, 0, oi, :],
                                    start=False, stop=False)
                                p4 = (tx0[:, oi, :] if k == 4
                                      else gdst[:, 0, oi, :])
                                nc.tensor.matmul(
                                    yp[:, oi, :], nid1[:], p4,
                                    start=False, stop=True)
                        # copies batched after all groups: the PE runs the 8
                        # groups back-to-back without PSUM bank locks against
                        # the DVE reads.
                        for oi in range(NTL):
                            do_recur(oi, yp[:, oi, :])

                    # push own block to the 3 XOR-peers' matching slots
                    if k <= K - 3:
                        for d in (1, 2, 3):
                            rd = [None] * 8
                            rd[d] = (0, d)
                            prep = nc.gpsimd.remote_dma_broadcast(
                                gdst[:, d, :, :], gdst[:, 0, :, :],
                                remote_sem=rsem[d - 1], local_sem=lsem,
                                rdests=rd)
                            add_dep_helper(prep.ins, bar.ins,
                                           reason="send after barrier")
                        nc.gpsimd.trigger_dma(count=None)

                    l1_contract(gdst[:, 0, :, :], k)

                # bias + relu + maxpool4 along nodes
                b1v = l1.tile([128, 1], F32)
                nc.sync.dma_start(b1v[:], b1_in.ap())
                h1p = l1.tile([128, 4, P2BLK], F32)
                for cht in range(4):
                    nc.scalar.activation(h1_sb[:, cht, :], h1_sb[:, cht, :], ACT.Relu,
                                         bias=b1v[:])
                    h4 = h1_sb[:, cht, :].rearrange("p (n f) -> p n f", f=4)
                    nc.vector.tensor_tensor(h1p[:, cht, :], h4[:, :, 0], h4[:, :, 1],
                                            ALU.max)
                    nc.vector.tensor_tensor(h1p[:, cht, :], h1p[:, cht, :], h4[:, :, 2],
                                            ALU.max)
                    nc.vector.tensor_tensor(h1p[:, cht, :], h1p[:, cht, :], h4[:, :, 3],
                                            ALU.max)
                if dbg:
                    nc.sync.dma_start(
                        h1_dbg.ap().rearrange("(t p) n -> p t n", p=128), h1_sb[:])

                # transpose pooled block -> [n2_local, (b_loc, g)] bf16
                h1pt = l1.tile([128, P2BLK // 128, BL * G1], BF16)
                for cht in range(4):
                    for nt in range(P2BLK // 128):
                        trp = ps_tr.tile([128, 128], F32, tag="tr")
                        nc.tensor.transpose(
                            trp[:], h1p[:, cht, 128 * nt:128 * (nt + 1)], ident[:])
                        nc.any.tensor_copy(
                            out=h1pt[:, nt, 128 * cht:128 * (cht + 1)], in_=trp[:])

                ccp_iv = ccp_in.ap().rearrange("(s t p) c -> s p t c", p=128,
                                               t=P2BLK // 128)
                for s in range(NCORES):
                    nc.sync.dma_start(ccp_iv[s],
                                      h1pt[:, :, 64 * s:64 * (s + 1)])
                nc.gpsimd.collective_compute(
                    "AllToAll", ALU.bypass, replica_groups=G8,
                    ins=[ccp_in.ap()], outs=[ccp_out.ap()])

            # ======================= LAYER 2 =======================
            # ccp_out rows: src_rank * P2BLK + n2l, src_rank = bh*4 + nb;
            # cols: (b_pair 2, g 32). My batches (c2 order): b = bh*2 + pair.
            with tc.tile_pool(name="l2", bufs=1) as l2, \
                 tc.tile_pool(name="l2bf", bufs=3) as l2bf, \
                 tc.tile_pool(name="l2cm", bufs=2) as l2cm, \
                 tc.tile_pool(name="ps2_y", bufs=2, space="PSUM") as ps2_y, \
                 tc.tile_pool(name="ps2_tr", bufs=2, space="PSUM") as ps2_tr, \
                 tc.tile_pool(name="ps2_ct", bufs=2, space="PSUM") as ps2_ct:

                a2t = l2.tile([128, N2 // 128, N2], BF16)
                nc.sync.dma_start(a2t[:], a2t_in.ap().rearrange("(t p) n -> p t n", p=128))
                w2a = l2.tile([128, K, 2, 128], BF16)
                nc.sync.dma_start(
                    w2a[:], w2_in.ap().rearrange("p (k h g) -> p k h g", k=K, h=2))

                # init state: [128 n2, 8 nt, (b 4, g 32)] from ccp_out
                st0_bf = l2bf.tile([128, N2 // 128, C2], BF16, tag="st2bf")
                ccp_v = ccp_out.ap().rearrange(
                    "(bh nb t p) c -> bh nb p t c", bh=2, nb=NB, t=P2BLK // 128)
                for bh in range(2):
                    for nb in range(NB):
                        # dest cols [bh*64, +64) = (b = bh*2 + pair, g)
                        nc.sync.dma_start(
                            st0_bf[:, 2 * nb:2 * (nb + 1),
                                   64 * bh:64 * (bh + 1)],
                            ccp_v[bh, nb])
                if dbg:
                    st0d = l2.tile([128, N2 // 128, C2], F32)
                    nc.vector.tensor_copy(st0d[:], st0_bf[:])
                    nc.sync.dma_start(
                        l2i_dbg.ap().rearrange("(t p) c -> p t c", p=128),
                        st0d[:])

                h2a = l2.tile([128, 2, N2], F32)
                nc.any.memset(h2a[:], 0.0)

                def l2_contract(src_bf, kk):
                    # src_bf: [128, 8, C2] bf16 state
                    cm = l2cm.tile([128, N2], BF16, tag="cm2")
                    for nt in range(N2 // 128):
                        trp = ps2_tr.tile([128, 128], BF16, tag="tr2")
                        nc.tensor.transpose(trp[:], src_bf[:, nt, :], identb[:])
                        nc.any.tensor_copy(
                            out=cm[:, 128 * nt:128 * (nt + 1)], in_=trp[:])
                    for hh in range(2):
                        cps = ps2_ct.tile([128, N2], F32, tag="ct2")
                        for ch in range(N2 // 512):
                            nc.tensor.matmul(
                                cps[:, 512 * ch:512 * (ch + 1)],
                                w2a[:, kk, hh, :],
                                cm[:, 512 * ch:512 * (ch + 1)],
                                start=True, stop=True)
                        nc.vector.tensor_tensor(h2a[:, hh, :], h2a[:, hh, :],
                                                cps[:], ALU.add)

                l2_contract(st0_bf, 0)
                bf2 = {0: st0_bf}
                for k in range(1, K):
                    g2bf = l2bf.tile([128, N2 // 128, C2], BF16, tag="st2bf")
                    bf2[k] = g2bf
                    gath2 = bf2[k - 1]
                    yps = []
                    for g in range(2):
                        yp = ps2_y.tile([128, 4, 128], F32, tag="y2")
                        yps.append(yp)
                        for oi in range(4):
                            ot = 4 * g + oi
                            for kt in range(N2 // 128):
                                nc.tensor.matmul(
                                    yp[:, oi, :],
                                    a2t[:, kt, 128 * ot:128 * (ot + 1)],
                                    gath2[:, kt, :],
                                    start=(kt == 0),
                                    stop=(k == 1 and kt == N2 // 128 - 1))
                            if k >= 2:
                                # fold -tx_{k-2} into the group
                                nc.tensor.matmul(
                                    yp[:, oi, :], nid1[:],
                                    bf2[k - 2][:, ot, :],
                                    start=False, stop=True)
                    for ot in range(8):
                        yap = yps[ot // 4][:, ot % 4, :]
                        if k == 1:
                            nc.vector.tensor_scalar_mul(g2bf[:, ot, :], yap, 0.5)
                        elif ot % 2:
                            nc.any.tensor_copy(out=g2bf[:, ot, :], in_=yap)
                        else:
                            nc.vector.tensor_copy(g2bf[:, ot, :], yap)
                    l2_contract(g2bf, k)
                    bf2.pop(k - 2, None)

                # bias + relu, then transpose h2 -> [n2, (b, g2)] bf16
                b2v = l2.tile([128, 2], F32)
                nc.sync.dma_start(b2v[:], b2_in.ap())
                h2r = l2.tile([128, 2, N2], F32)
                for hh in range(2):
                    nc.scalar.activation(h2r[:, hh, :], h2a[:, hh, :], ACT.Relu,
                                         bias=b2v[:, hh:hh + 1])
                if dbg:
                    nc.sync.dma_start(
                        h2_dbg.ap().rearrange("(t p) n -> p t n", p=128), h2r[:])
                # build f-major features: ft_sb[n2_l, nt, (g2 64, b 4)]
                ft_sb = l2.tile([128, N2 // 128, G2 * B2], BF16)
                for hh in range(2):
                    for nt in range(N2 // 128):
                        trp = ps2_tr.tile([128, 128], F32, tag="tr2")
                        nc.tensor.transpose(trp[:], h2r[:, hh, 128 * nt:128 * (nt + 1)],
                                            ident[:])
                        # cols of trp: (b 4, g2r 32) -> dest (g2 = hh*32+g2r, b)
                        nc.any.tensor_copy(
                            out=ft_sb[:, nt, :].rearrange("p (g b) -> p g b", g=G2)[
                                :, 32 * hh:32 * (hh + 1), :],
                            in_=trp[:].rearrange("p (b g) -> p g b", b=4))
                # AllToAll: slot j = my rows f in [FBLK*j, FBLK*(j+1))
                # cch_in rows (j, n2_l 128, g2 64), cols b
                nc.sync.dma_start(
                    cch_in.ap().rearrange("(j nl g) b -> nl j (g b)",
                                          j=NCORES, nl=128),
                    ft_sb[:])
                nc.gpsimd.collective_compute(
                    "AllToAll", ALU.bypass, replica_groups=G8,
                    ins=[cch_in.ap()], outs=[cch_out.ap()])

            # ======================= HEAD =======================
            with tc.tile_pool(name="fc", bufs=1) as fc, \
                 tc.tile_pool(name="fcw", bufs=4) as fcw, \
                 tc.tile_pool(name="ps3", bufs=2, space="PSUM") as ps3, \
                 tc.tile_pool(name="ps3z", bufs=1, space="PSUM") as ps3z:

                # flatT: [128 p, 8 r, 64 kt, 4 b]; cch rows (r, p, kt) so each
                # per-r DMA is 512B-contiguous per partition. flt[p, r, kt, b]
                # holds flat[b(r,b), f = p*64 + kt]; fc1w is host-permuted to
                # match (row kt*128+p = local f p*64+kt).
                NKT = FBLK // 128
                flt = fc.tile([128, NCORES, NKT, B2], BF16, tag="flt")
                flt2 = fc.tile([128, NKT, NCORES, B2], BF16, tag="flt2")
                zps = ps3z.tile([32, D], F32)
                cch_v = cch_out.ap().rearrange(
                    "(r p kt) b -> r p kt b", r=NCORES, p=128)
                for r in range(NCORES):
                    nc.sync.dma_start(flt[:, r, :, :], cch_v[r])
                nc.vector.tensor_copy(
                    flt2[:], flt[:].rearrange("p r kt b -> p kt r b"))
                fc1v = fc1w_in.ap().rearrange("(c kt p) d -> c p kt d",
                                              p=128, kt=8)
                for cb in range(NKT // 8):
                    fw = fcw.tile([128, 8, D], BF16, tag="fw")
                    nc.sync.dma_start(fw[:], fc1v[cb])
                    for j in range(8):
                        kt = 8 * cb + j
                        nc.tensor.matmul(
                            zps[:],
                            flt2[:, kt, :, :].rearrange("p r b -> p (r b)"),
                            fw[:, j, :],
                            start=(kt == 0), stop=(kt == NKT - 1))
                zblk = fc.tile([32, D], F32)
                nc.vector.tensor_copy(zblk[:], zps[:])
                nc.sync.dma_start(ccz_in.ap(), zblk[:])
                nc.gpsimd.collective_compute(
                    "AllReduce", ALU.add, replica_groups=G8,
                    ins=[ccz_in.ap()], outs=[ccz_out.ap()])
                zfull = fc.tile([32, D], F32)
                nc.sync.dma_start(zfull[:], ccz_out.ap())
                zb = fc.tile([32, D], F32)
                nc.sync.dma_start(zb[:], fc1b_in.ap())
                nc.vector.tensor_tensor(zfull[:], zfull[:], zb[:], ALU.add)
                zr = fc.tile([32, D], F32)
                nc.scalar.activation(zr[:], zfull[:], ACT.Relu)
                if dbg:
                    nc.sync.dma_start(z_dbg.ap(), zr[:])

                # fc2: transpose z, then [32, 10] = sum_kt zT[kt].T @ fc2w[kt]
                f2w = fc.tile([128, 4, C], BF16)
                nc.sync.dma_start(f2w[:],
                                  fc2w_in.ap().rearrange("(t p) c -> p t c", p=128))
                lps = ps3.tile([32, C], F32, tag="lg")
                for t4 in range(4):
                    ztp = ps3.tile([128, 32], F32, tag="zt")
                    nc.tensor.transpose(ztp[:], zr[:, 128 * t4:128 * (t4 + 1)],
                                        ident[:32, :32])
                    zts = fc.tile([128, 32], BF16, tag="zts")
                    nc.any.tensor_copy(out=zts[:], in_=ztp[:])
                    nc.tensor.matmul(lps[:], zts[:], f2w[:, t4, :],
                                     start=(t4 == 0), stop=(t4 == 3))
                logits = fc.tile([32, C], F32)
                f2b = fc.tile([32, C], F32)
                nc.sync.dma_start(f2b[:], fc2b_in.ap())
                nc.vector.tensor_tensor(logits[:], lps[:], f2b[:], ALU.add)

                mx = fc.tile([32, 1], F32)
                nc.vector.tensor_reduce(mx[:], logits[:], axis=AX.X, op=ALU.max)
                sh = fc.tile([32, C], F32)
                nc.vector.tensor_tensor(sh[:], logits[:], mx[:].to_broadcast((32, C)),
                                        ALU.subtract)
                ex = fc.tile([32, C], F32)
                nc.scalar.activation(ex[:], sh[:], ACT.Exp)
                sm = fc.tile([32, 1], F32)
                nc.vector.tensor_reduce(sm[:], ex[:], axis=AX.X, op=ALU.add)
                lg = fc.tile([32, 1], F32)
                nc.scalar.activation(lg[:], sm[:], ACT.Ln)
                res = fc.tile([32, C], F32)
                nc.vector.tensor_tensor(res[:], sh[:], lg[:].to_broadcast((32, C)),
                                        ALU.subtract)
                nc.sync.dma_start(out_t.ap(), res[:])

    # restore the real wait thresholds the scheduling sim couldn't model
    for wait_obj, val in patches:
        wait_obj.wait_value = val
    nc.compile()
    return nc


def make_inputs(x, edge_index0, edge_index2, W1, b1, W2, b2,
                fc1_w, fc1_b, fc2_w, fc2_b):
    """Build the 8 per-core input maps."""
    A0 = _dense_adj(np.asarray(edge_index0), N0)
    A2 = _dense_adj(np.asarray(edge_index2), N2)
    A1T2 = (2.0 * A0).T.astype(np.float32)     # [N0, N0] cols -> row blocks
    M1T = (4.0 * (A0 @ A0)).T.astype(np.float32)
    A2T2 = _b16((2.0 * A2).T)                  # [N2, N2]

    U, Vm = _fold_uv()
    W1f = np.asarray(W1, np.float32)
    # W~[k] = Vm @ W1[k]  [16, G1]
    W1t = np.einsum("jf,kfg->kjg", Vm, W1f)
    # block-diag pack: rows 64h+16i..+16, cols 32i..+32 = W~[k]
    w1a = np.zeros((128, K, 2, 128), np.float32)
    for h in range(2):
        for i in range(4):
            w1a[64 * h + 16 * i:64 * h + 16 * (i + 1), :, h,
                32 * i:32 * (i + 1)] = W1t.transpose(1, 0, 2)
    w1a = _b16(w1a.reshape(128, K * 2 * 128))

    W2f = np.asarray(W2, np.float32)       # [K, G1, G2]
    # block-diag pack: rows 32i..+32, cols 32i..+32 = W2[k][:, hh half]
    w2a = np.zeros((128, K, 2, 128), np.float32)
    for bb in range(4):
        for hh in range(2):
            w2a[32 * bb:32 * bb + 32, :, hh, 32 * bb:32 * bb + 32] = \
                W2f[:, :, 32 * hh:32 * hh + 32].transpose(1, 0, 2)
    w2a = _b16(w2a.reshape(128, K * 2 * 128))

    b1v = np.tile(np.asarray(b1, np.float32), 4).reshape(128, 1)
    b2f = np.asarray(b2, np.float32)
    b2v = np.stack([np.tile(b2f[:32], 4), np.tile(b2f[32:], 4)], 1).astype(np.float32)

    fc1b = np.tile(np.asarray(fc1_b, np.float32)[None, :], (B, 1))
    fc2b = np.tile(np.asarray(fc2_b, np.float32)[None, :], (B, 1))
    fc2w = _b16(np.asarray(fc2_w, np.float32))

    # fc1w row permutation: stored row kt*128+p holds local f = p*64+kt
    NKT = FBLK // 128
    kt_a = np.arange(NKT)
    fperm = (np.arange(128)[None, :] * NKT + kt_a[:, None]).reshape(-1)

    xt = np.einsum("bnt,tj->bnj", np.asarray(x, np.float32), U)  # [B, N0, 16]
    fc1wf = np.asarray(fc1_w, np.float32)   # [N2*G2, D]

    # stored-row -> node permutation per core: row kt*128+p holds node
    # (nb^ (kt//8))*1024 + (kt%8)*128 + p
    kt_i = np.arange(NT0)
    p_i = np.arange(128)
    ins = []
    for core in range(NCORES):
        bh, nb = core // 4, core % 4
        slot_rank = (nb ^ (kt_i // NTL))
        node_idx = (slot_rank[:, None] * NBLK
                    + (kt_i % NTL)[:, None] * 128 + p_i[None, :]).reshape(-1)
        xs = xt[16 * bh:16 * (bh + 1)]          # [16, N0, 16]
        x_all = np.ascontiguousarray(
            xs.transpose(1, 0, 2).reshape(N0, C1))  # c = b_loc*16 + t
        ins.append({
            "a1t": _b16(A1T2[node_idx][:, NBLK * nb:NBLK * (nb + 1)]),
            "m1t": _b16(M1T[node_idx][:, NBLK * nb:NBLK * (nb + 1)]),
            "a2t": A2T2,
            "x_nm": _b16(x_all[node_idx]),
            "w1a": w1a, "w2a": w2a, "b1v": b1v, "b2v": b2v,
            "fc1w": _b16(fc1wf[FBLK * core:FBLK * (core + 1), :][fperm]),
            "fc1b": fc1b, "fc2b": fc2b, "fc2w": fc2w,
        })
    return ins


def batch_perm():
    """flat row order (r, b_c2) -> global batch id."""
    perm = []
    for r in range(NCORES):
        for b_c2 in range(4):
            bh, pair = b_c2 // 2, b_c2 % 2
            perm.append(16 * bh + 2 * r + pair)
    return np.array(perm)


_CACHED = {}


def kernel(**inputs):
    if "nc" not in _CACHED:
        _CACHED["nc"] = build_program(dbg=False)
    nc = _CACHED["nc"]
    ins = make_inputs(**inputs)
    res = run_bass_kernel_spmd(nc, ins, core_ids=list(range(NCORES)))
    out = np.zeros((B, C), np.float32)
    out[batch_perm()] = res.results[0]["out"]
    return out


# revision 54
# speedup vs baseline: 1.3493x; 1.0001x over previous
"""NetTGCN forward pass on 8 Trainium2 NeuronCores (Bass/Tile).

Key structure (v2):
  real(FFT) rank-16 fold: real(FFT(x, t)) = x @ Ccos with rank(Ccos)=16
  (cos(2pi t f/30) columns f and 30-f coincide), so x is host-folded to
  x~ = x @ U [B, N0, 16] and W1~[k] = V @ W1[k]; the layer-1 Chebyshev
  recurrence runs on 16 taps instead of 32 - half the matmul work.

  Layer 1 (4096-node graph): 4-way node-shard x 2-way batch-shard.
  M = 4*A^2 even/odd chains as before, but the per-step AllGather is
  replaced by direct SBUF->SBUF remote_dma_broadcast pushes into the
  peers' gather buffers (XOR-distance slots), signalled by per-peer
  arrival semaphores. Buffer reuse is safe without credits because the
  recurrence dataflow implies peers consumed parity p before the next
  write to p can be produced. Scheduling-sim deadlock is avoided by
  emitting arrival waits as >=0 and patching the real thresholds after
  Tile scheduling.

  Transition/layer 2/head: identical to the baseline (AllToAll to
  batch-parallel layer 2, fc1 sharded over contraction + AllReduce).
"""

import sys

if "/opt/trn_rl_repo" not in sys.path:
    sys.path.insert(0, "/opt/trn_rl_repo")

import numpy as np
import ml_dtypes

import concourse.bacc as bacc
import concourse.mybir as mybir
import concourse.bass_utils as _bu
from concourse.bass_utils import run_bass_kernel_spmd
from concourse.tile import TileContext
from concourse.tile_rust import add_dep_helper
from concourse.masks import make_identity

_bu.upload_artifacts = lambda tmpdir: f"file://{tmpdir}"  # no bucket in sandbox

F32 = mybir.dt.float32
BF16 = mybir.dt.bfloat16
AX = mybir.AxisListType
ALU = mybir.AluOpType
ACT = mybir.ActivationFunctionType

B, N0, T, K = 32, 4096, 30, 25
G1, G2, D, C = 32, 64, 512, 10
N2 = N0 // 4
NCORES = 8
NB = 4                 # layer-1 node shards
BL = B // 2            # 16 batches per layer-1 batch-half
TF = 16                # folded taps (rank of Ccos)
C1 = BL * TF           # 256 layer-1 channels per core
NBLK = N0 // NB        # 1024 nodes per layer-1 shard
NTL = NBLK // 128      # 8 state tiles
NT0 = N0 // 128        # 32 gathered-node tiles
P2BLK = N2 // NB       # 256 pooled nodes per layer-1 shard
B2 = 4                 # batches per layer-2 core
C2 = B2 * G1           # 128 layer-2 channels
FBLK = (N2 * G2) // NCORES  # 8192 fc1 contraction rows per core

G4 = [[0, 1, 2, 3], [4, 5, 6, 7]]
G8 = [list(range(NCORES))]


def _b16(a):
    return np.ascontiguousarray(a.astype(ml_dtypes.bfloat16))


def _dense_adj(edge_index, n):
    row = edge_index[0].astype(np.int64)
    col = edge_index[1].astype(np.int64)
    deg = np.zeros(n, np.float32)
    np.add.at(deg, row, 1.0)
    dis = np.where(deg > 0, 1.0 / np.sqrt(np.maximum(deg, 1.0)), 0.0).astype(np.float32)
    w = (-dis[row] * dis[col]).astype(np.float32)
    a = np.zeros((n, n), np.float32)
    np.add.at(a, (row, col), w)
    return a


def _fold_uv():
    """Ccos = U @ V with U [30,16], V [16,30]."""
    t = np.arange(T)
    U = np.cos(2.0 * np.pi * np.outer(t, np.arange(TF)) / T).astype(np.float32)
    Vm = np.zeros((TF, T), np.float32)
    for j in range(TF):
        Vm[j, j] = 1.0
        if 0 < j < TF - 1:
            Vm[j, T - j] += 1.0
    return U, Vm


def build_program(dbg=False):
    nc = bacc.Bacc("TRN2", target_bir_lowering=False, debug=False,
                   num_devices=NCORES)

    a1t_in = nc.dram_tensor("a1t", [N0, NBLK], BF16, kind="ExternalInput")
    m1t_in = nc.dram_tensor("m1t", [N0, NBLK], BF16, kind="ExternalInput")
    a2t_in = nc.dram_tensor("a2t", [N2, N2], BF16, kind="ExternalInput")
    x_nm_in = nc.dram_tensor("x_nm", [N0, C1], BF16, kind="ExternalInput")
    w1_in = nc.dram_tensor("w1a", [128, K * 2 * 128], BF16, kind="ExternalInput")
    w2_in = nc.dram_tensor("w2a", [128, K * 2 * 128], BF16, kind="ExternalInput")
    b1_in = nc.dram_tensor("b1v", [128, 1], F32, kind="ExternalInput")
    b2_in = nc.dram_tensor("b2v", [128, 2], F32, kind="ExternalInput")
    fc1w_in = nc.dram_tensor("fc1w", [FBLK, D], BF16, kind="ExternalInput")
    fc1b_in = nc.dram_tensor("fc1b", [B, D], F32, kind="ExternalInput")
    fc2w_in = nc.dram_tensor("fc2w", [D, C], BF16, kind="ExternalInput")
    fc2b_in = nc.dram_tensor("fc2b", [B, C], F32, kind="ExternalInput")

    out_t = nc.dram_tensor("out", [B, C], F32, kind="ExternalOutput")
    if dbg:
        h1_dbg = nc.dram_tensor("h1_dbg", [512, NBLK], F32, kind="ExternalOutput")
        l2i_dbg = nc.dram_tensor("l2i_dbg", [N2, C2], F32, kind="ExternalOutput")
        h2_dbg = nc.dram_tensor("h2_dbg", [256, N2], F32, kind="ExternalOutput")
        z_dbg = nc.dram_tensor("z_dbg", [B, D], F32, kind="ExternalOutput")

    ccp_in = nc.dram_tensor("ccp_in", [NCORES * P2BLK, 2 * G1], BF16)
    ccp_out = nc.dram_tensor("ccp_out", [NCORES * P2BLK, 2 * G1], BF16)
    cch_in = nc.dram_tensor("cch_in", [N2 * G2, B2], BF16)
    cch_out = nc.dram_tensor("cch_out", [N2 * G2, B2], BF16)
    ccz_in = nc.dram_tensor("ccz_in", [B, D], F32)
    ccz_out = nc.dram_tensor("ccz_out", [B, D], F32, addr_space="Shared")

    # arrival semaphores: rsem[d-1] counts pushes from the peer at
    # XOR-distance d (+2 per 8-slot broadcast arrival, FIFO per peer).
    rsem = [nc.alloc_semaphore(f"rsem{d}") for d in (1, 2, 3)]
    lsem = nc.alloc_semaphore("lsem")
    patches = []

    with TileContext(nc) as tc:
        with tc.tile_pool(name="const", bufs=1) as cpool:
            ident = cpool.tile([128, 128], F32)
            make_identity(nc, ident[:])
            identb = cpool.tile([128, 128], BF16)
            nc.vector.tensor_copy(identb[:], ident[:])
            # (-2I), (-1I), (-3I) in bf16: Chebyshev corrections run on the
            # PE as extra contraction tiles (exact small-int coefficients).
            nid2 = cpool.tile([128, 128], BF16)
            nc.vector.tensor_scalar_mul(nid2[:], identb[:], -2.0)
            nid1 = cpool.tile([128, 128], BF16)
            nc.vector.tensor_scalar_mul(nid1[:], identb[:], -1.0)
            nid3 = cpool.tile([128, 128], BF16)
            nc.vector.tensor_scalar_mul(nid3[:], identb[:], -3.0)

            # NOTE: no manual sem_clear here - the preamble's per-kernel
            # sem_clear zeroes all Bass-managed sems BEFORE the prelude
            # AllGather, so peer pushes can never race a clear.
            bar = nc.gpsimd.bir_kernel_barrier_wait(replica_groups=G8)
            bar_wait = bar.ins.sync_info.on_wait[0]
            patches.append((bar_wait, bar_wait.wait_value))
            bar_wait.wait_value = 0

            # ======================= LAYER 1 =======================
            with tc.tile_pool(name="l1", bufs=1) as l1, \
                 tc.tile_pool(name="l1g", bufs=32) as l1g, \
                 tc.tile_pool(name="l1a", bufs=6) as l1a, \
                 tc.tile_pool(name="l1cm", bufs=3) as l1cm, \
                 tc.tile_pool(name="ps_y", bufs=1, space="PSUM") as ps_y, \
                 tc.tile_pool(name="ps_tr", bufs=2, space="PSUM") as ps_tr, \
                 tc.tile_pool(name="ps_ct", bufs=2, space="PSUM") as ps_ct:

                m1t = l1.tile([128, NT0, NBLK], BF16)
                w1a = l1.tile([128, K, 2, 128], BF16)
                nc.sync.dma_start(w1a[:], w1_in.ap().rearrange("p (k h c) -> p k h c", k=K, h=2))
                h1_sb = l1.tile([128, 4, NBLK], F32)
                nc.any.memset(h1_sb[:], 0.0)

                # gather buffers: [chain][parity] -> [128, 4 slots, 8 nt, C1]
                # slot 0 = own block (local bf16 copy), slot d = XOR-peer d.
                gb = [[l1.tile([128, NB, NTL, C1], BF16, tag=f"gb{c}{q}",
                               name=f"gb{c}{q}")
                       for q in range(2)] for c in range(2)]

                # own x~ block, bf16 (slot-0 image of x_nm)
                tx0 = l1.tile([128, NTL, C1], BF16)
                nc.sync.dma_start(
                    tx0[:],
                    x_nm_in.ap().rearrange("(kt p) c -> p kt c", p=128)[:, 0:NTL])

                def l1_contract(src, kk):
                    # src: [128, NTL, C1] bf16 state -> cm via PE transpose
                    # (bf16 in/out, 1 cyc/row; no cross-engine stall).
                    cm = l1cm.tile([128, 2, NBLK], BF16, tag="cm", name=f"cm{kk}")
                    for cc in range(2):
                        for nt in range(NTL):
                            trt = ps_tr.tile([128, 128], BF16, tag="tr",
                                             name=f"tr{kk}_{cc}_{nt}")
                            nc.tensor.transpose(
                                trt[:], src[:, nt, 128 * cc:128 * (cc + 1)],
                                identb[:])
                            nc.any.tensor_copy(
                                out=cm[:, cc, 128 * nt:128 * (nt + 1)],
                                in_=trt[:])
                    for cc in range(2):
                        for h in range(2):
                            for ch in range(2):
                                cps = ps_ct.tile([128, 512], F32, tag="ct",
                                                 name=f"ct{kk}_{cc}_{h}_{ch}")
                                nc.tensor.matmul(
                                    cps[:], w1a[:, kk, h, :],
                                    cm[:, cc, 512 * ch:512 * (ch + 1)],
                                    start=True, stop=True)
                                nc.vector.tensor_tensor(
                                    h1_sb[:, 2 * cc + h, 512 * ch:512 * (ch + 1)],
                                    h1_sb[:, 2 * cc + h, 512 * ch:512 * (ch + 1)],
                                    cps[:], ALU.add)

                l1_contract(tx0, 0)

                xtiles = {}  # x~ rhs tiles cached across the k<=2 passes
                for k in range(1, K):
                    cq = (k % 2, (k // 2) % 2)
                    gdst = gb[cq[0]][cq[1]]

                    # Chebyshev corrections (-2tx_{k-2}, -tx_{k-4}, -3tx_1,
                    # -x~) are folded into the PE accumulation groups as
                    # identity matmuls; the recurrence is one PSUM->bf16
                    # copy into the gather buffer's own slot.
                    def do_recur(ot, yap, k=k, gdst=gdst):
                        if k <= 2:
                            nc.vector.tensor_scalar_mul(
                                gdst[:, 0, ot, :], yap, 0.5)
                        elif ot % 2:
                            # odd tiles via nc.any (usually ACT): frees the
                            # shared PSUM banks in parallel with DVE.
                            nc.any.tensor_copy(out=gdst[:, 0, ot, :], in_=yap)
                        else:
                            nc.vector.tensor_copy(gdst[:, 0, ot, :], yap)

                    if k == 2:
                        # m1t is first needed here; deferring + chunking the
                        # 8MB load keeps the k=1 streams off the DMA queues.
                        m1v = m1t_in.ap().rearrange("(t p) n -> p t n", p=128)
                        for mc in range(8):
                            nc.sync.dma_start(
                                m1t[:, 4 * mc:4 * (mc + 1), :],
                                m1v[:, 4 * mc:4 * (mc + 1), :])
                    if k <= 2:
                        # streamed rhs (x~): kt-outer needs bank-aligned
                        # accumulation groups -> two half-passes of 4 out
                        # tiles padded to one bank each.
                        for oh in range(2):
                            yp4 = ps_y.tile([128, 4, 512], F32, tag="y",
                                            name=f"y{k}_{oh}")
                            for kt in range(NT0):
                                if kt in xtiles:
                                    rhs = xtiles[kt]
                                else:
                                    rhs = l1g.tile([128, C1], BF16, tag="gkt",
                                                   name=f"g{kt}")
                                    nc.sync.dma_start(
                                        rhs[:],
                                        x_nm_in.ap().rearrange(
                                            "(t p) c -> t p c", p=128)[kt])
                                    xtiles[kt] = rhs
                                if k == 1:
                                    # stream only this half-pass's columns
                                    op = l1a.tile([128, 512], BF16, tag="aop",
                                                  name=f"a{oh}_{kt}")
                                    nc.sync.dma_start(
                                        op[:], a1t_in.ap().rearrange(
                                            "(t p) n -> t p n", p=128)[kt][
                                            :, 512 * oh:512 * (oh + 1)])
                                for j in range(4):
                                    oi = 4 * oh + j
                                    opv = (op[:, 128 * j:128 * (j + 1)]
                                           if k == 1 else
                                           m1t[:, kt, 128 * oi:128 * (oi + 1)])
                                    last = (kt == NT0 - 1) and k == 1
                                    nc.tensor.matmul(
                                        yp4[:, j, 0:C1],
                                        opv,
                                        rhs[:],
                                        start=(kt == 0), stop=last)
                            for j in range(4):
                                oi = 4 * oh + j
                                if k == 2:  # tx_2 = 0.5(M x~ - 2 x~)
                                    nc.tensor.matmul(
                                        yp4[:, j, 0:C1], nid2[:],
                                        tx0[:, oi, :],
                                        start=False, stop=True)
                                do_recur(oi, yp4[:, j, 0:C1])
                    else:
                        # SBUF gather source: oi-outer so each PSUM
                        # accumulation group completes before the next
                        # starts (groups share banks at 1KB offsets).
                        gsrc = gb[(k - 2) % 2][((k - 2) // 2) % 2]
                        yp = ps_y.tile([128, NTL, C1], F32, tag="y",
                                       name=f"y{k}")
                        kwaits = []
                        for oi in range(NTL):
                            for kt in range(NT0):
                                mm = nc.tensor.matmul(
                                    yp[:, oi, :],
                                    m1t[:, kt, 128 * oi:128 * (oi + 1)],
                                    gsrc[:, kt // NTL, kt % NTL, :],
                                    start=(kt == 0), stop=False)
                                if oi == 0 and kt == NTL - 1:
                                    # arrival waits anchored after the
                                    # own-slot tiles of the first group so
                                    # the scheduler cannot hoist them ahead
                                    # of the sends peers depend on.
                                    for d in (1, 2, 3):
                                        w = nc.tensor.wait_ge(rsem[d - 1], 0)
                                        patches.append(
                                            (w.ins.sync_info.on_wait[0],
                                             2 * (k - 2)))
                                        add_dep_helper(
                                            w.ins, mm.ins,
                                            reason="wait after own tiles")
                                        kwaits.append(w)
                                if kt % NTL == 0 and kt > 0:
                                    add_dep_helper(
                                        mm.ins, kwaits[kt // NTL - 1].ins,
                                        reason="gather arrival")
                            # fold the Chebyshev corrections into the group
                            if k == 3:  # tx_3 = M tx_1 - 3 tx_1
                                nc.tensor.matmul(
                                    yp[:, oi, :], nid3[:],
                                    gsrc[:, 0, oi, :],
                                    start=False, stop=True)
                            else:       # tx_k = M tx_{k-2} -2tx_{k-2} -tx_{k-4}
                                nc.tensor.matmul(
                                    yp[:, oi, :], nid2[:],
                                    gsrc[:, 0, oi, :],
                                    start=False, stop=False)
                                p4 = (tx0[:, oi, :] if k == 4
                                      else gdst[:, 0, oi, :])
                                nc.tensor.matmul(
                                    yp[:, oi, :], nid1[:], p4,
                                    start=False, stop=True)
                        # copies batched after all groups: the PE runs the 8
                        # groups back-to-back without PSUM bank locks against
                        # the DVE reads.
                        for oi in range(NTL):
                            do_recur(oi, yp[:, oi, :])

                    # push own block to the 3 XOR-peers' matching slots
                    if k <= K - 3:
                        for d in (1, 2, 3):
                            rd = [None] * 8
                            rd[d] = (0, d)
                            prep = nc.gpsimd.remote_dma_broadcast(
                                gdst[:, d, :, :], gdst[:, 0, :, :],
                                remote_sem=rsem[d - 1], local_sem=lsem,
                                rdests=rd)
                            add_dep_helper(prep.ins, bar.ins,
                                           reason="send after barrier")
                        nc.gpsimd.trigger_dma(count=None)

                    l1_contract(gdst[:, 0, :, :], k)

                # bias + relu + maxpool4 along nodes
                b1v = l1.tile([128, 1], F32)
                nc.sync.dma_start(b1v[:], b1_in.ap())
                h1p = l1.tile([128, 4, P2BLK], F32)
                for cht in range(4):
                    nc.scalar.activation(h1_sb[:, cht, :], h1_sb[:, cht, :], ACT.Relu,
                                         bias=b1v[:])
                    h4 = h1_sb[:, cht, :].rearrange("p (n f) -> p n f", f=4)
                    nc.vector.tensor_tensor(h1p[:, cht, :], h4[:, :, 0], h4[:, :, 1],
                                            ALU.max)
                    nc.vector.tensor_tensor(h1p[:, cht, :], h1p[:, cht, :], h4[:, :, 2],
                                            ALU.max)
                    nc.vector.tensor_tensor(h1p[:, cht, :], h1p[:, cht, :], h4[:, :, 3],
                                            ALU.max)
                if dbg:
                    nc.sync.dma_start(
                        h1_dbg.ap().rearrange("(t p) n -> p t n", p=128), h1_sb[:])

                # transpose pooled block -> [n2_local, (b_loc, g)] bf16
                h1pt = l1.tile([128, P2BLK // 128, BL * G1], BF16)
                for cht in range(4):
                    for nt in range(P2BLK // 128):
                        trp = ps_tr.tile([128, 128], F32, tag="tr")
                        nc.tensor.transpose(
                            trp[:], h1p[:, cht, 128 * nt:128 * (nt + 1)], ident[:])
                        nc.any.tensor_copy(
                            out=h1pt[:, nt, 128 * cht:128 * (cht + 1)], in_=trp[:])

                ccp_iv = ccp_in.ap().rearrange("(s t p) c -> s p t c", p=128,
                                               t=P2BLK // 128)
                for s in range(NCORES):
                    nc.sync.dma_start(ccp_iv[s],
                                      h1pt[:, :, 64 * s:64 * (s + 1)])
                nc.gpsimd.collective_compute(
                    "AllToAll", ALU.bypass, replica_groups=G8,
                    ins=[ccp_in.ap()], outs=[ccp_out.ap()])

            # ======================= LAYER 2 =======================
            # ccp_out rows: src_rank * P2BLK + n2l, src_rank = bh*4 + nb;
            # cols: (b_pair 2, g 32). My batches (c2 order): b = bh*2 + pair.
            with tc.tile_pool(name="l2", bufs=1) as l2, \
                 tc.tile_pool(name="l2bf", bufs=3) as l2bf, \
                 tc.tile_pool(name="l2cm", bufs=2) as l2cm, \
                 tc.tile_pool(name="ps2_y", bufs=2, space="PSUM") as ps2_y, \
                 tc.tile_pool(name="ps2_tr", bufs=2, space="PSUM") as ps2_tr, \
                 tc.tile_pool(name="ps2_ct", bufs=2, space="PSUM") as ps2_ct:

                a2t = l2.tile([128, N2 // 128, N2], BF16)
                nc.sync.dma_start(a2t[:], a2t_in.ap().rearrange("(t p) n -> p t n", p=128))
                w2a = l2.tile([128, K, 2, 128], BF16)
                nc.sync.dma_start(
                    w2a[:], w2_in.ap().rearrange("p (k h g) -> p k h g", k=K, h=2))

                # init state: [128 n2, 8 nt, (b 4, g 32)] from ccp_out
                st0_bf = l2bf.tile([128, N2 // 128, C2], BF16, tag="st2bf")
                ccp_v = ccp_out.ap().rearrange(
                    "(bh nb t p) c -> bh nb p t c", bh=2, nb=NB, t=P2BLK // 128)
                for bh in range(2):
                    for nb in range(NB):
                        # dest cols [bh*64, +64) = (b = bh*2 + pair, g)
                        nc.sync.dma_start(
                            st0_bf[:, 2 * nb:2 * (nb + 1),
                                   64 * bh:64 * (bh + 1)],
                            ccp_v[bh, nb])
                if dbg:
                    st0d = l2.tile([128, N2 // 128, C2], F32)
                    nc.vector.tensor_copy(st0d[:], st0_bf[:])
                    nc.sync.dma_start(
                        l2i_dbg.ap().rearrange("(t p) c -> p t c", p=128),
                        st0d[:])

                h2a = l2.tile([128, 2, N2], F32)
                nc.any.memset(h2a[:], 0.0)

                def l2_contract(src_bf, kk):
                    # src_bf: [128, 8, C2] bf16 state
                    cm = l2cm.tile([128, N2], BF16, tag="cm2")
                    for nt in range(N2 // 128):
                        trp = ps2_tr.tile([128, 128], BF16, tag="tr2")
                        nc.tensor.transpose(trp[:], src_bf[:, nt, :], identb[:])
                        nc.any.tensor_copy(
                            out=cm[:, 128 * nt:128 * (nt + 1)], in_=trp[:])
                    for hh in range(2):
                        cps = ps2_ct.tile([128, N2], F32, tag="ct2")
                        for ch in range(N2 // 512):
                            nc.tensor.matmul(
                                cps[:, 512 * ch:512 * (ch + 1)],
                                w2a[:, kk, hh, :],
                                cm[:, 512 * ch:512 * (ch + 1)],
                                start=True, stop=True)
                        nc.vector.tensor_tensor(h2a[:, hh, :], h2a[:, hh, :],
                                                cps[:], ALU.add)

                l2_contract(st0_bf, 0)
                bf2 = {0: st0_bf}
                for k in range(1, K):
                    g2bf = l2bf.tile([128, N2 // 128, C2], BF16, tag="st2bf")
                    bf2[k] = g2bf
                    gath2 = bf2[k - 1]
                    yps = []
                    for g in range(2):
                        yp = ps2_y.tile([128, 4, 128], F32, tag="y2")
                        yps.append(yp)
                        for oi in range(4):
                            ot = 4 * g + oi
                            for kt in range(N2 // 128):
                                nc.tensor.matmul(
                                    yp[:, oi, :],
                                    a2t[:, kt, 128 * ot:128 * (ot + 1)],
                                    gath2[:, kt, :],
                                    start=(kt == 0),
                                    stop=(k == 1 and kt == N2 // 128 - 1))
                            if k >= 2:
                                # fold -tx_{k-2} into the group
                                nc.tensor.matmul(
                                    yp[:, oi, :], nid1[:],
                                    bf2[k - 2][:, ot, :],
                                    start=False, stop=True)
                    for ot in range(8):
                        yap = yps[ot // 4][:, ot % 4, :]
                        if k == 1:
                            nc.vector.tensor_scalar_mul(g2bf[:, ot, :], yap, 0.5)
                        elif ot % 2:
                            nc.any.tensor_copy(out=g2bf[:, ot, :], in_=yap)
                        else:
                            nc.vector.tensor_copy(g2bf[:, ot, :], yap)
                    l2_contract(g2bf, k)
                    bf2.pop(k - 2, None)

                # bias + relu, then transpose h2 -> [n2, (b, g2)] bf16
                b2v = l2.tile([128, 2], F32)
                nc.sync.dma_start(b2v[:], b2_in.ap())
                h2r = l2.tile([128, 2, N2], F32)
                for hh in range(2):
                    nc.scalar.activation(h2r[:, hh, :], h2a[:, hh, :], ACT.Relu,
                                         bias=b2v[:, hh:hh + 1])
                if dbg:
                    nc.sync.dma_start(
                        h2_dbg.ap().rearrange("(t p) n -> p t n", p=128), h2r[:])
                # build f-major features: ft_sb[n2_l, nt, (g2 64, b 4)]
                ft_sb = l2.tile([128, N2 // 128, G2 * B2], BF16)
                for hh in range(2):
                    for nt in range(N2 // 128):
                        trp = ps2_tr.tile([128, 128], F32, tag="tr2")
                        nc.tensor.transpose(trp[:], h2r[:, hh, 128 * nt:128 * (nt + 1)],
                                            ident[:])
                        # cols of trp: (b 4, g2r 32) -> dest (g2 = hh*32+g2r, b)
                        nc.any.tensor_copy(
                            out=ft_sb[:, nt, :].rearrange("p (g b) -> p g b", g=G2)[
                                :, 32 * hh:32 * (hh + 1), :],
                            in_=trp[:].rearrange("p (b g) -> p g b", b=4))
                # AllToAll: slot j = my rows f in [FBLK*j, FBLK*(j+1))
                # cch_in rows (j, n2_l 128, g2 64), cols b
                nc.sync.dma_start(
                    cch_in.ap().rearrange("(j nl g) b -> nl j (g b)",
                                          j=NCORES, nl=128),
                    ft_sb[:])
                nc.gpsimd.collective_compute(
                    "AllToAll", ALU.bypass, replica_groups=G8,
                    ins=[cch_in.ap()], outs=[cch_out.ap()])

            # ======================= HEAD =======================
            with tc.tile_pool(name="fc", bufs=1) as fc, \
                 tc.tile_pool(name="fcw", bufs=6) as fcw, \
                 tc.tile_pool(name="ps3", bufs=2, space="PSUM") as ps3, \
                 tc.tile_pool(name="ps3z", bufs=1, space="PSUM") as ps3z:

                # flatT: [128 p, 8 r, 64 kt, 4 b]; cch rows (r, p, kt) so each
                # per-r DMA is 512B-contiguous per partition. flt[p, r, kt, b]
                # holds flat[b(r,b), f = p*64 + kt]; fc1w is host-permuted to
                # match (row kt*128+p = local f p*64+kt).
                NKT = FBLK // 128
                flt = fc.tile([128, NCORES, NKT, B2], BF16, tag="flt")
                flt2 = fc.tile([128, NKT, NCORES, B2], BF16, tag="flt2")
                zps = ps3z.tile([32, D], F32)
                cch_v = cch_out.ap().rearrange(
                    "(r p kt) b -> r p kt b", r=NCORES, p=128)
                for r in range(NCORES):
                    nc.sync.dma_start(flt[:, r, :, :], cch_v[r])
                nc.vector.tensor_copy(
                    flt2[:], flt[:].rearrange("p r kt b -> p kt r b"))
                fc1v = fc1w_in.ap().rearrange("(c kt p) d -> c p kt d",
                                              p=128, kt=8)
                for cb in range(NKT // 8):
                    fw = fcw.tile([128, 8, D], BF16, tag="fw")
                    nc.sync.dma_start(fw[:], fc1v[cb])
                    for j in range(8):
                        kt = 8 * cb + j
                        nc.tensor.matmul(
                            zps[:],
                            flt2[:, kt, :, :].rearrange("p r b -> p (r b)"),
                            fw[:, j, :],
                            start=(kt == 0), stop=(kt == NKT - 1))
                zblk = fc.tile([32, D], F32)
                nc.vector.tensor_copy(zblk[:], zps[:])
                nc.sync.dma_start(ccz_in.ap(), zblk[:])
                nc.gpsimd.collective_compute(
                    "AllReduce", ALU.add, replica_groups=G8,
                    ins=[ccz_in.ap()], outs=[ccz_out.ap()])
                zfull = fc.tile([32, D], F32)
                nc.sync.dma_start(zfull[:], ccz_out.ap())
                zb = fc.tile([32, D], F32)
                nc.sync.dma_start(zb[:], fc1b_in.ap())
                nc.vector.tensor_tensor(zfull[:], zfull[:], zb[:], ALU.add)
                zr = fc.tile([32, D], F32)
                nc.scalar.activation(zr[:], zfull[:], ACT.Relu)
                if dbg:
                    nc.sync.dma_start(z_dbg.ap(), zr[:])

                # fc2: transpose z, then [32, 10] = sum_kt zT[kt].T @ fc2w[kt]
                f2w = fc.tile([128, 4, C], BF16)
                nc.sync.dma_start(f2w[:],
                                  fc2w_in.ap().rearrange("(t p) c -> p t c", p=128))
                lps = ps3.tile([32, C], F32, tag="lg")
                for t4 in range(4):
                    ztp = ps3.tile([128, 32], F32, tag="zt")
                    nc.tensor.transpose(ztp[:], zr[:, 128 * t4:128 * (t4 + 1)],
                                        ident[:32, :32])
                    zts = fc.tile([128, 32], BF16, tag="zts")
                    nc.any.tensor_copy(out=zts[:], in_=ztp[:])
                    nc.tensor.matmul(lps[:], zts[:], f2w[:, t4, :],
                                     start=(t4 == 0), stop=(t4 == 3))
                logits = fc.tile([32, C], F32)
                f2b = fc.tile([32, C], F32)
                nc.sync.dma_start(f2b[:], fc2b_in.ap())
                nc.vector.tensor_tensor(logits[:], lps[:], f2b[:], ALU.add)

                mx = fc.tile([32, 1], F32)
                nc.vector.tensor_reduce(mx[:], logits[:], axis=AX.X, op=ALU.max)
                sh = fc.tile([32, C], F32)
                nc.vector.tensor_tensor(sh[:], logits[:], mx[:].to_broadcast((32, C)),
                                        ALU.subtract)
                ex = fc.tile([32, C], F32)
                nc.scalar.activation(ex[:], sh[:], ACT.Exp)
                sm = fc.tile([32, 1], F32)
                nc.vector.tensor_reduce(sm[:], ex[:], axis=AX.X, op=ALU.add)
                lg = fc.tile([32, 1], F32)
                nc.scalar.activation(lg[:], sm[:], ACT.Ln)
                res = fc.tile([32, C], F32)
                nc.vector.tensor_tensor(res[:], sh[:], lg[:].to_broadcast((32, C)),
                                        ALU.subtract)
                nc.sync.dma_start(out_t.ap(), res[:])

    # restore the real wait thresholds the scheduling sim couldn't model
    for wait_obj, val in patches:
        wait_obj.wait_value = val
    nc.compile()
    return nc


def make_inputs(x, edge_index0, edge_index2, W1, b1, W2, b2,
                fc1_w, fc1_b, fc2_w, fc2_b):
    """Build the 8 per-core input maps."""
    A0 = _dense_adj(np.asarray(edge_index0), N0)
    A2 = _dense_adj(np.asarray(edge_index2), N2)
    A1T2 = (2.0 * A0).T.astype(np.float32)     # [N0, N0] cols -> row blocks
    M1T = (4.0 * (A0 @ A0)).T.astype(np.float32)
    A2T2 = _b16((2.0 * A2).T)                  # [N2, N2]

    U, Vm = _fold_uv()
    W1f = np.asarray(W1, np.float32)
    # W~[k] = Vm @ W1[k]  [16, G1]
    W1t = np.einsum("jf,kfg->kjg", Vm, W1f)
    # block-diag pack: rows 64h+16i..+16, cols 32i..+32 = W~[k]
    w1a = np.zeros((128, K, 2, 128), np.float32)
    for h in range(2):
        for i in range(4):
            w1a[64 * h + 16 * i:64 * h + 16 * (i + 1), :, h,
                32 * i:32 * (i + 1)] = W1t.transpose(1, 0, 2)
    w1a = _b16(w1a.reshape(128, K * 2 * 128))

    W2f = np.asarray(W2, np.float32)       # [K, G1, G2]
    # block-diag pack: rows 32i..+32, cols 32i..+32 = W2[k][:, hh half]
    w2a = np.zeros((128, K, 2, 128), np.float32)
    for bb in range(4):
        for hh in range(2):
            w2a[32 * bb:32 * bb + 32, :, hh, 32 * bb:32 * bb + 32] = \
                W2f[:, :, 32 * hh:32 * hh + 32].transpose(1, 0, 2)
    w2a = _b16(w2a.reshape(128, K * 2 * 128))

    b1v = np.tile(np.asarray(b1, np.float32), 4).reshape(128, 1)
    b2f = np.asarray(b2, np.float32)
    b2v = np.stack([np.tile(b2f[:32], 4), np.tile(b2f[32:], 4)], 1).astype(np.float32)

    fc1b = np.tile(np.asarray(fc1_b, np.float32)[None, :], (B, 1))
    fc2b = np.tile(np.asarray(fc2_b, np.float32)[None, :], (B, 1))
    fc2w = _b16(np.asarray(fc2_w, np.float32))

    # fc1w row permutation: stored row kt*128+p holds local f = p*64+kt
    NKT = FBLK // 128
    kt_a = np.arange(NKT)
    fperm = (np.arange(128)[None, :] * NKT + kt_a[:, None]).reshape(-1)

    xt = np.einsum("bnt,tj->bnj", np.asarray(x, np.float32), U)  # [B, N0, 16]
    fc1wf = np.asarray(fc1_w, np.float32)   # [N2*G2, D]

    # stored-row -> node permutation per core: row kt*128+p holds node
    # (nb^ (kt//8))*1024 + (kt%8)*128 + p
    kt_i = np.arange(NT0)
    p_i = np.arange(128)
    ins = []
    for core in range(NCORES):
        bh, nb = core // 4, core % 4
        slot_rank = (nb ^ (kt_i // NTL))
        node_idx = (slot_rank[:, None] * NBLK
                    + (kt_i % NTL)[:, None] * 128 + p_i[None, :]).reshape(-1)
        xs = xt[16 * bh:16 * (bh + 1)]          # [16, N0, 16]
        x_all = np.ascontiguousarray(
            xs.transpose(1, 0, 2).reshape(N0, C1))  # c = b_loc*16 + t
        ins.append({
            "a1t": _b16(A1T2[node_idx][:, NBLK * nb:NBLK * (nb + 1)]),
            "m1t": _b16(M1T[node_idx][:, NBLK * nb:NBLK * (nb + 1)]),
            "a2t": A2T2,
            "x_nm": _b16(x_all[node_idx]),
            "w1a": w1a, "w2a": w2a, "b1v": b1v, "b2v": b2v,
            "fc1w": _b16(fc1wf[FBLK * core:FBLK * (core + 1), :][fperm]),
            "fc1b": fc1b, "fc2b": fc2b, "fc2w": fc2w,
        })
    return ins


def batch_perm():
    """flat row order (r, b_c2) -> global batch id."""
    perm = []
    for r in range(NCORES):
        for b_c2 in range(4):
            bh, pair = b_c2 // 2, b_c2 % 2
            perm.append(16 * bh + 2 * r + pair)
    return np.array(perm)


_CACHED = {}


def kernel(**inputs):
    if "nc" not in _CACHED:
        _CACHED["nc"] = build_program(dbg=False)
    nc = _CACHED["nc"]
    ins = make_inputs(**inputs)
    res = run_bass_kernel_spmd(nc, ins, core_ids=list(range(NCORES)))
    out = np.zeros((B, C), np.float32)
    out[batch_perm()] = res.results[0]["out"]
    return out
